# revision 35
# baseline (speedup 1.0000x reference)
"""Trainium2 Bass kernel for nn_AutoregressiveInstructionHead.

Data-parallel over batch B=256 across 8 NeuronCores (32 rows each).
Head weights / embeddings / action-derived tables are replicated.

Per-core device pipeline (all heavy compute on device):
  - constants packed into a few DRAM tensors -> few big DMAs ordered by
    first consumer (HWDGE per-DMA overhead is ~625ns, so fewer transfers
    and compute starts ~3.5us in instead of ~18us)
  - fp_head = features @ W1_feat.T (+b1)  -> [H=128, B=32] via PE
  - ep tables = embeddings @ W1_emb.T     -> [H=128, A] via PE
  - imm head (NI=2): logp = m*X - softplus(X) with X = l0 - l1, so only a
    single M=1 matmul strip per (b, 512-col block) with w_d = W2[0]-W2[1];
    the strip-scattered psum rows are gathered to a compact X32[32,A] tile
    by a psum->SBUF DMA (free on the DMA engines), then one softplus + two
    DVE ops finish the head.  No exp / mask / su-sel matmuls needed.
  - op head: logits -> exp/sum/ln -> gather via one-hot matmul into PSUM acc
  - rs head: deduplicated over the 65 opcodes ([B,65,17] table), gathered
    back to the 1024 actions with block-sparse one-hot matmuls (actions are
    host-sorted by (opcode, reg_src) so each table chunk touches a
    contiguous column range; inverse permutation applied on host at the end)
  - rd head: h=relu(fp[b]+ep[a]) [128,1024] per b, logits matmul with
    col-tiling (4 b's concurrently in 32-partition strips), exp(+b2) on ACT,
    one-hot mask multiply, partition sums via indicator matmuls,
    contribution = ln(sum mask*exp) - ln(sum exp)
"""

import sys

for _p in ("/opt/trn_rl_repo",):
    if _p not in sys.path:
        sys.path.insert(0, _p)

import numpy as np
from contextlib import ExitStack

import json

import concourse.bass as bass
import concourse.tile as tile
from concourse import mybir
from concourse import bass2jax as _bass2jax
from concourse.bass_utils import run_bass_kernel_spmd
from concourse.bass_utils import compile_bir_kernel as _orig_compile_bir_kernel

# --- workaround: this container's walrus rejects instructions carrying more
# than one sync-wait command ("Too many sync wait commands"), but Tile's
# scheduler emits multi-wait instructions.  Split them in the serialized BIR
# by inserting wait-only EventSemaphore carriers immediately before, on the
# same engine queue (semantically identical: same queue position, waits
# simply execute as separate instructions).
_WSPLIT_UID = [0]


def _split_bir_waits(bir_json: bytes, maxw: int = 1) -> bytes:
    m = json.loads(bir_json)
    tmpl = None
    for fn in m["functions"]:
        for bb in fn["blocks"]:
            for ins in bb["instructions"]:
                if ins.get("opcode") == "EventSemaphore":
                    tmpl = json.loads(json.dumps(ins))
                    break
            if tmpl:
                break
    if tmpl is None:
        return bir_json
    for fn in m["functions"]:
        for bb in fn["blocks"]:
            out = []
            for ins in bb["instructions"]:
                si = ins.get("sync_info")
                waits = (si or {}).get("on_wait") or []
                if len(waits) > maxw:
                    keep = waits[-maxw:]
                    extra = waits[:-maxw]
                    for i in range(0, len(extra), maxw):
                        _WSPLIT_UID[0] += 1
                        d = json.loads(json.dumps(tmpl))
                        d["name"] = f"WSPLIT-{_WSPLIT_UID[0]}"
                        d["engine"] = ins["engine"]
                        d["ins"] = []
                        d["outs"] = []
                        d["sync_info"] = {
                            "on_wait": extra[i : i + maxw],
                            "on_update": [],
                        }
                        d.pop("debug", None)
                        d.pop("bass_addl_debug", None)
                        out.append(d)
                    si["on_wait"] = keep
                out.append(ins)
            bb["instructions"] = out
    return json.dumps(m).encode()


def _patched_compile_bir_kernel(bir_json, tmpdir, neff_name="file.neff"):
    return _orig_compile_bir_kernel(
        _split_bir_waits(bir_json), tmpdir, neff_name=neff_name
    )


_bass2jax.compile_bir_kernel = _patched_compile_bir_kernel

# dims
B, D, A = 256, 512, 1024
NO, NR, NI, E, H = 65, 17, 2, 64, 128
NCORES = 8
BL = B // NCORES  # 32 batch rows per core

F32 = mybir.dt.float32
BF16 = mybir.dt.bfloat16
AF = mybir.ActivationFunctionType
ALU = mybir.AluOpType

NOP = 68  # rs head padded to a multiple of 4 opcodes
NGRP = NOP // 4  # 17 groups of 4 opcodes (rs head)


def _bf(x):
    import ml_dtypes

    return np.asarray(x, dtype=ml_dtypes.bfloat16)


def _f32(x):
    return np.ascontiguousarray(np.asarray(x, dtype=np.float32))


# ---------------------------------------------------------------------------
# packed-constant layout: chunk -> (partitions, dtype, [(name, rows, cols)])
# one DRAM param + one SBUF tile + one DMA per chunk; DMAs are issued in
# this order (first-consumer order)
_PACKS = [
    ("c_a2", 128, BF16, [("embrd", 2 * E, A), ("wrde", 2 * E, H),
                         ("wime1", 2 * E, H), ("wd32", H, 32)]),
    ("c_c64", 64, BF16, [("wrse", E, H), ("embrs", E, NOP), ("wime2", E, H),
                         ("embim2", E, A)]),
    ("c_feat", 128, BF16, [("featT", 128, 128)]),
    ("c_w1im", 128, BF16, [("w1im", 128, 512)]),
    ("c_f32", 128, F32, [("b1s", H, 4), ("b2op", NO, 1), ("b2rs", H, 1),
                         ("b2rd", H, 1), ("db2im", BL, 1)]),
    ("c_a2r", 128, BF16, [("w2opT", H, NO), ("w2rsT", H, 32),
                          ("w2rdT", H, 32), ("suind_rs", H, 4),
                          ("embrd_u", 2 * E, 704), ("onescol", 128, 1),
                          ("onesrow", 1, 32)]),
    ("c_w1rs", 128, BF16, [("w1rs", 128, 512)]),
    ("c_w1rd", 128, BF16, [("w1rd", 128, 512)]),
    ("c_w1op", 128, BF16, [("w1op", 128, 512)]),
    ("c_b1", 128, BF16, [("gop", NO, A), ("grs", H, A)]),
    ("c_low32", BL, BF16, [("m32", BL, A), ("id32", BL, 32)]),
    ("c_low4", 4, BF16, [("negones", 1, A), ("mdb2", 1, A), ("g2rs", 4, A)]),
    ("c_b2", 128, BF16, [("grd", H, A), ("gpair", H, A)]),
]
_PACK_ENG = {n: (i % 2) for i, (n, _, _, _) in enumerate(_PACKS)}


def _host_prep(inputs):
    """Build all per-core / shared device constants on host (index ops only
    plus dtype packing; all real FLOPs happen on device)."""
    feats = _f32(inputs["features"])
    o = np.clip(inputs["act_o"].astype(np.int64), 0, NO - 1)
    rs = np.clip(inputs["act_rs"].astype(np.int64), 0, NR - 1)
    rd = np.clip(inputs["act_rd"].astype(np.int64), 0, NR - 1)
    im = np.clip(inputs["act_imm"].astype(np.int64), 0, NI - 1)

    perm = np.lexsort((rs, o))  # sort by (opcode, reg_src)
    os_, rss, rds, ims = o[perm], rs[perm], rd[perm], im[perm]

    opcode_embed = _f32(inputs["opcode_embed"])  # [65, 64]
    reg_embed = _f32(inputs["reg_embed"])  # [17, 64]
    op_e = opcode_embed[os_]  # [A, 64] sorted
    rs_e = reg_embed[rss]
    rd_e = reg_embed[rds]

    W = {k: _f32(inputs[k]) for k in inputs if k.endswith(("W1", "W2", "b1", "b2"))}

    c = {}
    # feature-path weights per head: [128, 512] with K-chunk k at cols
    # [128k, 128k+128)
    for nm, wk in (("im", "imm_W1"), ("rs", "rs_W1"), ("rd", "rd_W1"),
                   ("op", "op_W1")):
        wT = W[wk][:, :D].T  # [D, H]
        c["w1" + nm] = np.concatenate(
            [wT[128 * k : 128 * (k + 1), :] for k in range(4)], axis=1
        )  # [128, 512]
    c["b1s"] = _f32(
        np.stack([W["imm_b1"], W["rs_b1"], W["rd_b1"], W["op_b1"]], axis=1)
    )  # [128, 4]

    # embedding-path weights + gathered embeddings (stacked on K)
    c["wrse"] = W["rs_W1"][:, D : D + E].T  # [64, 128]
    embrs = np.zeros((E, NOP), np.float32)
    embrs[:, :NO] = opcode_embed.T
    c["embrs"] = embrs  # [64, 68] all opcodes (padded)
    c["wrde"] = np.concatenate(
        [W["rd_W1"][:, D : D + E].T, W["rd_W1"][:, D + E : D + 2 * E].T], axis=0
    )  # [128, 128]
    c["embrd"] = np.concatenate([op_e.T, rs_e.T], axis=0)  # [128, A]
    c["wime1"] = np.concatenate(
        [W["imm_W1"][:, D : D + E].T, W["imm_W1"][:, D + E : D + 2 * E].T], axis=0
    )  # [128, 128]
    c["wime2"] = W["imm_W1"][:, D + 2 * E :].T  # [64, 128]
    c["embim2"] = rd_e.T  # [64, A]

    # head-2 weights (V padded to 32 with zeros so PSUM pad rows are written)
    c["w2opT"] = W["op_W2"].T  # [128, 65]
    w2rs = np.zeros((H, 32), np.float32)
    w2rs[:, :NR] = W["rs_W2"].T
    c["w2rsT"] = w2rs
    w2rd = np.zeros((H, 32), np.float32)
    w2rd[:, :NR] = W["rd_W2"].T
    c["w2rdT"] = w2rd
    # imm head: difference vector w_d = W2[0] - W2[1], replicated to 32
    # PE columns so every psum row of a strip holds d (no garbage rows)
    c["wd32"] = np.tile((W["imm_W2"][0] - W["imm_W2"][1]).reshape(H, 1), (1, 32))
    db2 = float(W["imm_b2"][0] - W["imm_b2"][1])
    c["db2im"] = np.full((BL, 1), db2, np.float32)

    # biases b2 as per-partition columns
    c["b2op"] = _f32(W["op_b2"][:, None])  # [65, 1]
    for nm, b2 in (("b2rs", W["rs_b2"]), ("b2rd", W["rd_b2"])):
        t = np.zeros((H, 1), np.float32)
        for s in range(4):
            t[32 * s : 32 * s + NR, 0] = b2
        c[nm] = t

    # op-head gather one-hot + misc rows
    m = (ims == 0).astype(np.float32)  # [A] imm-head class-0 selector
    gop = np.zeros((NO, A), np.float32)
    gop[os_, np.arange(A)] = 1.0
    c["gop"] = gop
    c["onescol"] = np.ones((128, 1), np.float32)
    c["onesrow"] = np.ones((1, 32), np.float32)
    c["negones"] = -np.ones((1, A), np.float32)
    c["mdb2"] = (db2 * m)[None, :]  # [1, A]
    c["m32"] = np.broadcast_to(m, (BL, A)).copy()

    # rs-head gather tables (block one-hot; actions sorted by opcode)
    grs = np.zeros((H, A), np.float32)
    grs[(os_ % 4) * 32 + rss, np.arange(A)] = 1.0
    c["grs"] = grs
    g2rs = np.zeros((4, A), np.float32)
    g2rs[os_ % 4, np.arange(A)] = -1.0
    c["g2rs"] = g2rs
    suind_rs = np.zeros((H, 4), np.float32)
    for s in range(4):
        suind_rs[32 * s : 32 * s + NR, s] = 1.0
    c["suind_rs"] = suind_rs

    # rd head deduplicated over distinct (opcode, reg_src) pairs
    pairs_all = os_ * NR + rss                      # non-decreasing (sorted)
    u_pairs, pid = np.unique(pairs_all, return_inverse=True)
    U = len(u_pairs)
    U4 = 704  # fixed pad (U ~ 660-680 for random actions; assert below)
    assert U <= U4, f"U={U} exceeds pad {U4}"
    NPG = U4 // 4  # 176 pair-groups of 4
    o_u = u_pairs // NR
    rs_u = u_pairs % NR
    embrd_u = np.zeros((2 * E, U4), np.float32)
    embrd_u[:E, :U] = opcode_embed[o_u].T
    embrd_u[E:, :U] = reg_embed[rs_u].T
    c["embrd_u"] = embrd_u
    # grd: one-hot [(strip, v), a] selecting (pid(a)%4, rd(a))
    grd = np.zeros((H, A), np.float32)
    grd[32 * (pid % 4) + rds, np.arange(A)] = 1.0
    c["grd"] = grd
    # gpair: -1 one-hot [pid%128, a] for the lnsu gather (per 128-pair chunk)
    gpair = np.zeros((H, A), np.float32)
    gpair[pid % H, np.arange(A)] = -1.0
    c["gpair"] = gpair
    c["id32"] = np.eye(BL, 32, dtype=np.float32)

    # rs gather chunk column ranges (static, baked into program; identical
    # on every core since actions are replicated)
    bounds = np.searchsorted(os_, np.arange(0, NO + 4, 4)[: NGRP + 1])
    chunks = []
    for g in range(NGRP):
        lo, hi = int(bounds[g]), int(bounds[g + 1])
        while lo < hi:
            nxt = min(hi, ((lo // 512) + 1) * 512, lo + 512)
            chunks.append((g, lo, nxt))
            lo = nxt
    # rd-head gather chunk ranges: per pair-group (grd/q gather) and per
    # 128-pair chunk (lnsu gather), split at psum bank boundaries
    pgrp_a = pid // 4
    qchunks = []
    for pg in range(int(pgrp_a.max()) + 1):
        lo = int(np.searchsorted(pgrp_a, pg))
        hi = int(np.searchsorted(pgrp_a, pg + 1))
        while lo < hi:
            nxt = min(hi, ((lo // 512) + 1) * 512)
            qchunks.append((pg, lo, nxt))
            lo = nxt
    lchunks = []
    for ch in range((U + H - 1) // H):
        lo = int(np.searchsorted(pid, H * ch))
        hi = int(np.searchsorted(pid, H * (ch + 1)))
        while lo < hi:
            nxt = min(hi, ((lo // 512) + 1) * 512)
            lchunks.append((ch, lo, nxt))
            lo = nxt
    feat_T = feats.T  # [D, B]
    per_core = []
    for cid in range(NCORES):
        ft = feat_T[:, cid * BL : (cid + 1) * BL]  # [512, 32]
        ftp = np.concatenate(
            [ft[128 * k : 128 * (k + 1), :] for k in range(4)], axis=1
        )  # [128, 128]
        per_core.append({"featT": ftp})

    # assemble packed chunk arrays
    packed = {}
    for chunk, parts, dt, entries in _PACKS:
        ncols = sum(e[2] for e in entries)
        arr = np.zeros((parts, ncols), np.float32)
        off = 0
        for name, rows, cols in entries:
            if name != "featT":
                arr[:rows, off : off + cols] = c[name]
            off += cols
        packed[chunk] = arr if dt == F32 else _bf(arr)
    return packed, per_core, (tuple(chunks), tuple(qchunks), tuple(lchunks)), perm


def build_program(allchunks):
    chunks, qchunks, lchunks = allchunks
    nc = bass.Bass()
    dr = {}
    for chunk, parts, dt, entries in _PACKS:
        ncols = sum(e[2] for e in entries)
        dr[chunk] = nc.declare_dram_parameter(chunk, [parts, ncols], dt,
                                              isOutput=False)
    out_d = nc.declare_dram_parameter("out", [BL, A], F32, isOutput=True)

    def MM(*a, **k):
        k.setdefault("skip_group_check", True)
        return nc.tensor.matmul(*a, **k)

    with ExitStack() as ctx:
        tc = ctx.enter_context(tile.TileContext(nc))
        cp = ctx.enter_context(tc.tile_pool(name="consts", bufs=1))
        sb = ctx.enter_context(tc.tile_pool(name="sbuf", bufs=2))
        hb = ctx.enter_context(tc.tile_pool(name="hbuf", bufs=8))
        eb = ctx.enter_context(tc.tile_pool(name="ebuf", bufs=3))
        pA = ctx.enter_context(tc.tile_pool(name="pA", bufs=1, space="PSUM"))
        pB = ctx.enter_context(tc.tile_pool(name="pB", bufs=1, space="PSUM"))
        pG = ctx.enter_context(tc.tile_pool(name="pG", bufs=2, space="PSUM"))

        # ---- load packed constants: one tile + one DMA per chunk, in
        # first-consumer order; ct[] = slices into the chunk tiles
        dma_engs = [nc.sync, nc.gpsimd]
        ct = {}
        for chunk, parts, dt, entries in _PACKS:
            ncols = sum(e[2] for e in entries)
            t = cp.tile([parts, ncols], dt, tag=chunk, name=chunk)
            dma_engs[_PACK_ENG[chunk]].dma_start(t[:, :], dr[chunk][:, :])
            off = 0
            for name, rows, cols in entries:
                ct[name] = t[0:rows, off : off + cols]
                off += cols

        # ---- ep tables (embedding partials) on PE; psum->sbuf copies on ACT
        ep_im = eb.tile([H, A], BF16, tag="ep_im")
        for j in range(2):
            pe_h = pG.tile([H, 512], F32, tag="lgh", name=f"pei{j}")
            MM(pe_h[:, :], ct["wime1"][:, :], ct["embrd"][:, 512 * j : 512 * (j + 1)],
               start=True, stop=False)
            MM(pe_h[:, :], ct["wime2"][:, :], ct["embim2"][:, 512 * j : 512 * (j + 1)],
               start=False, stop=True)
            nc.scalar.activation(ep_im[:, 512 * j : 512 * (j + 1)], pe_h[:, :],
                                 AF.Identity)
        U4, NPG = 704, 176
        ep_rd = eb.tile([H, U4], BF16, tag="ep_rd")
        for j, (c0, c1) in enumerate(((0, 512), (512, U4))):
            pe_h = pG.tile([H, c1 - c0], F32, tag="lgh", name=f"pep{j}",
                           padded_shape=[H, 512])
            MM(pe_h[:, :], ct["wrde"][:, :], ct["embrd_u"][:, c0:c1])
            nc.scalar.activation(ep_rd[:, c0:c1], pe_h[:, :], AF.Identity)
        psum_ep3 = pG.tile([H, NOP], F32, tag="lgh", padded_shape=[H, 512])
        MM(psum_ep3[:, :], ct["wrse"][:, :], ct["embrs"][:, :])
        ep_rs = eb.tile([H, NOP], BF16, tag="ep_rs")
        nc.scalar.activation(ep_rs[:, :], psum_ep3[:, :], AF.Identity)

        # ---- fp (feature partials): one [H, 4*BL] psum tile, one
        # accumulation group per head, emitted head-by-head as w1 DMAs land
        psum_fp = pB.tile([H, 4 * BL], F32, tag="seqB", padded_shape=[H, 512])
        fp = {}

        def emit_fp(hd, nm):
            for k in range(4):
                MM(
                    psum_fp[:, 32 * hd : 32 * hd + BL],
                    ct["w1" + nm][:, 128 * k : 128 * (k + 1)],
                    ct["featT"][:, BL * k : BL * (k + 1)],
                    start=(k == 0),
                    stop=(k == 3),
                )
            if nm == "op":
                t = sb.tile([H, BL], BF16, tag="op_h", name="op_h")
                nc.scalar.activation(t[:, :], psum_fp[:, 96:128], AF.Relu,
                                     bias=ct["b1s"][:, 3:4])
            else:
                t = sb.tile([H, BL], F32, tag=f"fp_{nm}", name=f"fp_{nm}")
                nc.scalar.activation(t[:, :], psum_fp[:, 32 * hd : 32 * hd + BL],
                                     AF.Identity, bias=ct["b1s"][:, hd : hd + 1])
            fp[nm] = t

        emit_fp(0, "im")

        # ---- deferred op/rs head pieces (emitted inside the imm loop)
        st = {}

        def emit_h_rs(b0, b1):
            if "h_rs" not in st:
                st["h_rs"] = cp.tile([H, NOP * BL], BF16, tag="h_rs", name="h_rs")
            for b in range(b0, b1):
                nc.gpsimd.tensor_scalar(
                    st["h_rs"][:, NOP * b : NOP * (b + 1)],
                    ep_rs[:, :], fp["rs"][:, b : b + 1], 0.0,
                    op0=ALU.add, op1=ALU.max,
                )

        def emit_op_logits():
            psum_opl = pG.tile([NO, BL], F32, tag="lgh", padded_shape=[NO, 512],
                               name="psum_opl")
            MM(psum_opl[:, :], ct["w2opT"][:, :], fp["op"][:, :])
            st["exp_op"] = sb.tile([NO, BL], BF16, tag="exp_op", name="exp_op")
            nc.scalar.activation(st["exp_op"][:, :], psum_opl[:, :], AF.Exp,
                                 bias=ct["b2op"][:, :])
            st["lb2_op"] = sb.tile([NO, BL], BF16, tag="lb2_op", name="lb2_op")
            nc.scalar.activation(st["lb2_op"][:, :], psum_opl[:, :], AF.Identity,
                                 bias=ct["b2op"][:, :])

        def emit_rs_logits():
            h_rs_v = st["h_rs"][:, :].rearrange("p (b c) -> p c b", c=NOP)
            psum_rsl = pB.tile([H, 32 * NGRP], F32, tag="seqB", name="psum_rsl")
            for c_ in range(NOP):
                g, s = c_ // 4, c_ % 4
                MM(
                    psum_rsl[32 * s : 32 * s + 32, 32 * g : 32 * g + 32],
                    ct["w2rsT"][:, :],
                    h_rs_v[:, c_, :],
                    tile_position=(0, 32 * s),
                )
            st["exp_rs"] = sb.tile([H, 32 * NGRP], BF16, tag="exp_rs", name="exp_rs")
            nc.scalar.activation(st["exp_rs"][:, :], psum_rsl[:, :], AF.Exp,
                                 bias=ct["b2rs"][:, :])
            st["lb2_rs"] = sb.tile([H, 32 * NGRP], BF16, tag="lb2_rs", name="lb2_rs")
            nc.scalar.activation(st["lb2_rs"][:, :], psum_rsl[:, :], AF.Identity,
                                 bias=ct["b2rs"][:, :])

        def emit_su_sections():
            psum_osu = pG.tile([1, BL], F32, tag="lgh", padded_shape=[1, 512],
                               name="psum_osu")
            MM(psum_osu[:, :], ct["onescol"][0:NO, :], st["exp_op"][:, :])
            st["lnsu_op"] = sb.tile([1, BL], BF16, tag="lnsu_op", name="lnsu_op")
            nc.scalar.activation(st["lnsu_op"][:, :], psum_osu[:, :], AF.Ln)
            psum_rsu = pG.tile([4, 512], F32, tag="lgh", name="psum_rsu")
            MM(psum_rsu[:, :], ct["suind_rs"][:, :], st["exp_rs"][:, 0:512])
            psum_rsu2 = pG.tile([4, 32 * NGRP - 512], F32, tag="lgh",
                                padded_shape=[4, 512], name="psum_rsu2")
            MM(psum_rsu2[:, :], ct["suind_rs"][:, :], st["exp_rs"][:, 512 : 32 * NGRP])
            st["lnsu_rs"] = sb.tile([4, 32 * NGRP], BF16, tag="lnsu_rs",
                                    name="lnsu_rs")
            nc.scalar.activation(st["lnsu_rs"][:, 0:512], psum_rsu[:, :], AF.Ln)
            nc.scalar.activation(st["lnsu_rs"][:, 512 : 32 * NGRP],
                                 psum_rsu2[:, :], AF.Ln)

        def emit_acc():
            psum_acc = pA.tile([BL, A], F32, tag="seqA", name="psum_acc")
            for j in range(2):
                MM(psum_acc[:, 512 * j : 512 * (j + 1)], st["lb2_op"][:, :],
                   ct["gop"][:, 512 * j : 512 * (j + 1)], start=True, stop=False)
                MM(psum_acc[:, 512 * j : 512 * (j + 1)], st["lnsu_op"][:, :],
                   ct["negones"][:, 512 * j : 512 * (j + 1)], start=False,
                   stop=False)
                MM(psum_acc[:, 512 * j : 512 * (j + 1)], ct["onesrow"][:, :],
                   ct["mdb2"][:, 512 * j : 512 * (j + 1)], start=False, stop=False)
            last_for_bank = {}
            for i, (g, lo, hi) in enumerate(chunks):
                last_for_bank[lo // 512] = i
            for i, (g, lo, hi) in enumerate(chunks):
                MM(psum_acc[:, lo:hi], st["lb2_rs"][:, 32 * g : 32 * g + 32],
                   ct["grs"][:, lo:hi], start=False, stop=False)
                MM(psum_acc[:, lo:hi], st["lnsu_rs"][:, 32 * g : 32 * g + 32],
                   ct["g2rs"][:, lo:hi], start=False, stop=False)
            st["psum_acc"] = psum_acc

        # ---- rd-head h table (deduplicated over (o, rs) pairs), b-major
        h_rd_all = cp.tile([H, U4 * BL], BF16, tag="h_rd_all", name="h_rd_all")

        def emit_h_rd(b0, b1, eng):
            for b in range(b0, b1):
                eng.tensor_scalar(
                    h_rd_all[:, U4 * b : U4 * (b + 1)],
                    ep_rd[:, :], fp["rd"][:, b : b + 1], 0.0,
                    op0=ALU.add, op1=ALU.max,
                )

        # ---- imm head phase: h_im tiles + strip matmuls (every psum row of
        # strip s holds d for b=4g+s) -> bf16 copies into d_all -> one
        # SBUF->SBUF gather DMA per strip into compact X32[32, A].
        # op/rs/acc sections are emitted between waves so each engine's
        # in-order queue reaches them right as their inputs land.
        X32 = cp.tile([BL, A], BF16, tag="X32", name="X32")
        d_all = cp.tile([H, 16 * 512], BF16, tag="d_all", name="d_all")
        CP_ENG = ["A", "A", "A", "D"]  # psum reads: ACT/DVE only
        for g in range(8):
            hts_im = []
            for s in range(4):
                b = 4 * g + s
                h_t = hb.tile([H, A], BF16, tag="h", name=f"him{b}")
                nc.vector.tensor_scalar(
                    h_t[:, :], ep_im[:, :], fp["im"][:, b : b + 1], 0.0,
                    op0=ALU.add, op1=ALU.max,
                )
                hts_im.append(h_t)
            psum_d = pG.tile([H, 1024], F32, tag="lgh", name=f"d{g}")
            for j in range(2):
                for s in range(4):
                    MM(
                        psum_d[32 * s : 32 * s + 32, 512 * j : 512 * (j + 1)],
                        ct["wd32"][:, :],
                        hts_im[s][:, 512 * j : 512 * (j + 1)],
                        tile_position=(0, 32 * s),
                    )
            dsl = d_all[:, 1024 * g : 1024 * (g + 1)]
            lane = CP_ENG[g % len(CP_ENG)]
            if lane == "A":
                nc.scalar.activation(dsl, psum_d[:, :], AF.Identity)
            else:
                nc.vector.tensor_copy(dsl, psum_d[:, :])
            if g == 0:
                emit_fp(1, "rs")
            elif g == 1:
                emit_fp(2, "rd")
                emit_h_rs(0, 16)
            elif g == 2:
                emit_fp(3, "op")
                emit_h_rs(16, BL)
                emit_h_rd(0, 2, nc.vector)
            elif g == 3:
                emit_op_logits()
                emit_h_rd(2, 4, nc.vector)
            elif g == 4:
                emit_rs_logits()
                emit_h_rd(4, 6, nc.vector)
            elif g == 5:
                emit_su_sections()
                emit_h_rd(6, 8, nc.vector)
                emit_h_rd(24, 27, nc.gpsimd)
            elif g == 6:
                emit_h_rd(8, 11, nc.vector)
                emit_h_rd(27, 30, nc.gpsimd)
            elif g == 7:
                emit_h_rd(11, 14, nc.vector)
                emit_h_rd(30, BL, nc.gpsimd)
        # gather: X32[4g+s, 512j+c] = d_all[32s, (2g+j)*512 + c]
        for s in range(4):
            dma_engs[s % 2].dma_start(
                X32[s : BL : 4, :].rearrange("g (j c) -> g j c", j=2),
                d_all[32 * s : 32 * s + 1, :].rearrange(
                    "p (g j c) -> p g j c", g=8, j=2
                ),
            )

        # finish h_rd, then acc gathers overlap the DVE tail
        emit_h_rd(14, 22, nc.vector)
        for b in (22, 23):
            nc.scalar.activation(
                h_rd_all[:, U4 * b : U4 * (b + 1)], ep_rd[:, :], AF.Relu,
                bias=fp["rd"][:, b : b + 1],
            )
        emit_acc()

        # ---- rd head: deduplicated logits table [(strip, v), (pgrp, b)]
        # in 11 psum waves; exp on ACT; su-reduce (4 cols per pair-group)
        sp32 = cp.tile([BL, A], BF16, tag="sp32")
        u32 = cp.tile([BL, A], BF16, tag="u32")
        ctr_im = cp.tile([BL, A], BF16, tag="ctr_im")
        h_rd_v = h_rd_all[:, :].rearrange("p (b c) -> p c b", c=U4)
        exp_tbl = cp.tile([H, 32 * NPG], BF16, tag="exp_tbl", name="exp_tbl")
        psum_sutbl = pB.tile([BL, U4], F32, tag="seqB", name="psum_sutbl")

        NT = 6  # table tiles of [128, 1024] = 32 pair-groups each (last: 16)

        def tcols(t):
            return min(32 * NPG - 1024 * t, 1024)

        def emit_su(t):
            # su-reduce for tile t, emitted one tile late so exp(t) is done
            # and the PE wait-queue never blocks mid-loop
            for pgl in range(tcols(t) // 32):
                pg = 32 * t + pgl
                MM(
                    psum_sutbl[:, 4 * pg : 4 * pg + 4],
                    exp_tbl[:, 1024 * t + 32 * pgl : 1024 * t + 32 * pgl + 32],
                    ct["suind_rs"][:, :],
                )

        for t in range(NT):
            ptbl = pG.tile([H, 1024], F32, tag="lgh", name=f"tbl{t}")
            for pl in range(tcols(t) // 8):
                p_ = 128 * t + pl
                s = p_ % 4
                pgl = (p_ // 4) % 32
                MM(
                    ptbl[32 * s : 32 * s + 32, 32 * pgl : 32 * pgl + 32],
                    ct["w2rdT"][:, :],
                    h_rd_v[:, p_, :],
                    tile_position=(0, 32 * s),
                )
            nc.scalar.activation(exp_tbl[:, 1024 * t : 1024 * t + tcols(t)],
                                 ptbl[:, 0 : tcols(t)],
                                 AF.Exp, bias=ct["b2rd"][:, :])
            if t >= 1:
                emit_su(t - 1)
            if t == 1:
                # imm-head tail: sp = softplus(X + db2) = ln(1 + e^(X+db2)),
                # u = X*m, ctr_im = u - sp (db2*m is already in the acc)
                e32 = cp.tile([BL, A], BF16, tag="e32", name="e32")
                nc.scalar.activation(e32[:, :], X32[:, :], AF.Exp,
                                     bias=ct["db2im"][:, :])
                e1 = cp.tile([BL, A], BF16, tag="e1", name="e1")
                nc.vector.tensor_scalar_add(e1[:, :], e32[:, :], 1.0)
                nc.scalar.activation(sp32[:, :], e1[:, :], AF.Ln)
                nc.vector.tensor_mul(u32[:, :], X32[:, :], ct["m32"][:, :])
                nc.vector.tensor_sub(ctr_im[:, :], u32[:, :], sp32[:, :])

        emit_su(NT - 1)

        # ---- lnsu table [32, U4] -> transpose to [pair, b] chunks
        lnsu_tbl = cp.tile([BL, 768], BF16, tag="lnsu_tbl", name="lnsu_tbl")
        nc.scalar.activation(lnsu_tbl[:, 0:U4], psum_sutbl[:, :], AF.Ln)
        nc.vector.memset(lnsu_tbl[:, U4:768], 0.0)
        lnsuT = cp.tile([H, 6 * 32], BF16, tag="lnsuT", name="lnsuT")
        for ch in range(6):
            pt = pG.tile([H, 32], BF16, tag="lgh", name=f"ptr{ch}",
                         padded_shape=[H, 512])
            nc.tensor.transpose(pt[:, :], lnsu_tbl[:, 128 * ch : 128 * (ch + 1)],
                                ct["id32"][:, :])
            nc.vector.tensor_copy(lnsuT[:, 32 * ch : 32 * (ch + 1)], pt[:, :])

        # ---- gathers: exp_sel into psum_q (by pair-group), -lnsu into the
        # open accumulator (by 128-pair chunk); bank-wise start/stop
        psum_q = pB.tile([BL, A], F32, tag="seqB", name="psum_q")
        qfirst, qlast = {}, {}
        for i, (pg, lo, hi) in enumerate(qchunks):
            b = lo // 512
            qfirst.setdefault(b, i)
            qlast[b] = i
        for i, (pg, lo, hi) in enumerate(qchunks):
            b = lo // 512
            MM(
                psum_q[:, lo:hi],
                exp_tbl[:, 32 * pg : 32 * pg + 32],
                ct["grd"][:, lo:hi],
                start=(qfirst[b] == i),
                stop=(qlast[b] == i),
            )
        lnsel = cp.tile([BL, A], BF16, tag="lnsel")
        for j in range(2):
            nc.scalar.activation(lnsel[:, 512 * j : 512 * (j + 1)],
                                 psum_q[:, 512 * j : 512 * (j + 1)], AF.Ln)
        w32 = cp.tile([BL, A], BF16, tag="w32")
        nc.vector.tensor_add(w32[:, :], ctr_im[:, :], lnsel[:, :])

        llast = {}
        for i, (ch, lo, hi) in enumerate(lchunks):
            llast[lo // 512] = i
        psum_acc = st["psum_acc"]
        for i, (ch, lo, hi) in enumerate(lchunks):
            MM(
                psum_acc[:, lo:hi],
                lnsuT[:, 32 * ch : 32 * ch + 32],
                ct["gpair"][:, lo:hi],
                start=False,
                stop=(llast[lo // 512] == i),
            )

        # ---- final combine + store, per psum bank
        for j in range(2):
            sl = slice(512 * j, 512 * (j + 1))
            out_sb = sb.tile([BL, 512], F32, tag="out_sb", name=f"out{j}")
            nc.vector.tensor_add(out_sb[:, :], w32[:, sl], psum_acc[:, sl])
            nc.sync.dma_start(out_d[:, sl], out_sb[:, :])

    return nc.tensor.matmul(*a, **k)

    with ExitStack() as ctx:
        tc = ctx.enter_context(tile.TileContext(nc))
        cp = ctx.enter_context(tc.tile_pool(name="consts", bufs=1))
        sb = ctx.enter_context(tc.tile_pool(name="sbuf", bufs=2))
        hb = ctx.enter_context(tc.tile_pool(name="hbuf", bufs=8))
        eb = ctx.enter_context(tc.tile_pool(name="ebuf", bufs=3))
        pA = ctx.enter_context(tc.tile_pool(name="pA", bufs=1, space="PSUM"))
        pB = ctx.enter_context(tc.tile_pool(name="pB", bufs=1, space="PSUM"))
        pG = ctx.enter_context(tc.tile_pool(name="pG", bufs=2, space="PSUM"))

        # ---- load packed constants: one tile + one DMA per chunk, in
        # first-consumer order; ct[] = slices into the chunk tiles
        dma_engs = [nc.sync, nc.gpsimd]
        ct = {}
        for chunk, parts, dt, entries in _PACKS:
            ncols = sum(e[2] for e in entries)
            t = cp.tile([parts, ncols], dt, tag=chunk, name=chunk)
            dma_engs[_PACK_ENG[chunk]].dma_start(t[:, :], dr[chunk][:, :])
            off = 0
            for name, rows, cols in entries:
                ct[name] = t[0:rows, off : off + cols]
                off += cols

        # ---- ep tables (embedding partials) on PE; psum->sbuf copies on ACT
        ep_im = eb.tile([H, A], BF16, tag="ep_im")
        for j in range(2):
            pe_h = pG.tile([H, 512], F32, tag="lgh", name=f"pei{j}")
            MM(pe_h[:, :], ct["wime1"][:, :], ct["embrd"][:, 512 * j : 512 * (j + 1)],
               start=True, stop=False)
            MM(pe_h[:, :], ct["wime2"][:, :], ct["embim2"][:, 512 * j : 512 * (j + 1)],
               start=False, stop=True)
            nc.scalar.activation(ep_im[:, 512 * j : 512 * (j + 1)], pe_h[:, :],
                                 AF.Identity)
        U4, NPG = 704, 176
        ep_rd = eb.tile([H, U4], BF16, tag="ep_rd")
        for j, (c0, c1) in enumerate(((0, 512), (512, U4))):
            pe_h = pG.tile([H, c1 - c0], F32, tag="lgh", name=f"pep{j}",
                           padded_shape=[H, 512])
            MM(pe_h[:, :], ct["wrde"][:, :], ct["embrd_u"][:, c0:c1])
            nc.scalar.activation(ep_rd[:, c0:c1], pe_h[:, :], AF.Identity)
        psum_ep3 = pG.tile([H, NOP], F32, tag="lgh", padded_shape=[H, 512])
        MM(psum_ep3[:, :], ct["wrse"][:, :], ct["embrs"][:, :])
        ep_rs = eb.tile([H, NOP], BF16, tag="ep_rs")
        nc.scalar.activation(ep_rs[:, :], psum_ep3[:, :], AF.Identity)

        # ---- fp (feature partials): one [H, 4*BL] psum tile, one
        # accumulation group per head, emitted head-by-head as w1 DMAs land
        psum_fp = pB.tile([H, 4 * BL], F32, tag="seqB", padded_shape=[H, 512])
        fp = {}

        def emit_fp(hd, nm):
            for k in range(4):
                MM(
                    psum_fp[:, 32 * hd : 32 * hd + BL],
                    ct["w1" + nm][:, 128 * k : 128 * (k + 1)],
                    ct["featT"][:, BL * k : BL * (k + 1)],
                    start=(k == 0),
                    stop=(k == 3),
                )
            if nm == "op":
                t = sb.tile([H, BL], BF16, tag="op_h", name="op_h")
                nc.scalar.activation(t[:, :], psum_fp[:, 96:128], AF.Relu,
                                     bias=ct["b1s"][:, 3:4])
            else:
                t = sb.tile([H, BL], F32, tag=f"fp_{nm}", name=f"fp_{nm}")
                nc.scalar.activation(t[:, :], psum_fp[:, 32 * hd : 32 * hd + BL],
                                     AF.Identity, bias=ct["b1s"][:, hd : hd + 1])
            fp[nm] = t

        emit_fp(0, "im")

        # ---- deferred op/rs head pieces (emitted inside the imm loop)
        st = {}

        def emit_h_rs(b0, b1):
            if "h_rs" not in st:
                st["h_rs"] = cp.tile([H, NOP * BL], BF16, tag="h_rs", name="h_rs")
            for b in range(b0, b1):
                nc.gpsimd.tensor_scalar(
                    st["h_rs"][:, NOP * b : NOP * (b + 1)],
                    ep_rs[:, :], fp["rs"][:, b : b + 1], 0.0,
                    op0=ALU.add, op1=ALU.max,
                )

        def emit_op_logits():
            psum_opl = pG.tile([NO, BL], F32, tag="lgh", padded_shape=[NO, 512],
                               name="psum_opl")
            MM(psum_opl[:, :], ct["w2opT"][:, :], fp["op"][:, :])
            st["exp_op"] = sb.tile([NO, BL], BF16, tag="exp_op", name="exp_op")
            nc.scalar.activation(st["exp_op"][:, :], psum_opl[:, :], AF.Exp,
                                 bias=ct["b2op"][:, :])
            st["lb2_op"] = sb.tile([NO, BL], BF16, tag="lb2_op", name="lb2_op")
            nc.scalar.activation(st["lb2_op"][:, :], psum_opl[:, :], AF.Identity,
                                 bias=ct["b2op"][:, :])

        def emit_rs_logits():
            h_rs_v = st["h_rs"][:, :].rearrange("p (b c) -> p c b", c=NOP)
            psum_rsl = pB.tile([H, 32 * NGRP], F32, tag="seqB", name="psum_rsl")
            for c_ in range(NOP):
                g, s = c_ // 4, c_ % 4
                MM(
                    psum_rsl[32 * s : 32 * s + 32, 32 * g : 32 * g + 32],
                    ct["w2rsT"][:, :],
                    h_rs_v[:, c_, :],
                    tile_position=(0, 32 * s),
                )
            st["exp_rs"] = sb.tile([H, 32 * NGRP], BF16, tag="exp_rs", name="exp_rs")
            nc.scalar.activation(st["exp_rs"][:, :], psum_rsl[:, :], AF.Exp,
                                 bias=ct["b2rs"][:, :])
            st["lb2_rs"] = sb.tile([H, 32 * NGRP], BF16, tag="lb2_rs", name="lb2_rs")
            nc.scalar.activation(st["lb2_rs"][:, :], psum_rsl[:, :], AF.Identity,
                                 bias=ct["b2rs"][:, :])

        def emit_su_sections():
            psum_osu = pG.tile([1, BL], F32, tag="lgh", padded_shape=[1, 512],
                               name="psum_osu")
            MM(psum_osu[:, :], ct["onescol"][0:NO, :], st["exp_op"][:, :])
            st["lnsu_op"] = sb.tile([1, BL], BF16, tag="lnsu_op", name="lnsu_op")
            nc.scalar.activation(st["lnsu_op"][:, :], psum_osu[:, :], AF.Ln)
            psum_rsu = pG.tile([4, 512], F32, tag="lgh", name="psum_rsu")
            MM(psum_rsu[:, :], ct["suind_rs"][:, :], st["exp_rs"][:, 0:512])
            psum_rsu2 = pG.tile([4, 32 * NGRP - 512], F32, tag="lgh",
                                padded_shape=[4, 512], name="psum_rsu2")
            MM(psum_rsu2[:, :], ct["suind_rs"][:, :], st["exp_rs"][:, 512 : 32 * NGRP])
            st["lnsu_rs"] = sb.tile([4, 32 * NGRP], BF16, tag="lnsu_rs",
                                    name="lnsu_rs")
            nc.scalar.activation(st["lnsu_rs"][:, 0:512], psum_rsu[:, :], AF.Ln)
            nc.scalar.activation(st["lnsu_rs"][:, 512 : 32 * NGRP],
                                 psum_rsu2[:, :], AF.Ln)

        def emit_acc():
            psum_acc = pA.tile([BL, A], F32, tag="seqA", name="psum_acc")
            for j in range(2):
                MM(psum_acc[:, 512 * j : 512 * (j + 1)], st["lb2_op"][:, :],
                   ct["gop"][:, 512 * j : 512 * (j + 1)], start=True, stop=False)
                MM(psum_acc[:, 512 * j : 512 * (j + 1)], st["lnsu_op"][:, :],
                   ct["negones"][:, 512 * j : 512 * (j + 1)], start=False,
                   stop=False)
                MM(psum_acc[:, 512 * j : 512 * (j + 1)], ct["onesrow"][:, :],
                   ct["mdb2"][:, 512 * j : 512 * (j + 1)], start=False, stop=False)
            last_for_bank = {}
            for i, (g, lo, hi) in enumerate(chunks):
                last_for_bank[lo // 512] = i
            for i, (g, lo, hi) in enumerate(chunks):
                MM(psum_acc[:, lo:hi], st["lb2_rs"][:, 32 * g : 32 * g + 32],
                   ct["grs"][:, lo:hi], start=False, stop=False)
                MM(psum_acc[:, lo:hi], st["lnsu_rs"][:, 32 * g : 32 * g + 32],
                   ct["g2rs"][:, lo:hi], start=False, stop=False)
            st["psum_acc"] = psum_acc

        # ---- rd-head h table (deduplicated over (o, rs) pairs), b-major
        h_rd_all = cp.tile([H, U4 * BL], BF16, tag="h_rd_all", name="h_rd_all")

        def emit_h_rd(b0, b1, eng):
            for b in range(b0, b1):
                eng.tensor_scalar(
                    h_rd_all[:, U4 * b : U4 * (b + 1)],
                    ep_rd[:, :], fp["rd"][:, b : b + 1], 0.0,
                    op0=ALU.add, op1=ALU.max,
                )

        # ---- imm head phase: h_im tiles + strip matmuls (every psum row of
        # strip s holds d for b=4g+s) -> bf16 copies into d_all -> one
        # SBUF->SBUF gather DMA per strip into compact X32[32, A].
        # op/rs/acc sections are emitted between waves so each engine's
        # in-order queue reaches them right as their inputs land.
        X32 = cp.tile([BL, A], BF16, tag="X32", name="X32")
        d_all = cp.tile([H, 16 * 512], BF16, tag="d_all", name="d_all")
        CP_ENG = ["A", "A", "A", "D"]  # psum reads: ACT/DVE only
        for g in range(8):
            hts_im = []
            for s in range(4):
                b = 4 * g + s
                h_t = hb.tile([H, A], BF16, tag="h", name=f"him{b}")
                nc.vector.tensor_scalar(
                    h_t[:, :], ep_im[:, :], fp["im"][:, b : b + 1], 0.0,
                    op0=ALU.add, op1=ALU.max,
                )
                hts_im.append(h_t)
            psum_d = pG.tile([H, 1024], F32, tag="lgh", name=f"d{g}")
            for j in range(2):
                for s in range(4):
                    MM(
                        psum_d[32 * s : 32 * s + 32, 512 * j : 512 * (j + 1)],
                        ct["wd32"][:, :],
                        hts_im[s][:, 512 * j : 512 * (j + 1)],
                        tile_position=(0, 32 * s),
                    )
            dsl = d_all[:, 1024 * g : 1024 * (g + 1)]
            lane = CP_ENG[g % len(CP_ENG)]
            if lane == "A":
                nc.scalar.activation(dsl, psum_d[:, :], AF.Identity)
            else:
                nc.vector.tensor_copy(dsl, psum_d[:, :])
            if g == 0:
                emit_fp(1, "rs")
            elif g == 1:
                emit_fp(2, "rd")
                emit_h_rs(0, 16)
            elif g == 2:
                emit_fp(3, "op")
                emit_h_rs(16, BL)
                emit_h_rd(0, 2, nc.vector)
            elif g == 3:
                emit_op_logits()
                emit_h_rd(2, 4, nc.vector)
            elif g == 4:
                emit_rs_logits()
                emit_h_rd(4, 6, nc.vector)
            elif g == 5:
                emit_su_sections()
                emit_h_rd(6, 8, nc.vector)
                emit_h_rd(24, 27, nc.gpsimd)
            elif g == 6:
                emit_h_rd(8, 11, nc.vector)
                emit_h_rd(27, 30, nc.gpsimd)
            elif g == 7:
                emit_h_rd(11, 14, nc.vector)
                emit_h_rd(30, BL, nc.gpsimd)
        # gather: X32[4g+s, 512j+c] = d_all[32s, (2g+j)*512 + c]
        for s in range(4):
            dma_engs[s % 2].dma_start(
                X32[s : BL : 4, :].rearrange("g (j c) -> g j c", j=2),
                d_all[32 * s : 32 * s + 1, :].rearrange(
                    "p (g j c) -> p g j c", g=8, j=2
                ),
            )

        # ---- rd head phase (imm tail ops are emitted after wave g==3 so
        # the engines reach them right around when X32's gather DMA lands)
        sp32 = cp.tile([BL, A], BF16, tag="sp32")
        u32 = cp.tile([BL, A], BF16, tag="u32")
        ctr_im = cp.tile([BL, A], BF16, tag="ctr_im")
        psum_su = pA.tile([BL, A], F32, tag="seqA")
        psum_sel = pB.tile([BL, A], F32, tag="seqB")
        for g in range(8):
            hts_rd = []
            for s in range(4):
                b = 4 * g + s
                h_t = hb.tile([H, A], BF16, tag="h", name=f"hrd{b}")
                nc.vector.tensor_scalar(
                    h_t[:, :], ep_rd[:, :], fp["rd"][:, b : b + 1], 0.0,
                    op0=ALU.add, op1=ALU.max,
                )
                hts_rd.append(h_t)
            exp_t = sb.tile([H, A], BF16, tag="exp_t")
            mexp_t = sb.tile([H, A], BF16, tag="mexp_t")
            for j in range(2):
                psum_lg = pG.tile([H, 512], F32, tag="lgh", name=f"lg{g}{j}")
                for s in range(4):
                    MM(
                        psum_lg[32 * s : 32 * s + 32, :],
                        ct["w2rdT"][:, :],
                        hts_rd[s][:, 512 * j : 512 * (j + 1)],
                        tile_position=(0, 32 * s),
                    )
                nc.scalar.activation(
                    exp_t[:, 512 * j : 512 * (j + 1)], psum_lg[:, :], AF.Exp,
                    bias=ct["b2rd"][:, :],
                )
                nc.vector.tensor_mul(
                    mexp_t[:, 512 * j : 512 * (j + 1)],
                    exp_t[:, 512 * j : 512 * (j + 1)],
                    ct["mask_rd"][:, 512 * j : 512 * (j + 1)],
                )
            for j in range(2):
                MM(
                    psum_su[:, 512 * j : 512 * (j + 1)],
                    ct["suind_rd"][:, 32 * g : 32 * g + 32],
                    exp_t[:, 512 * j : 512 * (j + 1)],
                    start=(g == 0),
                    stop=(g == 7),
                )
                MM(
                    psum_sel[:, 512 * j : 512 * (j + 1)],
                    ct["suind_rd"][:, 32 * g : 32 * g + 32],
                    mexp_t[:, 512 * j : 512 * (j + 1)],
                    start=(g == 0),
                    stop=(g == 7),
                )
            if g == 3:
                # imm-head tail: sp = softplus(X + db2) = ln(1 + e^(X+db2))
                # (built from Exp/Ln, which are already ACT-table residents),
                # u = X*m, ctr_im = u - sp (db2*m is already in the acc)
                e32 = cp.tile([BL, A], BF16, tag="e32", name="e32")
                nc.scalar.activation(e32[:, :], X32[:, :], AF.Exp,
                                     bias=ct["db2im"][:, :])
                e1 = cp.tile([BL, A], BF16, tag="e1", name="e1")
                nc.vector.tensor_scalar_add(e1[:, :], e32[:, :], 1.0)
                nc.scalar.activation(sp32[:, :], e1[:, :], AF.Ln)
                nc.vector.tensor_mul(u32[:, :], X32[:, :], ct["m32"][:, :])
                nc.vector.tensor_sub(ctr_im[:, :], u32[:, :], sp32[:, :])
            if g == 4:
                t3 = sb.tile([BL, A], BF16, tag="t3")
                nc.vector.tensor_add(t3[:, :], ctr_im[:, :], st["acc_sb"][:, :])

        # ---- rd-head tail + final combine + store, pipelined by psum bank
        for j in range(2):
            sl = slice(512 * j, 512 * (j + 1))
            lnsu_t = sb.tile([BL, 512], BF16, tag="lnsu_rdt", name=f"lnsu{j}")
            nc.scalar.activation(lnsu_t[:, :], psum_su[:, sl], AF.Ln)
            lnsel_t = sb.tile([BL, 512], BF16, tag="lnsel_rdt", name=f"lnsel{j}")
            nc.scalar.activation(lnsel_t[:, :], psum_sel[:, sl], AF.Ln)
            ctr_rd = sb.tile([BL, 512], BF16, tag="ctr_rd", name=f"ctrrd{j}")
            nc.vector.tensor_sub(ctr_rd[:, :], lnsel_t[:, :], lnsu_t[:, :])
            out_sb = sb.tile([BL, 512], F32, tag="out_sb", name=f"out{j}")
            nc.vector.tensor_add(out_sb[:, :], ctr_rd[:, :], t3[:, sl])
            nc.sync.dma_start(out_d[:, sl], out_sb[:, :])

    return nc


_CACHE = {}


def _get_program(chunks):
    key = chunks
    if key not in _CACHE:
        _CACHE[key] = build_program(chunks)
    return _CACHE[key]


def kernel(**inputs) -> np.ndarray:
    packed, per_core, chunks, perm = _host_prep(inputs)
    nc = _get_program(chunks)
    in_maps = []
    for cid in range(NCORES):
        m = {k: np.ascontiguousarray(v) for k, v in packed.items()}
        m["c_feat"] = np.ascontiguousarray(_bf(per_core[cid]["featT"]))
        in_maps.append(m)
    res = run_bass_kernel_spmd(nc, in_maps, core_ids=list(range(NCORES)))
    out_sorted = np.concatenate(
        [res.results[cid]["out"] for cid in range(NCORES)], axis=0
    )  # [B, A] in sorted-action order
    out = np.empty_like(out_sorted)
    out[:, perm] = out_sorted
    return out.astype(np.float32)


# revision 36
# speedup vs baseline: 1.0303x; 1.0303x over previous
"""Trainium2 Bass kernel for nn_AutoregressiveInstructionHead.

Data-parallel over batch B=256 across 8 NeuronCores (32 rows each).
Head weights / embeddings / action-derived tables are replicated.

Per-core device pipeline (all heavy compute on device):
  - constants packed into a few DRAM tensors -> few big DMAs ordered by
    first consumer (HWDGE per-DMA overhead is ~625ns, so fewer transfers
    and compute starts ~3.5us in instead of ~18us)
  - fp_head = features @ W1_feat.T (+b1)  -> [H=128, B=32] via PE
  - ep tables = embeddings @ W1_emb.T     -> [H=128, A] via PE
  - imm head (NI=2): logp = m*X - softplus(X) with X = l0 - l1, so only a
    single M=1 matmul strip per (b, 512-col block) with w_d = W2[0]-W2[1];
    the strip-scattered psum rows are gathered to a compact X32[32,A] tile
    by a psum->SBUF DMA (free on the DMA engines), then one softplus + two
    DVE ops finish the head.  No exp / mask / su-sel matmuls needed.
  - op head: logits -> exp/sum/ln -> gather via one-hot matmul into PSUM acc
  - rs head: deduplicated over the 65 opcodes ([B,65,17] table), gathered
    back to the 1024 actions with block-sparse one-hot matmuls (actions are
    host-sorted by (opcode, reg_src) so each table chunk touches a
    contiguous column range; inverse permutation applied on host at the end)
  - rd head: h=relu(fp[b]+ep[a]) [128,1024] per b, logits matmul with
    col-tiling (4 b's concurrently in 32-partition strips), exp(+b2) on ACT,
    one-hot mask multiply, partition sums via indicator matmuls,
    contribution = ln(sum mask*exp) - ln(sum exp)
"""

import sys

for _p in ("/opt/trn_rl_repo",):
    if _p not in sys.path:
        sys.path.insert(0, _p)

import numpy as np
from contextlib import ExitStack

import json

import concourse.bass as bass
import concourse.tile as tile
from concourse import mybir
from concourse import bass2jax as _bass2jax
from concourse.bass_utils import run_bass_kernel_spmd
from concourse.bass_utils import compile_bir_kernel as _orig_compile_bir_kernel

# --- workaround: this container's walrus rejects instructions carrying more
# than one sync-wait command ("Too many sync wait commands"), but Tile's
# scheduler emits multi-wait instructions.  Split them in the serialized BIR
# by inserting wait-only EventSemaphore carriers immediately before, on the
# same engine queue (semantically identical: same queue position, waits
# simply execute as separate instructions).
_WSPLIT_UID = [0]


def _split_bir_waits(bir_json: bytes, maxw: int = 1) -> bytes:
    m = json.loads(bir_json)
    tmpl = None
    for fn in m["functions"]:
        for bb in fn["blocks"]:
            for ins in bb["instructions"]:
                if ins.get("opcode") == "EventSemaphore":
                    tmpl = json.loads(json.dumps(ins))
                    break
            if tmpl:
                break
    if tmpl is None:
        return bir_json
    for fn in m["functions"]:
        for bb in fn["blocks"]:
            out = []
            for ins in bb["instructions"]:
                si = ins.get("sync_info")
                waits = (si or {}).get("on_wait") or []
                if len(waits) > maxw:
                    keep = waits[-maxw:]
                    extra = waits[:-maxw]
                    for i in range(0, len(extra), maxw):
                        _WSPLIT_UID[0] += 1
                        d = json.loads(json.dumps(tmpl))
                        d["name"] = f"WSPLIT-{_WSPLIT_UID[0]}"
                        d["engine"] = ins["engine"]
                        d["ins"] = []
                        d["outs"] = []
                        d["sync_info"] = {
                            "on_wait": extra[i : i + maxw],
                            "on_update": [],
                        }
                        d.pop("debug", None)
                        d.pop("bass_addl_debug", None)
                        out.append(d)
                    si["on_wait"] = keep
                out.append(ins)
            bb["instructions"] = out
    return json.dumps(m).encode()


def _patched_compile_bir_kernel(bir_json, tmpdir, neff_name="file.neff"):
    return _orig_compile_bir_kernel(
        _split_bir_waits(bir_json), tmpdir, neff_name=neff_name
    )


_bass2jax.compile_bir_kernel = _patched_compile_bir_kernel

# dims
B, D, A = 256, 512, 1024
NO, NR, NI, E, H = 65, 17, 2, 64, 128
NCORES = 8
BL = B // NCORES  # 32 batch rows per core

F32 = mybir.dt.float32
BF16 = mybir.dt.bfloat16
AF = mybir.ActivationFunctionType
ALU = mybir.AluOpType

NOP = 68  # rs head padded to a multiple of 4 opcodes
NGRP = NOP // 4  # 17 groups of 4 opcodes (rs head)


def _bf(x):
    import ml_dtypes

    return np.asarray(x, dtype=ml_dtypes.bfloat16)


def _f32(x):
    return np.ascontiguousarray(np.asarray(x, dtype=np.float32))


# ---------------------------------------------------------------------------
# packed-constant layout: chunk -> (partitions, dtype, [(name, rows, cols)])
# one DRAM param + one SBUF tile + one DMA per chunk; DMAs are issued in
# this order (first-consumer order)
_PACKS = [
    ("c_a2", 128, BF16, [("embrd", 2 * E, A), ("wrde", 2 * E, H),
                         ("wime1", 2 * E, H), ("wd32", H, 32)]),
    ("c_c64", 64, BF16, [("wrse", E, H), ("embrs", E, NOP), ("wime2", E, H),
                         ("embim2", E, A)]),
    ("c_feat", 128, BF16, [("featT", 128, 128)]),
    ("c_w1im", 128, BF16, [("w1im", 128, 512)]),
    ("c_f32", 128, F32, [("b1s", H, 4), ("b2op", NO, 1), ("b2rs", H, 1),
                         ("b2rd", H, 1), ("db2im", BL, 1)]),
    ("c_a2r", 128, BF16, [("w2opT", H, NO), ("w2rsT", H, 32),
                          ("w2rdT", H, 32), ("suind_rs", H, 4),
                          ("embrd_u", 2 * E, 704), ("onescol", 128, 1),
                          ("onesrow", 1, 32)]),
    ("c_w1rs", 128, BF16, [("w1rs", 128, 512)]),
    ("c_w1rd", 128, BF16, [("w1rd", 128, 512)]),
    ("c_w1op", 128, BF16, [("w1op", 128, 512)]),
    ("c_b1", 128, BF16, [("gop", NO, A), ("grs", H, A)]),
    ("c_low32", BL, BF16, [("m32", BL, A), ("id32", BL, 32)]),
    ("c_low4", 4, BF16, [("negones", 1, A), ("mdb2", 1, A), ("g2rs", 4, A)]),
    ("c_b2", 128, BF16, [("grd", H, A), ("gpair", H, A)]),
]
_PACK_ENG = {n: (i % 2) for i, (n, _, _, _) in enumerate(_PACKS)}


def _host_prep(inputs):
    """Build all per-core / shared device constants on host (index ops only
    plus dtype packing; all real FLOPs happen on device)."""
    feats = _f32(inputs["features"])
    o = np.clip(inputs["act_o"].astype(np.int64), 0, NO - 1)
    rs = np.clip(inputs["act_rs"].astype(np.int64), 0, NR - 1)
    rd = np.clip(inputs["act_rd"].astype(np.int64), 0, NR - 1)
    im = np.clip(inputs["act_imm"].astype(np.int64), 0, NI - 1)

    perm = np.lexsort((rs, o))  # sort by (opcode, reg_src)
    os_, rss, rds, ims = o[perm], rs[perm], rd[perm], im[perm]

    opcode_embed = _f32(inputs["opcode_embed"])  # [65, 64]
    reg_embed = _f32(inputs["reg_embed"])  # [17, 64]
    op_e = opcode_embed[os_]  # [A, 64] sorted
    rs_e = reg_embed[rss]
    rd_e = reg_embed[rds]

    W = {k: _f32(inputs[k]) for k in inputs if k.endswith(("W1", "W2", "b1", "b2"))}

    c = {}
    # feature-path weights per head: [128, 512] with K-chunk k at cols
    # [128k, 128k+128)
    for nm, wk in (("im", "imm_W1"), ("rs", "rs_W1"), ("rd", "rd_W1"),
                   ("op", "op_W1")):
        wT = W[wk][:, :D].T  # [D, H]
        c["w1" + nm] = np.concatenate(
            [wT[128 * k : 128 * (k + 1), :] for k in range(4)], axis=1
        )  # [128, 512]
    c["b1s"] = _f32(
        np.stack([W["imm_b1"], W["rs_b1"], W["rd_b1"], W["op_b1"]], axis=1)
    )  # [128, 4]

    # embedding-path weights + gathered embeddings (stacked on K)
    c["wrse"] = W["rs_W1"][:, D : D + E].T  # [64, 128]
    embrs = np.zeros((E, NOP), np.float32)
    embrs[:, :NO] = opcode_embed.T
    c["embrs"] = embrs  # [64, 68] all opcodes (padded)
    c["wrde"] = np.concatenate(
        [W["rd_W1"][:, D : D + E].T, W["rd_W1"][:, D + E : D + 2 * E].T], axis=0
    )  # [128, 128]
    c["embrd"] = np.concatenate([op_e.T, rs_e.T], axis=0)  # [128, A]
    c["wime1"] = np.concatenate(
        [W["imm_W1"][:, D : D + E].T, W["imm_W1"][:, D + E : D + 2 * E].T], axis=0
    )  # [128, 128]
    c["wime2"] = W["imm_W1"][:, D + 2 * E :].T  # [64, 128]
    c["embim2"] = rd_e.T  # [64, A]

    # head-2 weights (V padded to 32 with zeros so PSUM pad rows are written)
    c["w2opT"] = W["op_W2"].T  # [128, 65]
    w2rs = np.zeros((H, 32), np.float32)
    w2rs[:, :NR] = W["rs_W2"].T
    c["w2rsT"] = w2rs
    w2rd = np.zeros((H, 32), np.float32)
    w2rd[:, :NR] = W["rd_W2"].T
    c["w2rdT"] = w2rd
    # imm head: difference vector w_d = W2[0] - W2[1], replicated to 32
    # PE columns so every psum row of a strip holds d (no garbage rows)
    c["wd32"] = np.tile((W["imm_W2"][0] - W["imm_W2"][1]).reshape(H, 1), (1, 32))
    db2 = float(W["imm_b2"][0] - W["imm_b2"][1])
    c["db2im"] = np.full((BL, 1), db2, np.float32)

    # biases b2 as per-partition columns
    c["b2op"] = _f32(W["op_b2"][:, None])  # [65, 1]
    for nm, b2 in (("b2rs", W["rs_b2"]), ("b2rd", W["rd_b2"])):
        t = np.zeros((H, 1), np.float32)
        for s in range(4):
            t[32 * s : 32 * s + NR, 0] = b2
        c[nm] = t

    # op-head gather one-hot + misc rows
    m = (ims == 0).astype(np.float32)  # [A] imm-head class-0 selector
    gop = np.zeros((NO, A), np.float32)
    gop[os_, np.arange(A)] = 1.0
    c["gop"] = gop
    c["onescol"] = np.ones((128, 1), np.float32)
    c["onesrow"] = np.ones((1, 32), np.float32)
    c["negones"] = -np.ones((1, A), np.float32)
    c["mdb2"] = (db2 * m)[None, :]  # [1, A]
    c["m32"] = np.broadcast_to(m, (BL, A)).copy()

    # rs-head gather tables (block one-hot; actions sorted by opcode)
    grs = np.zeros((H, A), np.float32)
    grs[(os_ % 4) * 32 + rss, np.arange(A)] = 1.0
    c["grs"] = grs
    g2rs = np.zeros((4, A), np.float32)
    g2rs[os_ % 4, np.arange(A)] = -1.0
    c["g2rs"] = g2rs
    suind_rs = np.zeros((H, 4), np.float32)
    for s in range(4):
        suind_rs[32 * s : 32 * s + NR, s] = 1.0
    c["suind_rs"] = suind_rs

    # rd head deduplicated over distinct (opcode, reg_src) pairs
    pairs_all = os_ * NR + rss                      # non-decreasing (sorted)
    u_pairs, pid = np.unique(pairs_all, return_inverse=True)
    U = len(u_pairs)
    U4 = 704  # fixed pad (U ~ 660-680 for random actions; assert below)
    assert U <= U4, f"U={U} exceeds pad {U4}"
    NPG = U4 // 4  # 176 pair-groups of 4
    o_u = u_pairs // NR
    rs_u = u_pairs % NR
    embrd_u = np.zeros((2 * E, U4), np.float32)
    embrd_u[:E, :U] = opcode_embed[o_u].T
    embrd_u[E:, :U] = reg_embed[rs_u].T
    c["embrd_u"] = embrd_u
    # grd: one-hot [(strip, v), a] selecting (pid(a)%4, rd(a))
    grd = np.zeros((H, A), np.float32)
    grd[32 * (pid % 4) + rds, np.arange(A)] = 1.0
    c["grd"] = grd
    # gpair: -1 one-hot [pid%128, a] for the lnsu gather (per 128-pair chunk)
    gpair = np.zeros((H, A), np.float32)
    gpair[pid % H, np.arange(A)] = -1.0
    c["gpair"] = gpair
    c["id32"] = np.eye(BL, 32, dtype=np.float32)

    # rs gather chunk column ranges (static, baked into program; identical
    # on every core since actions are replicated)
    bounds = np.searchsorted(os_, np.arange(0, NO + 4, 4)[: NGRP + 1])
    chunks = []
    for g in range(NGRP):
        lo, hi = int(bounds[g]), int(bounds[g + 1])
        while lo < hi:
            nxt = min(hi, ((lo // 512) + 1) * 512, lo + 512)
            chunks.append((g, lo, nxt))
            lo = nxt
    # rd-head gather chunk ranges: per pair-group (grd/q gather) and per
    # 128-pair chunk (lnsu gather), split at psum bank boundaries
    pgrp_a = pid // 4
    qchunks = []
    for pg in range(int(pgrp_a.max()) + 1):
        lo = int(np.searchsorted(pgrp_a, pg))
        hi = int(np.searchsorted(pgrp_a, pg + 1))
        while lo < hi:
            nxt = min(hi, ((lo // 512) + 1) * 512)
            qchunks.append((pg, lo, nxt))
            lo = nxt
    lchunks = []
    for ch in range((U + H - 1) // H):
        lo = int(np.searchsorted(pid, H * ch))
        hi = int(np.searchsorted(pid, H * (ch + 1)))
        while lo < hi:
            nxt = min(hi, ((lo // 512) + 1) * 512)
            lchunks.append((ch, lo, nxt))
            lo = nxt
    feat_T = feats.T  # [D, B]
    per_core = []
    for cid in range(NCORES):
        ft = feat_T[:, cid * BL : (cid + 1) * BL]  # [512, 32]
        ftp = np.concatenate(
            [ft[128 * k : 128 * (k + 1), :] for k in range(4)], axis=1
        )  # [128, 128]
        per_core.append({"featT": ftp})

    # assemble packed chunk arrays
    packed = {}
    for chunk, parts, dt, entries in _PACKS:
        ncols = sum(e[2] for e in entries)
        arr = np.zeros((parts, ncols), np.float32)
        off = 0
        for name, rows, cols in entries:
            if name != "featT":
                arr[:rows, off : off + cols] = c[name]
            off += cols
        packed[chunk] = arr if dt == F32 else _bf(arr)
    return packed, per_core, (tuple(chunks), tuple(qchunks), tuple(lchunks)), perm


def build_program(allchunks):
    chunks, qchunks, lchunks = allchunks
    nc = bass.Bass()
    dr = {}
    for chunk, parts, dt, entries in _PACKS:
        ncols = sum(e[2] for e in entries)
        dr[chunk] = nc.declare_dram_parameter(chunk, [parts, ncols], dt,
                                              isOutput=False)
    out_d = nc.declare_dram_parameter("out", [BL, A], F32, isOutput=True)

    def MM(*a, **k):
        k.setdefault("skip_group_check", True)
        return nc.tensor.matmul(*a, **k)

    with ExitStack() as ctx:
        tc = ctx.enter_context(tile.TileContext(nc))
        cp = ctx.enter_context(tc.tile_pool(name="consts", bufs=1))
        sb = ctx.enter_context(tc.tile_pool(name="sbuf", bufs=2))
        hb = ctx.enter_context(tc.tile_pool(name="hbuf", bufs=8))
        eb = ctx.enter_context(tc.tile_pool(name="ebuf", bufs=3))
        pA = ctx.enter_context(tc.tile_pool(name="pA", bufs=1, space="PSUM"))
        pB = ctx.enter_context(tc.tile_pool(name="pB", bufs=1, space="PSUM"))
        pG = ctx.enter_context(tc.tile_pool(name="pG", bufs=4, space="PSUM"))

        # ---- load packed constants: one tile + one DMA per chunk, in
        # first-consumer order; ct[] = slices into the chunk tiles
        dma_engs = [nc.sync, nc.gpsimd]
        ct = {}
        for chunk, parts, dt, entries in _PACKS:
            ncols = sum(e[2] for e in entries)
            t = cp.tile([parts, ncols], dt, tag=chunk, name=chunk)
            dma_engs[_PACK_ENG[chunk]].dma_start(t[:, :], dr[chunk][:, :])
            off = 0
            for name, rows, cols in entries:
                ct[name] = t[0:rows, off : off + cols]
                off += cols

        # ---- ep tables (embedding partials) on PE; psum->sbuf copies on ACT
        ep_im = eb.tile([H, A], BF16, tag="ep_im")
        for j in range(2):
            pe_h = pG.tile([H, 512], F32, tag="lgh", name=f"pei{j}")
            MM(pe_h[:, :], ct["wime1"][:, :], ct["embrd"][:, 512 * j : 512 * (j + 1)],
               start=True, stop=False)
            MM(pe_h[:, :], ct["wime2"][:, :], ct["embim2"][:, 512 * j : 512 * (j + 1)],
               start=False, stop=True)
            nc.scalar.activation(ep_im[:, 512 * j : 512 * (j + 1)], pe_h[:, :],
                                 AF.Identity)
        U4, NPG = 704, 176
        ep_rd = eb.tile([H, U4], BF16, tag="ep_rd")
        for j, (c0, c1) in enumerate(((0, 512), (512, U4))):
            pe_h = pG.tile([H, c1 - c0], F32, tag="lgh", name=f"pep{j}",
                           padded_shape=[H, 512])
            MM(pe_h[:, :], ct["wrde"][:, :], ct["embrd_u"][:, c0:c1])
            nc.scalar.activation(ep_rd[:, c0:c1], pe_h[:, :], AF.Identity)
        psum_ep3 = pG.tile([H, NOP], F32, tag="lgh", padded_shape=[H, 512])
        MM(psum_ep3[:, :], ct["wrse"][:, :], ct["embrs"][:, :])
        ep_rs = eb.tile([H, NOP], BF16, tag="ep_rs")
        nc.scalar.activation(ep_rs[:, :], psum_ep3[:, :], AF.Identity)

        # ---- fp (feature partials): one [H, 4*BL] psum tile, one
        # accumulation group per head, emitted head-by-head as w1 DMAs land
        psum_fp = pB.tile([H, 4 * BL], F32, tag="seqB", padded_shape=[H, 512])
        fp = {}

        def emit_fp(hd, nm):
            for k in range(4):
                MM(
                    psum_fp[:, 32 * hd : 32 * hd + BL],
                    ct["w1" + nm][:, 128 * k : 128 * (k + 1)],
                    ct["featT"][:, BL * k : BL * (k + 1)],
                    start=(k == 0),
                    stop=(k == 3),
                )
            if nm == "op":
                t = sb.tile([H, BL], BF16, tag="op_h", name="op_h")
                nc.scalar.activation(t[:, :], psum_fp[:, 96:128], AF.Relu,
                                     bias=ct["b1s"][:, 3:4])
            else:
                t = sb.tile([H, BL], F32, tag=f"fp_{nm}", name=f"fp_{nm}")
                nc.scalar.activation(t[:, :], psum_fp[:, 32 * hd : 32 * hd + BL],
                                     AF.Identity, bias=ct["b1s"][:, hd : hd + 1])
            fp[nm] = t

        emit_fp(0, "im")

        # ---- deferred op/rs head pieces (emitted inside the imm loop)
        st = {}

        def emit_h_rs(b0, b1):
            if "h_rs" not in st:
                st["h_rs"] = cp.tile([H, NOP * BL], BF16, tag="h_rs", name="h_rs")
            for b in range(b0, b1):
                nc.gpsimd.tensor_scalar(
                    st["h_rs"][:, NOP * b : NOP * (b + 1)],
                    ep_rs[:, :], fp["rs"][:, b : b + 1], 0.0,
                    op0=ALU.add, op1=ALU.max,
                )

        def emit_op_logits():
            psum_opl = pG.tile([NO, BL], F32, tag="lgh", padded_shape=[NO, 512],
                               name="psum_opl")
            MM(psum_opl[:, :], ct["w2opT"][:, :], fp["op"][:, :])
            st["exp_op"] = sb.tile([NO, BL], BF16, tag="exp_op", name="exp_op")
            nc.scalar.activation(st["exp_op"][:, :], psum_opl[:, :], AF.Exp,
                                 bias=ct["b2op"][:, :])
            st["lb2_op"] = sb.tile([NO, BL], BF16, tag="lb2_op", name="lb2_op")
            nc.scalar.activation(st["lb2_op"][:, :], psum_opl[:, :], AF.Identity,
                                 bias=ct["b2op"][:, :])

        def emit_rs_logits():
            h_rs_v = st["h_rs"][:, :].rearrange("p (b c) -> p c b", c=NOP)
            psum_rsl = pB.tile([H, 32 * NGRP], F32, tag="seqB", name="psum_rsl")
            for c_ in range(NOP):
                g, s = c_ // 4, c_ % 4
                MM(
                    psum_rsl[32 * s : 32 * s + 32, 32 * g : 32 * g + 32],
                    ct["w2rsT"][:, :],
                    h_rs_v[:, c_, :],
                    tile_position=(0, 32 * s),
                )
            st["exp_rs"] = sb.tile([H, 32 * NGRP], BF16, tag="exp_rs", name="exp_rs")
            nc.scalar.activation(st["exp_rs"][:, :], psum_rsl[:, :], AF.Exp,
                                 bias=ct["b2rs"][:, :])
            st["lb2_rs"] = sb.tile([H, 32 * NGRP], BF16, tag="lb2_rs", name="lb2_rs")
            nc.scalar.activation(st["lb2_rs"][:, :], psum_rsl[:, :], AF.Identity,
                                 bias=ct["b2rs"][:, :])

        def emit_su_sections():
            psum_osu = pG.tile([1, BL], F32, tag="lgh", padded_shape=[1, 512],
                               name="psum_osu")
            MM(psum_osu[:, :], ct["onescol"][0:NO, :], st["exp_op"][:, :])
            st["lnsu_op"] = sb.tile([1, BL], BF16, tag="lnsu_op", name="lnsu_op")
            nc.scalar.activation(st["lnsu_op"][:, :], psum_osu[:, :], AF.Ln)
            psum_rsu = pG.tile([4, 512], F32, tag="lgh", name="psum_rsu")
            MM(psum_rsu[:, :], ct["suind_rs"][:, :], st["exp_rs"][:, 0:512])
            psum_rsu2 = pG.tile([4, 32 * NGRP - 512], F32, tag="lgh",
                                padded_shape=[4, 512], name="psum_rsu2")
            MM(psum_rsu2[:, :], ct["suind_rs"][:, :], st["exp_rs"][:, 512 : 32 * NGRP])
            st["lnsu_rs"] = sb.tile([4, 32 * NGRP], BF16, tag="lnsu_rs",
                                    name="lnsu_rs")
            nc.scalar.activation(st["lnsu_rs"][:, 0:512], psum_rsu[:, :], AF.Ln)
            nc.scalar.activation(st["lnsu_rs"][:, 512 : 32 * NGRP],
                                 psum_rsu2[:, :], AF.Ln)

        def emit_acc():
            psum_acc = pA.tile([BL, A], F32, tag="seqA", name="psum_acc")
            for j in range(2):
                MM(psum_acc[:, 512 * j : 512 * (j + 1)], st["lb2_op"][:, :],
                   ct["gop"][:, 512 * j : 512 * (j + 1)], start=True, stop=False)
                MM(psum_acc[:, 512 * j : 512 * (j + 1)], st["lnsu_op"][:, :],
                   ct["negones"][:, 512 * j : 512 * (j + 1)], start=False,
                   stop=False)
                MM(psum_acc[:, 512 * j : 512 * (j + 1)], ct["onesrow"][:, :],
                   ct["mdb2"][:, 512 * j : 512 * (j + 1)], start=False, stop=False)
            last_for_bank = {}
            for i, (g, lo, hi) in enumerate(chunks):
                last_for_bank[lo // 512] = i
            for i, (g, lo, hi) in enumerate(chunks):
                MM(psum_acc[:, lo:hi], st["lb2_rs"][:, 32 * g : 32 * g + 32],
                   ct["grs"][:, lo:hi], start=False, stop=False)
                MM(psum_acc[:, lo:hi], st["lnsu_rs"][:, 32 * g : 32 * g + 32],
                   ct["g2rs"][:, lo:hi], start=False, stop=False)
            st["psum_acc"] = psum_acc

        # ---- rd-head h table (deduplicated over (o, rs) pairs), b-major
        h_rd_all = cp.tile([H, U4 * BL], BF16, tag="h_rd_all", name="h_rd_all")

        def emit_h_rd(b0, b1, eng):
            for b in range(b0, b1):
                eng.tensor_scalar(
                    h_rd_all[:, U4 * b : U4 * (b + 1)],
                    ep_rd[:, :], fp["rd"][:, b : b + 1], 0.0,
                    op0=ALU.add, op1=ALU.max,
                )

        # ---- imm head phase: h_im tiles + strip matmuls (every psum row of
        # strip s holds d for b=4g+s) -> bf16 copies into d_all -> one
        # SBUF->SBUF gather DMA per strip into compact X32[32, A].
        # op/rs/acc sections are emitted between waves so each engine's
        # in-order queue reaches them right as their inputs land.
        X32 = cp.tile([BL, A], BF16, tag="X32", name="X32")
        d_all = cp.tile([H, 16 * 512], BF16, tag="d_all", name="d_all")
        CP_ENG = ["A", "A", "A", "D"]  # psum reads: ACT/DVE only
        for g in range(8):
            hts_im = []
            for s in range(4):
                b = 4 * g + s
                h_t = hb.tile([H, A], BF16, tag="h", name=f"him{b}")
                nc.vector.tensor_scalar(
                    h_t[:, :], ep_im[:, :], fp["im"][:, b : b + 1], 0.0,
                    op0=ALU.add, op1=ALU.max,
                )
                hts_im.append(h_t)
            for j in range(2):
                psum_d = pG.tile([H, 512], F32, tag="lgh", name=f"d{g}{j}")
                for s in range(4):
                    MM(
                        psum_d[32 * s : 32 * s + 32, :],
                        ct["wd32"][:, :],
                        hts_im[s][:, 512 * j : 512 * (j + 1)],
                        tile_position=(0, 32 * s),
                    )
                dsl = d_all[:, 512 * (2 * g + j) : 512 * (2 * g + j + 1)]
                lane = CP_ENG[(2 * g + j) % len(CP_ENG)]
                if lane == "A":
                    nc.scalar.activation(dsl, psum_d[:, :], AF.Identity)
                else:
                    nc.vector.tensor_copy(dsl, psum_d[:, :])
            if g == 0:
                emit_fp(1, "rs")
            elif g == 1:
                emit_fp(2, "rd")
                emit_h_rs(0, 16)
            elif g == 2:
                emit_fp(3, "op")
                emit_h_rs(16, BL)
                emit_h_rd(0, 2, nc.vector)
            elif g == 3:
                emit_op_logits()
                emit_h_rd(2, 4, nc.vector)
            elif g == 4:
                emit_rs_logits()
                emit_h_rd(4, 6, nc.vector)
            elif g == 5:
                emit_su_sections()
                emit_h_rd(6, 8, nc.vector)
                emit_h_rd(24, 27, nc.gpsimd)
            elif g == 6:
                emit_h_rd(8, 11, nc.vector)
                emit_h_rd(27, 30, nc.gpsimd)
            elif g == 7:
                emit_h_rd(11, 14, nc.vector)
                emit_h_rd(30, BL, nc.gpsimd)
        # gather: X32[4g+s, 512j+c] = d_all[32s, (2g+j)*512 + c]
        for s in range(4):
            dma_engs[s % 2].dma_start(
                X32[s : BL : 4, :].rearrange("g (j c) -> g j c", j=2),
                d_all[32 * s : 32 * s + 1, :].rearrange(
                    "p (g j c) -> p g j c", g=8, j=2
                ),
            )

        # finish h_rd, then acc gathers overlap the DVE tail
        emit_h_rd(14, 22, nc.vector)
        for b in (22, 23):
            nc.scalar.activation(
                h_rd_all[:, U4 * b : U4 * (b + 1)], ep_rd[:, :], AF.Relu,
                bias=fp["rd"][:, b : b + 1],
            )
        emit_acc()

        # ---- rd head: deduplicated logits table [(strip, v), (pgrp, b)]
        # in 11 psum waves; exp on ACT; su-reduce (4 cols per pair-group)
        sp32 = cp.tile([BL, A], BF16, tag="sp32")
        u32 = cp.tile([BL, A], BF16, tag="u32")
        ctr_im = cp.tile([BL, A], BF16, tag="ctr_im")
        h_rd_v = h_rd_all[:, :].rearrange("p (b c) -> p c b", c=U4)
        exp_tbl = cp.tile([H, 32 * NPG], BF16, tag="exp_tbl", name="exp_tbl")
        psum_sutbl = pB.tile([BL, U4], F32, tag="seqB", name="psum_sutbl")

        # imm-head tail: sp = softplus(X + db2) = ln(1 + e^(X+db2)),
        # u = X*m, ctr_im = u - sp (db2*m is already in the acc); emitted
        # here so ACT does it in the imm->rd transition, not mid table-loop
        e32 = cp.tile([BL, A], BF16, tag="e32", name="e32")
        nc.scalar.activation(e32[:, :], X32[:, :], AF.Exp,
                             bias=ct["db2im"][:, :])
        e1 = cp.tile([BL, A], BF16, tag="e1", name="e1")
        nc.vector.tensor_scalar_add(e1[:, :], e32[:, :], 1.0)
        nc.scalar.activation(sp32[:, :], e1[:, :], AF.Ln)
        nc.vector.tensor_mul(u32[:, :], X32[:, :], ct["m32"][:, :])
        nc.vector.tensor_sub(ctr_im[:, :], u32[:, :], sp32[:, :])

        def emit_su(t):
            # su-reduce for tile t, emitted one tile late so exp(t) is done
            # and the PE wait-queue never blocks mid-loop
            for pgl in range(16):
                pg = 16 * t + pgl
                MM(
                    psum_sutbl[:, 4 * pg : 4 * pg + 4],
                    exp_tbl[:, 512 * t + 32 * pgl : 512 * t + 32 * pgl + 32],
                    ct["suind_rs"][:, :],
                )

        for t in range(NPG // 16):
            ptbl = pG.tile([H, 512], F32, tag="lgh", name=f"tbl{t}")
            for pl in range(64):
                p_ = 64 * t + pl
                s = p_ % 4
                pgl = (p_ // 4) % 16
                MM(
                    ptbl[32 * s : 32 * s + 32, 32 * pgl : 32 * pgl + 32],
                    ct["w2rdT"][:, :],
                    h_rd_v[:, p_, :],
                    tile_position=(0, 32 * s),
                )
            nc.scalar.activation(exp_tbl[:, 512 * t : 512 * (t + 1)], ptbl[:, :],
                                 AF.Exp, bias=ct["b2rd"][:, :])
            if t >= 1:
                emit_su(t - 1)

        emit_su(NPG // 16 - 1)

        # ---- lnsu table [32, U4] -> transpose to [pair, b] chunks
        lnsu_tbl = cp.tile([BL, 768], BF16, tag="lnsu_tbl", name="lnsu_tbl")
        nc.scalar.activation(lnsu_tbl[:, 0:U4], psum_sutbl[:, :], AF.Ln)
        nc.vector.memset(lnsu_tbl[:, U4:768], 0.0)
        lnsuT = cp.tile([H, 6 * 32], BF16, tag="lnsuT", name="lnsuT")
        for ch in range(6):
            pt = pG.tile([H, 32], BF16, tag="lgh", name=f"ptr{ch}",
                         padded_shape=[H, 512])
            nc.tensor.transpose(pt[:, :], lnsu_tbl[:, 128 * ch : 128 * (ch + 1)],
                                ct["id32"][:, :])
            nc.vector.tensor_copy(lnsuT[:, 32 * ch : 32 * (ch + 1)], pt[:, :])

        # ---- gathers: exp_sel into psum_q (by pair-group), -lnsu into the
        # open accumulator (by 128-pair chunk); bank-wise start/stop
        psum_q = pB.tile([BL, A], F32, tag="seqB", name="psum_q")
        qfirst, qlast = {}, {}
        for i, (pg, lo, hi) in enumerate(qchunks):
            b = lo // 512
            qfirst.setdefault(b, i)
            qlast[b] = i
        for i, (pg, lo, hi) in enumerate(qchunks):
            b = lo // 512
            MM(
                psum_q[:, lo:hi],
                exp_tbl[:, 32 * pg : 32 * pg + 32],
                ct["grd"][:, lo:hi],
                start=(qfirst[b] == i),
                stop=(qlast[b] == i),
            )
        lnsel = cp.tile([BL, A], BF16, tag="lnsel")
        for j in range(2):
            nc.scalar.activation(lnsel[:, 512 * j : 512 * (j + 1)],
                                 psum_q[:, 512 * j : 512 * (j + 1)], AF.Ln)
        w32 = cp.tile([BL, A], BF16, tag="w32")
        nc.vector.tensor_add(w32[:, :], ctr_im[:, :], lnsel[:, :])

        llast = {}
        for i, (ch, lo, hi) in enumerate(lchunks):
            llast[lo // 512] = i
        psum_acc = st["psum_acc"]
        for i, (ch, lo, hi) in enumerate(lchunks):
            MM(
                psum_acc[:, lo:hi],
                lnsuT[:, 32 * ch : 32 * ch + 32],
                ct["gpair"][:, lo:hi],
                start=False,
                stop=(llast[lo // 512] == i),
            )

        # ---- final combine + store, per psum bank
        for j in range(2):
            sl = slice(512 * j, 512 * (j + 1))
            out_sb = sb.tile([BL, 512], F32, tag="out_sb", name=f"out{j}")
            nc.vector.tensor_add(out_sb[:, :], w32[:, sl], psum_acc[:, sl])
            nc.sync.dma_start(out_d[:, sl], out_sb[:, :])

    return nc.tensor.matmul(*a, **k)

    with ExitStack() as ctx:
        tc = ctx.enter_context(tile.TileContext(nc))
        cp = ctx.enter_context(tc.tile_pool(name="consts", bufs=1))
        sb = ctx.enter_context(tc.tile_pool(name="sbuf", bufs=2))
        hb = ctx.enter_context(tc.tile_pool(name="hbuf", bufs=8))
        eb = ctx.enter_context(tc.tile_pool(name="ebuf", bufs=3))
        pA = ctx.enter_context(tc.tile_pool(name="pA", bufs=1, space="PSUM"))
        pB = ctx.enter_context(tc.tile_pool(name="pB", bufs=1, space="PSUM"))
        pG = ctx.enter_context(tc.tile_pool(name="pG", bufs=4, space="PSUM"))

        # ---- load packed constants: one tile + one DMA per chunk, in
        # first-consumer order; ct[] = slices into the chunk tiles
        dma_engs = [nc.sync, nc.gpsimd]
        ct = {}
        for chunk, parts, dt, entries in _PACKS:
            ncols = sum(e[2] for e in entries)
            t = cp.tile([parts, ncols], dt, tag=chunk, name=chunk)
            dma_engs[_PACK_ENG[chunk]].dma_start(t[:, :], dr[chunk][:, :])
            off = 0
            for name, rows, cols in entries:
                ct[name] = t[0:rows, off : off + cols]
                off += cols

        # ---- ep tables (embedding partials) on PE; psum->sbuf copies on ACT
        ep_im = eb.tile([H, A], BF16, tag="ep_im")
        for j in range(2):
            pe_h = pG.tile([H, 512], F32, tag="lgh", name=f"pei{j}")
            MM(pe_h[:, :], ct["wime1"][:, :], ct["embrd"][:, 512 * j : 512 * (j + 1)],
               start=True, stop=False)
            MM(pe_h[:, :], ct["wime2"][:, :], ct["embim2"][:, 512 * j : 512 * (j + 1)],
               start=False, stop=True)
            nc.scalar.activation(ep_im[:, 512 * j : 512 * (j + 1)], pe_h[:, :],
                                 AF.Identity)
        U4, NPG = 704, 176
        ep_rd = eb.tile([H, U4], BF16, tag="ep_rd")
        for j, (c0, c1) in enumerate(((0, 512), (512, U4))):
            pe_h = pG.tile([H, c1 - c0], F32, tag="lgh", name=f"pep{j}",
                           padded_shape=[H, 512])
            MM(pe_h[:, :], ct["wrde"][:, :], ct["embrd_u"][:, c0:c1])
            nc.scalar.activation(ep_rd[:, c0:c1], pe_h[:, :], AF.Identity)
        psum_ep3 = pG.tile([H, NOP], F32, tag="lgh", padded_shape=[H, 512])
        MM(psum_ep3[:, :], ct["wrse"][:, :], ct["embrs"][:, :])
        ep_rs = eb.tile([H, NOP], BF16, tag="ep_rs")
        nc.scalar.activation(ep_rs[:, :], psum_ep3[:, :], AF.Identity)

        # ---- fp (feature partials): one [H, 4*BL] psum tile, one
        # accumulation group per head, emitted head-by-head as w1 DMAs land
        psum_fp = pB.tile([H, 4 * BL], F32, tag="seqB", padded_shape=[H, 512])
        fp = {}

        def emit_fp(hd, nm):
            for k in range(4):
                MM(
                    psum_fp[:, 32 * hd : 32 * hd + BL],
                    ct["w1" + nm][:, 128 * k : 128 * (k + 1)],
                    ct["featT"][:, BL * k : BL * (k + 1)],
                    start=(k == 0),
                    stop=(k == 3),
                )
            if nm == "op":
                t = sb.tile([H, BL], BF16, tag="op_h", name="op_h")
                nc.scalar.activation(t[:, :], psum_fp[:, 96:128], AF.Relu,
                                     bias=ct["b1s"][:, 3:4])
            else:
                t = sb.tile([H, BL], F32, tag=f"fp_{nm}", name=f"fp_{nm}")
                nc.scalar.activation(t[:, :], psum_fp[:, 32 * hd : 32 * hd + BL],
                                     AF.Identity, bias=ct["b1s"][:, hd : hd + 1])
            fp[nm] = t

        emit_fp(0, "im")

        # ---- deferred op/rs head pieces (emitted inside the imm loop)
        st = {}

        def emit_h_rs(b0, b1):
            if "h_rs" not in st:
                st["h_rs"] = cp.tile([H, NOP * BL], BF16, tag="h_rs", name="h_rs")
            for b in range(b0, b1):
                nc.gpsimd.tensor_scalar(
                    st["h_rs"][:, NOP * b : NOP * (b + 1)],
                    ep_rs[:, :], fp["rs"][:, b : b + 1], 0.0,
                    op0=ALU.add, op1=ALU.max,
                )

        def emit_op_logits():
            psum_opl = pG.tile([NO, BL], F32, tag="lgh", padded_shape=[NO, 512],
                               name="psum_opl")
            MM(psum_opl[:, :], ct["w2opT"][:, :], fp["op"][:, :])
            st["exp_op"] = sb.tile([NO, BL], BF16, tag="exp_op", name="exp_op")
            nc.scalar.activation(st["exp_op"][:, :], psum_opl[:, :], AF.Exp,
                                 bias=ct["b2op"][:, :])
            st["lb2_op"] = sb.tile([NO, BL], BF16, tag="lb2_op", name="lb2_op")
            nc.scalar.activation(st["lb2_op"][:, :], psum_opl[:, :], AF.Identity,
                                 bias=ct["b2op"][:, :])

        def emit_rs_logits():
            h_rs_v = st["h_rs"][:, :].rearrange("p (b c) -> p c b", c=NOP)
            psum_rsl = pB.tile([H, 32 * NGRP], F32, tag="seqB", name="psum_rsl")
            for c_ in range(NOP):
                g, s = c_ // 4, c_ % 4
                MM(
                    psum_rsl[32 * s : 32 * s + 32, 32 * g : 32 * g + 32],
                    ct["w2rsT"][:, :],
                    h_rs_v[:, c_, :],
                    tile_position=(0, 32 * s),
                )
            st["exp_rs"] = sb.tile([H, 32 * NGRP], BF16, tag="exp_rs", name="exp_rs")
            nc.scalar.activation(st["exp_rs"][:, :], psum_rsl[:, :], AF.Exp,
                                 bias=ct["b2rs"][:, :])
            st["lb2_rs"] = sb.tile([H, 32 * NGRP], BF16, tag="lb2_rs", name="lb2_rs")
            nc.scalar.activation(st["lb2_rs"][:, :], psum_rsl[:, :], AF.Identity,
                                 bias=ct["b2rs"][:, :])

        def emit_su_sections():
            psum_osu = pG.tile([1, BL], F32, tag="lgh", padded_shape=[1, 512],
                               name="psum_osu")
            MM(psum_osu[:, :], ct["onescol"][0:NO, :], st["exp_op"][:, :])
            st["lnsu_op"] = sb.tile([1, BL], BF16, tag="lnsu_op", name="lnsu_op")
            nc.scalar.activation(st["lnsu_op"][:, :], psum_osu[:, :], AF.Ln)
            psum_rsu = pG.tile([4, 512], F32, tag="lgh", name="psum_rsu")
            MM(psum_rsu[:, :], ct["suind_rs"][:, :], st["exp_rs"][:, 0:512])
            psum_rsu2 = pG.tile([4, 32 * NGRP - 512], F32, tag="lgh",
                                padded_shape=[4, 512], name="psum_rsu2")
            MM(psum_rsu2[:, :], ct["suind_rs"][:, :], st["exp_rs"][:, 512 : 32 * NGRP])
            st["lnsu_rs"] = sb.tile([4, 32 * NGRP], BF16, tag="lnsu_rs",
                                    name="lnsu_rs")
            nc.scalar.activation(st["lnsu_rs"][:, 0:512], psum_rsu[:, :], AF.Ln)
            nc.scalar.activation(st["lnsu_rs"][:, 512 : 32 * NGRP],
                                 psum_rsu2[:, :], AF.Ln)

        def emit_acc():
            psum_acc = pA.tile([BL, A], F32, tag="seqA", name="psum_acc")
            for j in range(2):
                MM(psum_acc[:, 512 * j : 512 * (j + 1)], st["lb2_op"][:, :],
                   ct["gop"][:, 512 * j : 512 * (j + 1)], start=True, stop=False)
                MM(psum_acc[:, 512 * j : 512 * (j + 1)], st["lnsu_op"][:, :],
                   ct["negones"][:, 512 * j : 512 * (j + 1)], start=False,
                   stop=False)
                MM(psum_acc[:, 512 * j : 512 * (j + 1)], ct["onesrow"][:, :],
                   ct["mdb2"][:, 512 * j : 512 * (j + 1)], start=False, stop=False)
            last_for_bank = {}
            for i, (g, lo, hi) in enumerate(chunks):
                last_for_bank[lo // 512] = i
            for i, (g, lo, hi) in enumerate(chunks):
                MM(psum_acc[:, lo:hi], st["lb2_rs"][:, 32 * g : 32 * g + 32],
                   ct["grs"][:, lo:hi], start=False, stop=False)
                MM(psum_acc[:, lo:hi], st["lnsu_rs"][:, 32 * g : 32 * g + 32],
                   ct["g2rs"][:, lo:hi], start=False, stop=False)
            st["psum_acc"] = psum_acc

        # ---- rd-head h table (deduplicated over (o, rs) pairs), b-major
        h_rd_all = cp.tile([H, U4 * BL], BF16, tag="h_rd_all", name="h_rd_all")

        def emit_h_rd(b0, b1, eng):
            for b in range(b0, b1):
                eng.tensor_scalar(
                    h_rd_all[:, U4 * b : U4 * (b + 1)],
                    ep_rd[:, :], fp["rd"][:, b : b + 1], 0.0,
                    op0=ALU.add, op1=ALU.max,
                )

        # ---- imm head phase: h_im tiles + strip matmuls (every psum row of
        # strip s holds d for b=4g+s) -> bf16 copies into d_all -> one
        # SBUF->SBUF gather DMA per strip into compact X32[32, A].
        # op/rs/acc sections are emitted between waves so each engine's
        # in-order queue reaches them right as their inputs land.
        X32 = cp.tile([BL, A], BF16, tag="X32", name="X32")
        d_all = cp.tile([H, 16 * 512], BF16, tag="d_all", name="d_all")
        CP_ENG = ["A", "A", "A", "D"]  # psum reads: ACT/DVE only
        for g in range(8):
            hts_im = []
            for s in range(4):
                b = 4 * g + s
                h_t = hb.tile([H, A], BF16, tag="h", name=f"him{b}")
                nc.vector.tensor_scalar(
                    h_t[:, :], ep_im[:, :], fp["im"][:, b : b + 1], 0.0,
                    op0=ALU.add, op1=ALU.max,
                )
                hts_im.append(h_t)
            for j in range(2):
                psum_d = pG.tile([H, 512], F32, tag="lgh", name=f"d{g}{j}")
                for s in range(4):
                    MM(
                        psum_d[32 * s : 32 * s + 32, :],
                        ct["wd32"][:, :],
                        hts_im[s][:, 512 * j : 512 * (j + 1)],
                        tile_position=(0, 32 * s),
                    )
                dsl = d_all[:, 512 * (2 * g + j) : 512 * (2 * g + j + 1)]
                lane = CP_ENG[(2 * g + j) % len(CP_ENG)]
                if lane == "A":
                    nc.scalar.activation(dsl, psum_d[:, :], AF.Identity)
                else:
                    nc.vector.tensor_copy(dsl, psum_d[:, :])
            if g == 0:
                emit_fp(1, "rs")
            elif g == 1:
                emit_fp(2, "rd")
                emit_h_rs(0, 16)
            elif g == 2:
                emit_fp(3, "op")
                emit_h_rs(16, BL)
                emit_h_rd(0, 2, nc.vector)
            elif g == 3:
                emit_op_logits()
                emit_h_rd(2, 4, nc.vector)
            elif g == 4:
                emit_rs_logits()
                emit_h_rd(4, 6, nc.vector)
            elif g == 5:
                emit_su_sections()
                emit_h_rd(6, 8, nc.vector)
                emit_h_rd(24, 27, nc.gpsimd)
            elif g == 6:
                emit_h_rd(8, 11, nc.vector)
                emit_h_rd(27, 30, nc.gpsimd)
            elif g == 7:
                emit_h_rd(11, 14, nc.vector)
                emit_h_rd(30, BL, nc.gpsimd)
        # gather: X32[4g+s, 512j+c] = d_all[32s, (2g+j)*512 + c]
        for s in range(4):
            dma_engs[s % 2].dma_start(
                X32[s : BL : 4, :].rearrange("g (j c) -> g j c", j=2),
                d_all[32 * s : 32 * s + 1, :].rearrange(
                    "p (g j c) -> p g j c", g=8, j=2
                ),
            )

        # ---- rd head phase (imm tail ops are emitted after wave g==3 so
        # the engines reach them right around when X32's gather DMA lands)
        sp32 = cp.tile([BL, A], BF16, tag="sp32")
        u32 = cp.tile([BL, A], BF16, tag="u32")
        ctr_im = cp.tile([BL, A], BF16, tag="ctr_im")
        psum_su = pA.tile([BL, A], F32, tag="seqA")
        psum_sel = pB.tile([BL, A], F32, tag="seqB")
        for g in range(8):
            hts_rd = []
            for s in range(4):
                b = 4 * g + s
                h_t = hb.tile([H, A], BF16, tag="h", name=f"hrd{b}")
                nc.vector.tensor_scalar(
                    h_t[:, :], ep_rd[:, :], fp["rd"][:, b : b + 1], 0.0,
                    op0=ALU.add, op1=ALU.max,
                )
                hts_rd.append(h_t)
            exp_t = sb.tile([H, A], BF16, tag="exp_t")
            mexp_t = sb.tile([H, A], BF16, tag="mexp_t")
            for j in range(2):
                psum_lg = pG.tile([H, 512], F32, tag="lgh", name=f"lg{g}{j}")
                for s in range(4):
                    MM(
                        psum_lg[32 * s : 32 * s + 32, :],
                        ct["w2rdT"][:, :],
                        hts_rd[s][:, 512 * j : 512 * (j + 1)],
                        tile_position=(0, 32 * s),
                    )
                nc.scalar.activation(
                    exp_t[:, 512 * j : 512 * (j + 1)], psum_lg[:, :], AF.Exp,
                    bias=ct["b2rd"][:, :],
                )
                nc.vector.tensor_mul(
                    mexp_t[:, 512 * j : 512 * (j + 1)],
                    exp_t[:, 512 * j : 512 * (j + 1)],
                    ct["mask_rd"][:, 512 * j : 512 * (j + 1)],
                )
            for j in range(2):
                MM(
                    psum_su[:, 512 * j : 512 * (j + 1)],
                    ct["suind_rd"][:, 32 * g : 32 * g + 32],
                    exp_t[:, 512 * j : 512 * (j + 1)],
                    start=(g == 0),
                    stop=(g == 7),
                )
                MM(
                    psum_sel[:, 512 * j : 512 * (j + 1)],
                    ct["suind_rd"][:, 32 * g : 32 * g + 32],
                    mexp_t[:, 512 * j : 512 * (j + 1)],
                    start=(g == 0),
                    stop=(g == 7),
                )
            if g == 3:
                # imm-head tail: sp = softplus(X + db2) = ln(1 + e^(X+db2))
                # (built from Exp/Ln, which are already ACT-table residents),
                # u = X*m, ctr_im = u - sp (db2*m is already in the acc)
                e32 = cp.tile([BL, A], BF16, tag="e32", name="e32")
                nc.scalar.activation(e32[:, :], X32[:, :], AF.Exp,
                                     bias=ct["db2im"][:, :])
                e1 = cp.tile([BL, A], BF16, tag="e1", name="e1")
                nc.vector.tensor_scalar_add(e1[:, :], e32[:, :], 1.0)
                nc.scalar.activation(sp32[:, :], e1[:, :], AF.Ln)
                nc.vector.tensor_mul(u32[:, :], X32[:, :], ct["m32"][:, :])
                nc.vector.tensor_sub(ctr_im[:, :], u32[:, :], sp32[:, :])
            if g == 4:
                t3 = sb.tile([BL, A], BF16, tag="t3")
                nc.vector.tensor_add(t3[:, :], ctr_im[:, :], st["acc_sb"][:, :])

        # ---- rd-head tail + final combine + store, pipelined by psum bank
        for j in range(2):
            sl = slice(512 * j, 512 * (j + 1))
            lnsu_t = sb.tile([BL, 512], BF16, tag="lnsu_rdt", name=f"lnsu{j}")
            nc.scalar.activation(lnsu_t[:, :], psum_su[:, sl], AF.Ln)
            lnsel_t = sb.tile([BL, 512], BF16, tag="lnsel_rdt", name=f"lnsel{j}")
            nc.scalar.activation(lnsel_t[:, :], psum_sel[:, sl], AF.Ln)
            ctr_rd = sb.tile([BL, 512], BF16, tag="ctr_rd", name=f"ctrrd{j}")
            nc.vector.tensor_sub(ctr_rd[:, :], lnsel_t[:, :], lnsu_t[:, :])
            out_sb = sb.tile([BL, 512], F32, tag="out_sb", name=f"out{j}")
            nc.vector.tensor_add(out_sb[:, :], ctr_rd[:, :], t3[:, sl])
            nc.sync.dma_start(out_d[:, sl], out_sb[:, :])

    return nc


_CACHE = {}


def _get_program(chunks):
    key = chunks
    if key not in _CACHE:
        _CACHE[key] = build_program(chunks)
    return _CACHE[key]


def kernel(**inputs) -> np.ndarray:
    packed, per_core, chunks, perm = _host_prep(inputs)
    nc = _get_program(chunks)
    in_maps = []
    for cid in range(NCORES):
        m = {k: np.ascontiguousarray(v) for k, v in packed.items()}
        m["c_feat"] = np.ascontiguousarray(_bf(per_core[cid]["featT"]))
        in_maps.append(m)
    res = run_bass_kernel_spmd(nc, in_maps, core_ids=list(range(NCORES)))
    out_sorted = np.concatenate(
        [res.results[cid]["out"] for cid in range(NCORES)], axis=0
    )  # [B, A] in sorted-action order
    out = np.empty_like(out_sorted)
    out[:, perm] = out_sorted
    return out.astype(np.float32)


# revision 37
# speedup vs baseline: 1.0396x; 1.0090x over previous
"""Trainium2 Bass kernel for nn_AutoregressiveInstructionHead.

Data-parallel over batch B=256 across 8 NeuronCores (32 rows each).
Head weights / embeddings / action-derived tables are replicated.

Per-core device pipeline (all heavy compute on device):
  - constants packed into a few DRAM tensors -> few big DMAs ordered by
    first consumer (HWDGE per-DMA overhead is ~625ns, so fewer transfers
    and compute starts ~3.5us in instead of ~18us)
  - fp_head = features @ W1_feat.T (+b1)  -> [H=128, B=32] via PE
  - ep tables = embeddings @ W1_emb.T     -> [H=128, A] via PE
  - imm head (NI=2): logp = m*X - softplus(X) with X = l0 - l1, so only a
    single M=1 matmul strip per (b, 512-col block) with w_d = W2[0]-W2[1];
    the strip-scattered psum rows are gathered to a compact X32[32,A] tile
    by a psum->SBUF DMA (free on the DMA engines), then one softplus + two
    DVE ops finish the head.  No exp / mask / su-sel matmuls needed.
  - op head: logits -> exp/sum/ln -> gather via one-hot matmul into PSUM acc
  - rs head: deduplicated over the 65 opcodes ([B,65,17] table), gathered
    back to the 1024 actions with block-sparse one-hot matmuls (actions are
    host-sorted by (opcode, reg_src) so each table chunk touches a
    contiguous column range; inverse permutation applied on host at the end)
  - rd head: h=relu(fp[b]+ep[a]) [128,1024] per b, logits matmul with
    col-tiling (4 b's concurrently in 32-partition strips), exp(+b2) on ACT,
    one-hot mask multiply, partition sums via indicator matmuls,
    contribution = ln(sum mask*exp) - ln(sum exp)
"""

import sys

for _p in ("/opt/trn_rl_repo",):
    if _p not in sys.path:
        sys.path.insert(0, _p)

import numpy as np
from contextlib import ExitStack

import json

import concourse.bass as bass
import concourse.tile as tile
from concourse import mybir
from concourse import bass2jax as _bass2jax
from concourse.bass_utils import run_bass_kernel_spmd
from concourse.bass_utils import compile_bir_kernel as _orig_compile_bir_kernel

# --- workaround: this container's walrus rejects instructions carrying more
# than one sync-wait command ("Too many sync wait commands"), but Tile's
# scheduler emits multi-wait instructions.  Split them in the serialized BIR
# by inserting wait-only EventSemaphore carriers immediately before, on the
# same engine queue (semantically identical: same queue position, waits
# simply execute as separate instructions).
_WSPLIT_UID = [0]


def _split_bir_waits(bir_json: bytes, maxw: int = 1) -> bytes:
    m = json.loads(bir_json)
    tmpl = None
    for fn in m["functions"]:
        for bb in fn["blocks"]:
            for ins in bb["instructions"]:
                if ins.get("opcode") == "EventSemaphore":
                    tmpl = json.loads(json.dumps(ins))
                    break
            if tmpl:
                break
    if tmpl is None:
        return bir_json
    for fn in m["functions"]:
        for bb in fn["blocks"]:
            out = []
            for ins in bb["instructions"]:
                si = ins.get("sync_info")
                waits = (si or {}).get("on_wait") or []
                if len(waits) > maxw:
                    keep = waits[-maxw:]
                    extra = waits[:-maxw]
                    for i in range(0, len(extra), maxw):
                        _WSPLIT_UID[0] += 1
                        d = json.loads(json.dumps(tmpl))
                        d["name"] = f"WSPLIT-{_WSPLIT_UID[0]}"
                        d["engine"] = ins["engine"]
                        d["ins"] = []
                        d["outs"] = []
                        d["sync_info"] = {
                            "on_wait": extra[i : i + maxw],
                            "on_update": [],
                        }
                        d.pop("debug", None)
                        d.pop("bass_addl_debug", None)
                        out.append(d)
                    si["on_wait"] = keep
                out.append(ins)
            bb["instructions"] = out
    return json.dumps(m).encode()


def _patched_compile_bir_kernel(bir_json, tmpdir, neff_name="file.neff"):
    return _orig_compile_bir_kernel(
        _split_bir_waits(bir_json), tmpdir, neff_name=neff_name
    )


_bass2jax.compile_bir_kernel = _patched_compile_bir_kernel

# dims
B, D, A = 256, 512, 1024
NO, NR, NI, E, H = 65, 17, 2, 64, 128
NCORES = 8
BL = B // NCORES  # 32 batch rows per core

F32 = mybir.dt.float32
BF16 = mybir.dt.bfloat16
AF = mybir.ActivationFunctionType
ALU = mybir.AluOpType

NOP = 68  # rs head padded to a multiple of 4 opcodes
NGRP = NOP // 4  # 17 groups of 4 opcodes (rs head)


def _bf(x):
    import ml_dtypes

    return np.asarray(x, dtype=ml_dtypes.bfloat16)


def _f32(x):
    return np.ascontiguousarray(np.asarray(x, dtype=np.float32))


# ---------------------------------------------------------------------------
# packed-constant layout: chunk -> (partitions, dtype, [(name, rows, cols)])
# one DRAM param + one SBUF tile + one DMA per chunk; DMAs are issued in
# this order (first-consumer order)
_PACKS = [
    ("c_a2", 128, BF16, [("embrd", 2 * E, A), ("wrde", 2 * E, H),
                         ("wime1", 2 * E, H), ("wd32", H, 32)]),
    ("c_c64", 64, BF16, [("wrse", E, H), ("embrs", E, NOP), ("wime2", E, H),
                         ("embim2", E, A)]),
    ("c_feat", 128, BF16, [("featT", 128, 128)]),
    ("c_w1im", 128, BF16, [("w1im", 128, 512)]),
    ("c_f32", 128, F32, [("b1s", H, 4), ("b2op", NO, 1), ("b2rs", H, 1),
                         ("b2rd", H, 1), ("db2im", BL, 1)]),
    ("c_a2r", 128, BF16, [("w2opT", H, NO), ("w2rsT", H, 32),
                          ("w2rdT", H, 32), ("suind_rs", H, 4),
                          ("embrd_u", 2 * E, 704), ("onescol", 128, 1),
                          ("onesrow", 1, 32)]),
    ("c_w1rs", 128, BF16, [("w1rs", 128, 512)]),
    ("c_w1rd", 128, BF16, [("w1rd", 128, 512)]),
    ("c_w1op", 128, BF16, [("w1op", 128, 512)]),
    ("c_b1", 128, BF16, [("gop", NO, A), ("grs", H, A)]),
    ("c_low32", BL, BF16, [("m32", BL, A), ("id32", BL, 32)]),
    ("c_low4", 4, BF16, [("negones", 1, A), ("mdb2", 1, A), ("g2rs", 4, A)]),
    ("c_b2", 128, BF16, [("grd", H, A), ("gpair", H, A)]),
]
_PACK_ENG = {n: (i % 2) for i, (n, _, _, _) in enumerate(_PACKS)}


def _host_prep(inputs):
    """Build all per-core / shared device constants on host (index ops only
    plus dtype packing; all real FLOPs happen on device)."""
    feats = _f32(inputs["features"])
    o = np.clip(inputs["act_o"].astype(np.int64), 0, NO - 1)
    rs = np.clip(inputs["act_rs"].astype(np.int64), 0, NR - 1)
    rd = np.clip(inputs["act_rd"].astype(np.int64), 0, NR - 1)
    im = np.clip(inputs["act_imm"].astype(np.int64), 0, NI - 1)

    perm = np.lexsort((rs, o))  # sort by (opcode, reg_src)
    os_, rss, rds, ims = o[perm], rs[perm], rd[perm], im[perm]

    opcode_embed = _f32(inputs["opcode_embed"])  # [65, 64]
    reg_embed = _f32(inputs["reg_embed"])  # [17, 64]
    op_e = opcode_embed[os_]  # [A, 64] sorted
    rs_e = reg_embed[rss]
    rd_e = reg_embed[rds]

    W = {k: _f32(inputs[k]) for k in inputs if k.endswith(("W1", "W2", "b1", "b2"))}

    c = {}
    # feature-path weights per head: [128, 512] with K-chunk k at cols
    # [128k, 128k+128)
    for nm, wk in (("im", "imm_W1"), ("rs", "rs_W1"), ("rd", "rd_W1"),
                   ("op", "op_W1")):
        wT = W[wk][:, :D].T  # [D, H]
        c["w1" + nm] = np.concatenate(
            [wT[128 * k : 128 * (k + 1), :] for k in range(4)], axis=1
        )  # [128, 512]
    c["b1s"] = _f32(
        np.stack([W["imm_b1"], W["rs_b1"], W["rd_b1"], W["op_b1"]], axis=1)
    )  # [128, 4]

    # embedding-path weights + gathered embeddings (stacked on K)
    c["wrse"] = W["rs_W1"][:, D : D + E].T  # [64, 128]
    embrs = np.zeros((E, NOP), np.float32)
    embrs[:, :NO] = opcode_embed.T
    c["embrs"] = embrs  # [64, 68] all opcodes (padded)
    c["wrde"] = np.concatenate(
        [W["rd_W1"][:, D : D + E].T, W["rd_W1"][:, D + E : D + 2 * E].T], axis=0
    )  # [128, 128]
    c["embrd"] = np.concatenate([op_e.T, rs_e.T], axis=0)  # [128, A]
    c["wime1"] = np.concatenate(
        [W["imm_W1"][:, D : D + E].T, W["imm_W1"][:, D + E : D + 2 * E].T], axis=0
    )  # [128, 128]
    c["wime2"] = W["imm_W1"][:, D + 2 * E :].T  # [64, 128]
    c["embim2"] = rd_e.T  # [64, A]

    # head-2 weights (V padded to 32 with zeros so PSUM pad rows are written)
    c["w2opT"] = W["op_W2"].T  # [128, 65]
    w2rs = np.zeros((H, 32), np.float32)
    w2rs[:, :NR] = W["rs_W2"].T
    c["w2rsT"] = w2rs
    w2rd = np.zeros((H, 32), np.float32)
    w2rd[:, :NR] = W["rd_W2"].T
    c["w2rdT"] = w2rd
    # imm head: difference vector w_d = W2[0] - W2[1], replicated to 32
    # PE columns so every psum row of a strip holds d (no garbage rows)
    c["wd32"] = np.tile((W["imm_W2"][0] - W["imm_W2"][1]).reshape(H, 1), (1, 32))
    db2 = float(W["imm_b2"][0] - W["imm_b2"][1])
    c["db2im"] = np.full((BL, 1), db2, np.float32)

    # biases b2 as per-partition columns
    c["b2op"] = _f32(W["op_b2"][:, None])  # [65, 1]
    for nm, b2 in (("b2rs", W["rs_b2"]), ("b2rd", W["rd_b2"])):
        t = np.zeros((H, 1), np.float32)
        for s in range(4):
            t[32 * s : 32 * s + NR, 0] = b2
        c[nm] = t

    # op-head gather one-hot + misc rows
    m = (ims == 0).astype(np.float32)  # [A] imm-head class-0 selector
    gop = np.zeros((NO, A), np.float32)
    gop[os_, np.arange(A)] = 1.0
    c["gop"] = gop
    c["onescol"] = np.ones((128, 1), np.float32)
    c["onesrow"] = np.ones((1, 32), np.float32)
    c["negones"] = -np.ones((1, A), np.float32)
    c["mdb2"] = (db2 * m)[None, :]  # [1, A]
    c["m32"] = np.broadcast_to(m, (BL, A)).copy()

    # rs-head gather tables (block one-hot; actions sorted by opcode)
    grs = np.zeros((H, A), np.float32)
    grs[(os_ % 4) * 32 + rss, np.arange(A)] = 1.0
    c["grs"] = grs
    g2rs = np.zeros((4, A), np.float32)
    g2rs[os_ % 4, np.arange(A)] = -1.0
    c["g2rs"] = g2rs
    suind_rs = np.zeros((H, 4), np.float32)
    for s in range(4):
        suind_rs[32 * s : 32 * s + NR, s] = 1.0
    c["suind_rs"] = suind_rs

    # rd head deduplicated over distinct (opcode, reg_src) pairs
    pairs_all = os_ * NR + rss                      # non-decreasing (sorted)
    u_pairs, pid = np.unique(pairs_all, return_inverse=True)
    U = len(u_pairs)
    U4 = 704  # fixed pad (U ~ 660-680 for random actions; assert below)
    assert U <= U4, f"U={U} exceeds pad {U4}"
    NPG = U4 // 4  # 176 pair-groups of 4
    o_u = u_pairs // NR
    rs_u = u_pairs % NR
    embrd_u = np.zeros((2 * E, U4), np.float32)
    embrd_u[:E, :U] = opcode_embed[o_u].T
    embrd_u[E:, :U] = reg_embed[rs_u].T
    c["embrd_u"] = embrd_u
    # grd: one-hot [(strip, v), a] selecting (pid(a)%4, rd(a))
    grd = np.zeros((H, A), np.float32)
    grd[32 * (pid % 4) + rds, np.arange(A)] = 1.0
    c["grd"] = grd
    # gpair: -1 one-hot [pid%128, a] for the lnsu gather (per 128-pair chunk)
    gpair = np.zeros((H, A), np.float32)
    gpair[pid % H, np.arange(A)] = -1.0
    c["gpair"] = gpair
    c["id32"] = np.eye(BL, 32, dtype=np.float32)

    # rs gather chunk column ranges (static, baked into program; identical
    # on every core since actions are replicated)
    bounds = np.searchsorted(os_, np.arange(0, NO + 4, 4)[: NGRP + 1])
    chunks = []
    for g in range(NGRP):
        lo, hi = int(bounds[g]), int(bounds[g + 1])
        while lo < hi:
            nxt = min(hi, ((lo // 512) + 1) * 512, lo + 512)
            chunks.append((g, lo, nxt))
            lo = nxt
    # rd-head gather chunk ranges: per pair-group (grd/q gather) and per
    # 128-pair chunk (lnsu gather), split at psum bank boundaries
    pgrp_a = pid // 4
    qchunks = []
    for pg in range(int(pgrp_a.max()) + 1):
        lo = int(np.searchsorted(pgrp_a, pg))
        hi = int(np.searchsorted(pgrp_a, pg + 1))
        while lo < hi:
            nxt = min(hi, ((lo // 512) + 1) * 512)
            qchunks.append((pg, lo, nxt))
            lo = nxt
    lchunks = []
    for ch in range((U + H - 1) // H):
        lo = int(np.searchsorted(pid, H * ch))
        hi = int(np.searchsorted(pid, H * (ch + 1)))
        while lo < hi:
            nxt = min(hi, ((lo // 512) + 1) * 512)
            lchunks.append((ch, lo, nxt))
            lo = nxt
    feat_T = feats.T  # [D, B]
    per_core = []
    for cid in range(NCORES):
        ft = feat_T[:, cid * BL : (cid + 1) * BL]  # [512, 32]
        ftp = np.concatenate(
            [ft[128 * k : 128 * (k + 1), :] for k in range(4)], axis=1
        )  # [128, 128]
        per_core.append({"featT": ftp})

    # assemble packed chunk arrays
    packed = {}
    for chunk, parts, dt, entries in _PACKS:
        ncols = sum(e[2] for e in entries)
        arr = np.zeros((parts, ncols), np.float32)
        off = 0
        for name, rows, cols in entries:
            if name != "featT":
                arr[:rows, off : off + cols] = c[name]
            off += cols
        packed[chunk] = arr if dt == F32 else _bf(arr)
    return packed, per_core, (tuple(chunks), tuple(qchunks), tuple(lchunks)), perm


def build_program(allchunks):
    chunks, qchunks, lchunks = allchunks
    nc = bass.Bass()
    dr = {}
    for chunk, parts, dt, entries in _PACKS:
        ncols = sum(e[2] for e in entries)
        dr[chunk] = nc.declare_dram_parameter(chunk, [parts, ncols], dt,
                                              isOutput=False)
    out_d = nc.declare_dram_parameter("out", [BL, A], F32, isOutput=True)

    def MM(*a, **k):
        k.setdefault("skip_group_check", True)
        return nc.tensor.matmul(*a, **k)

    with ExitStack() as ctx:
        tc = ctx.enter_context(tile.TileContext(nc))
        cp = ctx.enter_context(tc.tile_pool(name="consts", bufs=1))
        sb = ctx.enter_context(tc.tile_pool(name="sbuf", bufs=2))
        hb = ctx.enter_context(tc.tile_pool(name="hbuf", bufs=8))
        eb = ctx.enter_context(tc.tile_pool(name="ebuf", bufs=3))
        pA = ctx.enter_context(tc.tile_pool(name="pA", bufs=1, space="PSUM"))
        pB = ctx.enter_context(tc.tile_pool(name="pB", bufs=1, space="PSUM"))
        pG = ctx.enter_context(tc.tile_pool(name="pG", bufs=4, space="PSUM"))

        # ---- load packed constants: one tile + one DMA per chunk, in
        # first-consumer order; ct[] = slices into the chunk tiles
        dma_engs = [nc.sync, nc.gpsimd]
        ct = {}
        for chunk, parts, dt, entries in _PACKS:
            ncols = sum(e[2] for e in entries)
            t = cp.tile([parts, ncols], dt, tag=chunk, name=chunk)
            dma_engs[_PACK_ENG[chunk]].dma_start(t[:, :], dr[chunk][:, :])
            off = 0
            for name, rows, cols in entries:
                ct[name] = t[0:rows, off : off + cols]
                off += cols

        # ---- ep tables (embedding partials) on PE; psum->sbuf copies on ACT
        ep_im = eb.tile([H, A], BF16, tag="ep_im")
        for j in range(2):
            pe_h = pG.tile([H, 512], F32, tag="lgh", name=f"pei{j}")
            MM(pe_h[:, :], ct["wime1"][:, :], ct["embrd"][:, 512 * j : 512 * (j + 1)],
               start=True, stop=False)
            MM(pe_h[:, :], ct["wime2"][:, :], ct["embim2"][:, 512 * j : 512 * (j + 1)],
               start=False, stop=True)
            nc.scalar.activation(ep_im[:, 512 * j : 512 * (j + 1)], pe_h[:, :],
                                 AF.Identity)
        U4, NPG = 704, 176
        ep_rd = eb.tile([H, U4], BF16, tag="ep_rd")
        for j, (c0, c1) in enumerate(((0, 512), (512, U4))):
            pe_h = pG.tile([H, c1 - c0], F32, tag="lgh", name=f"pep{j}",
                           padded_shape=[H, 512])
            MM(pe_h[:, :], ct["wrde"][:, :], ct["embrd_u"][:, c0:c1])
            nc.scalar.activation(ep_rd[:, c0:c1], pe_h[:, :], AF.Identity)
        psum_ep3 = pG.tile([H, NOP], F32, tag="lgh", padded_shape=[H, 512])
        MM(psum_ep3[:, :], ct["wrse"][:, :], ct["embrs"][:, :])
        ep_rs = eb.tile([H, NOP], BF16, tag="ep_rs")
        nc.scalar.activation(ep_rs[:, :], psum_ep3[:, :], AF.Identity)

        # ---- fp (feature partials): one [H, 4*BL] psum tile, one
        # accumulation group per head, emitted head-by-head as w1 DMAs land
        psum_fp = pB.tile([H, 4 * BL], F32, tag="seqB", padded_shape=[H, 512])
        fp = {}

        def emit_fp(hd, nm):
            for k in range(4):
                MM(
                    psum_fp[:, 32 * hd : 32 * hd + BL],
                    ct["w1" + nm][:, 128 * k : 128 * (k + 1)],
                    ct["featT"][:, BL * k : BL * (k + 1)],
                    start=(k == 0),
                    stop=(k == 3),
                )
            if nm == "op":
                t = sb.tile([H, BL], BF16, tag="op_h", name="op_h")
                nc.scalar.activation(t[:, :], psum_fp[:, 96:128], AF.Relu,
                                     bias=ct["b1s"][:, 3:4])
            else:
                t = sb.tile([H, BL], F32, tag=f"fp_{nm}", name=f"fp_{nm}")
                nc.scalar.activation(t[:, :], psum_fp[:, 32 * hd : 32 * hd + BL],
                                     AF.Identity, bias=ct["b1s"][:, hd : hd + 1])
            fp[nm] = t

        emit_fp(0, "im")

        # ---- deferred op/rs head pieces (emitted inside the imm loop)
        st = {}

        def emit_h_rs(b0, b1):
            if "h_rs" not in st:
                st["h_rs"] = cp.tile([H, NOP * BL], BF16, tag="h_rs", name="h_rs")
            for b in range(b0, b1):
                nc.gpsimd.tensor_scalar(
                    st["h_rs"][:, NOP * b : NOP * (b + 1)],
                    ep_rs[:, :], fp["rs"][:, b : b + 1], 0.0,
                    op0=ALU.add, op1=ALU.max,
                )

        def emit_op_logits():
            psum_opl = pG.tile([NO, BL], F32, tag="lgh", padded_shape=[NO, 512],
                               name="psum_opl")
            MM(psum_opl[:, :], ct["w2opT"][:, :], fp["op"][:, :])
            st["exp_op"] = sb.tile([NO, BL], BF16, tag="exp_op", name="exp_op")
            nc.scalar.activation(st["exp_op"][:, :], psum_opl[:, :], AF.Exp,
                                 bias=ct["b2op"][:, :])
            st["lb2_op"] = sb.tile([NO, BL], BF16, tag="lb2_op", name="lb2_op")
            nc.scalar.activation(st["lb2_op"][:, :], psum_opl[:, :], AF.Identity,
                                 bias=ct["b2op"][:, :])

        def emit_rs_logits():
            h_rs_v = st["h_rs"][:, :].rearrange("p (b c) -> p c b", c=NOP)
            psum_rsl = pB.tile([H, 32 * NGRP], F32, tag="seqB", name="psum_rsl")
            for c_ in range(NOP):
                g, s = c_ // 4, c_ % 4
                MM(
                    psum_rsl[32 * s : 32 * s + 32, 32 * g : 32 * g + 32],
                    ct["w2rsT"][:, :],
                    h_rs_v[:, c_, :],
                    tile_position=(0, 32 * s),
                )
            st["exp_rs"] = sb.tile([H, 32 * NGRP], BF16, tag="exp_rs", name="exp_rs")
            nc.scalar.activation(st["exp_rs"][:, :], psum_rsl[:, :], AF.Exp,
                                 bias=ct["b2rs"][:, :])
            st["lb2_rs"] = sb.tile([H, 32 * NGRP], BF16, tag="lb2_rs", name="lb2_rs")
            nc.scalar.activation(st["lb2_rs"][:, :], psum_rsl[:, :], AF.Identity,
                                 bias=ct["b2rs"][:, :])

        def emit_su_sections():
            psum_osu = pG.tile([1, BL], F32, tag="lgh", padded_shape=[1, 512],
                               name="psum_osu")
            MM(psum_osu[:, :], ct["onescol"][0:NO, :], st["exp_op"][:, :])
            st["lnsu_op"] = sb.tile([1, BL], BF16, tag="lnsu_op", name="lnsu_op")
            nc.scalar.activation(st["lnsu_op"][:, :], psum_osu[:, :], AF.Ln)
            psum_rsu = pG.tile([4, 512], F32, tag="lgh", name="psum_rsu")
            MM(psum_rsu[:, :], ct["suind_rs"][:, :], st["exp_rs"][:, 0:512])
            psum_rsu2 = pG.tile([4, 32 * NGRP - 512], F32, tag="lgh",
                                padded_shape=[4, 512], name="psum_rsu2")
            MM(psum_rsu2[:, :], ct["suind_rs"][:, :], st["exp_rs"][:, 512 : 32 * NGRP])
            st["lnsu_rs"] = sb.tile([4, 32 * NGRP], BF16, tag="lnsu_rs",
                                    name="lnsu_rs")
            nc.scalar.activation(st["lnsu_rs"][:, 0:512], psum_rsu[:, :], AF.Ln)
            nc.scalar.activation(st["lnsu_rs"][:, 512 : 32 * NGRP],
                                 psum_rsu2[:, :], AF.Ln)

        def emit_acc():
            psum_acc = pA.tile([BL, A], F32, tag="seqA", name="psum_acc")
            for j in range(2):
                MM(psum_acc[:, 512 * j : 512 * (j + 1)], st["lb2_op"][:, :],
                   ct["gop"][:, 512 * j : 512 * (j + 1)], start=True, stop=False)
                MM(psum_acc[:, 512 * j : 512 * (j + 1)], st["lnsu_op"][:, :],
                   ct["negones"][:, 512 * j : 512 * (j + 1)], start=False,
                   stop=False)
                MM(psum_acc[:, 512 * j : 512 * (j + 1)], ct["onesrow"][:, :],
                   ct["mdb2"][:, 512 * j : 512 * (j + 1)], start=False, stop=False)
            last_for_bank = {}
            for i, (g, lo, hi) in enumerate(chunks):
                last_for_bank[lo // 512] = i
            for i, (g, lo, hi) in enumerate(chunks):
                MM(psum_acc[:, lo:hi], st["lb2_rs"][:, 32 * g : 32 * g + 32],
                   ct["grs"][:, lo:hi], start=False, stop=False)
                MM(psum_acc[:, lo:hi], st["lnsu_rs"][:, 32 * g : 32 * g + 32],
                   ct["g2rs"][:, lo:hi], start=False, stop=False)
            st["psum_acc"] = psum_acc

        # ---- rd-head h table (deduplicated over (o, rs) pairs), b-major
        h_rd_all = cp.tile([H, U4 * BL], BF16, tag="h_rd_all", name="h_rd_all")

        def emit_h_rd(b0, b1, eng):
            for b in range(b0, b1):
                eng.tensor_scalar(
                    h_rd_all[:, U4 * b : U4 * (b + 1)],
                    ep_rd[:, :], fp["rd"][:, b : b + 1], 0.0,
                    op0=ALU.add, op1=ALU.max,
                )

        # ---- imm head phase: h_im tiles + strip matmuls (every psum row of
        # strip s holds d for b=4g+s) -> bf16 copies into d_all -> one
        # SBUF->SBUF gather DMA per strip into compact X32[32, A].
        # op/rs/acc sections are emitted between waves so each engine's
        # in-order queue reaches them right as their inputs land.
        X32 = cp.tile([BL, A], BF16, tag="X32", name="X32")
        d_all = cp.tile([H, 16 * 512], BF16, tag="d_all", name="d_all")
        CP_ENG = ["A", "A", "A", "D"]  # psum reads: ACT/DVE only
        for g in range(8):
            hts_im = []
            for s in range(4):
                b = 4 * g + s
                h_t = hb.tile([H, A], BF16, tag="h", name=f"him{b}")
                nc.vector.tensor_scalar(
                    h_t[:, :], ep_im[:, :], fp["im"][:, b : b + 1], 0.0,
                    op0=ALU.add, op1=ALU.max,
                )
                hts_im.append(h_t)
            for j in range(2):
                psum_d = pG.tile([H, 512], F32, tag="lgh", name=f"d{g}{j}")
                for s in range(4):
                    MM(
                        psum_d[32 * s : 32 * s + 32, :],
                        ct["wd32"][:, :],
                        hts_im[s][:, 512 * j : 512 * (j + 1)],
                        tile_position=(0, 32 * s),
                    )
                dsl = d_all[:, 512 * (2 * g + j) : 512 * (2 * g + j + 1)]
                lane = CP_ENG[(2 * g + j) % len(CP_ENG)]
                if lane == "A":
                    nc.scalar.activation(dsl, psum_d[:, :], AF.Identity)
                else:
                    nc.vector.tensor_copy(dsl, psum_d[:, :])
            if g == 0:
                emit_fp(1, "rs")
            elif g == 1:
                emit_fp(2, "rd")
                emit_h_rs(0, 16)
            elif g == 2:
                emit_fp(3, "op")
                emit_h_rs(16, BL)
                emit_h_rd(0, 2, nc.vector)
            elif g == 3:
                emit_op_logits()
                emit_h_rd(2, 4, nc.vector)
            elif g == 4:
                emit_rs_logits()
                emit_h_rd(4, 6, nc.vector)
            elif g == 5:
                emit_su_sections()
                emit_h_rd(6, 8, nc.vector)
                emit_h_rd(24, 27, nc.gpsimd)
            elif g == 6:
                emit_h_rd(8, 11, nc.vector)
                emit_h_rd(27, 30, nc.gpsimd)
            elif g == 7:
                emit_h_rd(11, 14, nc.vector)
                emit_h_rd(30, BL, nc.gpsimd)
        # gather: X32[4g+s, 512j+c] = d_all[32s, (2g+j)*512 + c]
        for s in range(4):
            dma_engs[s % 2].dma_start(
                X32[s : BL : 4, :].rearrange("g (j c) -> g j c", j=2),
                d_all[32 * s : 32 * s + 1, :].rearrange(
                    "p (g j c) -> p g j c", g=8, j=2
                ),
            )

        # finish h_rd, then acc gathers overlap the DVE tail
        emit_h_rd(14, 22, nc.vector)
        for b in (22, 23):
            nc.scalar.activation(
                h_rd_all[:, U4 * b : U4 * (b + 1)], ep_rd[:, :], AF.Relu,
                bias=fp["rd"][:, b : b + 1],
            )
        emit_acc()

        # ---- rd head: deduplicated logits table [(strip, v), (pgrp, b)]
        # in 11 psum waves; exp on ACT; su-reduce (4 cols per pair-group)
        sp32 = cp.tile([BL, A], BF16, tag="sp32")
        u32 = cp.tile([BL, A], BF16, tag="u32")
        ctr_im = cp.tile([BL, A], BF16, tag="ctr_im")
        h_rd_v = h_rd_all[:, :].rearrange("p (b c) -> p c b", c=U4)
        exp_tbl = cp.tile([H, 32 * NPG], BF16, tag="exp_tbl", name="exp_tbl")
        psum_sutbl = pB.tile([BL, U4], F32, tag="seqB", name="psum_sutbl")

        # imm-head tail: sp = softplus(X + db2) = ln(1 + e^(X+db2)),
        # u = X*m, ctr_im = u - sp (db2*m is already in the acc); emitted
        # here so ACT does it in the imm->rd transition, not mid table-loop
        e32 = cp.tile([BL, A], BF16, tag="e32", name="e32")
        nc.scalar.activation(e32[:, :], X32[:, :], AF.Exp,
                             bias=ct["db2im"][:, :])
        e1 = cp.tile([BL, A], BF16, tag="e1", name="e1")
        nc.vector.tensor_scalar_add(e1[:, :], e32[:, :], 1.0)
        nc.scalar.activation(sp32[:, :], e1[:, :], AF.Ln)
        nc.vector.tensor_mul(u32[:, :], X32[:, :], ct["m32"][:, :])
        nc.vector.tensor_sub(ctr_im[:, :], u32[:, :], sp32[:, :])

        def emit_su(t):
            # su-reduce for tile t, emitted one tile late so exp(t) is done
            # and the PE wait-queue never blocks mid-loop
            for pgl in range(16):
                pg = 16 * t + pgl
                MM(
                    psum_sutbl[:, 4 * pg : 4 * pg + 4],
                    exp_tbl[:, 512 * t + 32 * pgl : 512 * t + 32 * pgl + 32],
                    ct["suind_rs"][:, :],
                )

        for t in range(NPG // 16):
            ptbl = pG.tile([H, 512], F32, tag="lgh", name=f"tbl{t}")
            for pl in range(64):
                p_ = 64 * t + pl
                s = p_ % 4
                pgl = (p_ // 4) % 16
                MM(
                    ptbl[32 * s : 32 * s + 32, 32 * pgl : 32 * pgl + 32],
                    ct["w2rdT"][:, :],
                    h_rd_v[:, p_, :],
                    tile_position=(0, 32 * s),
                )
            nc.scalar.activation(exp_tbl[:, 512 * t : 512 * (t + 1)], ptbl[:, :],
                                 AF.Exp, bias=ct["b2rd"][:, :])
            if t >= 1:
                emit_su(t - 1)

        emit_su(NPG // 16 - 1)

        # ---- lnsu table [32, U4] -> transpose to [pair, b] chunks
        lnsu_tbl = cp.tile([BL, 768], BF16, tag="lnsu_tbl", name="lnsu_tbl")
        nc.scalar.activation(lnsu_tbl[:, 0:U4], psum_sutbl[:, :], AF.Ln)
        nc.vector.memset(lnsu_tbl[:, U4:768], 0.0)
        lnsuT = cp.tile([H, 6 * 32], BF16, tag="lnsuT", name="lnsuT")
        for ch in range(6):
            pt = pG.tile([H, 32], BF16, tag="lgh", name=f"ptr{ch}",
                         padded_shape=[H, 512])
            nc.tensor.transpose(pt[:, :], lnsu_tbl[:, 128 * ch : 128 * (ch + 1)],
                                ct["id32"][:, :])
            nc.vector.tensor_copy(lnsuT[:, 32 * ch : 32 * (ch + 1)], pt[:, :])

        # ---- gathers: exp_sel into psum_q (by pair-group), -lnsu into the
        # open accumulator (by 128-pair chunk); bank-wise start/stop
        psum_q = pB.tile([BL, A], F32, tag="seqB", name="psum_q")
        qfirst, qlast = {}, {}
        for i, (pg, lo, hi) in enumerate(qchunks):
            b = lo // 512
            qfirst.setdefault(b, i)
            qlast[b] = i
        for i, (pg, lo, hi) in enumerate(qchunks):
            b = lo // 512
            MM(
                psum_q[:, lo:hi],
                exp_tbl[:, 32 * pg : 32 * pg + 32],
                ct["grd"][:, lo:hi],
                start=(qfirst[b] == i),
                stop=(qlast[b] == i),
            )
        lnsel = cp.tile([BL, A], BF16, tag="lnsel")

        llast = {}
        for i, (ch, lo, hi) in enumerate(lchunks):
            llast[lo // 512] = i
        psum_acc = st["psum_acc"]
        for i, (ch, lo, hi) in enumerate(lchunks):
            MM(
                psum_acc[:, lo:hi],
                lnsuT[:, 32 * ch : 32 * ch + 32],
                ct["gpair"][:, lo:hi],
                start=False,
                stop=(llast[lo // 512] == i),
            )

        # ---- final combine + store, per psum bank
        for j in range(2):
            sl = slice(512 * j, 512 * (j + 1))
            nc.scalar.activation(lnsel[:, sl], psum_q[:, sl], AF.Ln)
            w32 = sb.tile([BL, 512], BF16, tag="w32", name=f"w32{j}")
            nc.vector.tensor_add(w32[:, :], ctr_im[:, sl], lnsel[:, sl])
            out_sb = sb.tile([BL, 512], F32, tag="out_sb", name=f"out{j}")
            nc.vector.tensor_add(out_sb[:, :], w32[:, :], psum_acc[:, sl])
            dma_engs[j].dma_start(out_d[:, sl], out_sb[:, :])

    return nc.tensor.matmul(*a, **k)

    with ExitStack() as ctx:
        tc = ctx.enter_context(tile.TileContext(nc))
        cp = ctx.enter_context(tc.tile_pool(name="consts", bufs=1))
        sb = ctx.enter_context(tc.tile_pool(name="sbuf", bufs=2))
        hb = ctx.enter_context(tc.tile_pool(name="hbuf", bufs=8))
        eb = ctx.enter_context(tc.tile_pool(name="ebuf", bufs=3))
        pA = ctx.enter_context(tc.tile_pool(name="pA", bufs=1, space="PSUM"))
        pB = ctx.enter_context(tc.tile_pool(name="pB", bufs=1, space="PSUM"))
        pG = ctx.enter_context(tc.tile_pool(name="pG", bufs=4, space="PSUM"))

        # ---- load packed constants: one tile + one DMA per chunk, in
        # first-consumer order; ct[] = slices into the chunk tiles
        dma_engs = [nc.sync, nc.gpsimd]
        ct = {}
        for chunk, parts, dt, entries in _PACKS:
            ncols = sum(e[2] for e in entries)
            t = cp.tile([parts, ncols], dt, tag=chunk, name=chunk)
            dma_engs[_PACK_ENG[chunk]].dma_start(t[:, :], dr[chunk][:, :])
            off = 0
            for name, rows, cols in entries:
                ct[name] = t[0:rows, off : off + cols]
                off += cols

        # ---- ep tables (embedding partials) on PE; psum->sbuf copies on ACT
        ep_im = eb.tile([H, A], BF16, tag="ep_im")
        for j in range(2):
            pe_h = pG.tile([H, 512], F32, tag="lgh", name=f"pei{j}")
            MM(pe_h[:, :], ct["wime1"][:, :], ct["embrd"][:, 512 * j : 512 * (j + 1)],
               start=True, stop=False)
            MM(pe_h[:, :], ct["wime2"][:, :], ct["embim2"][:, 512 * j : 512 * (j + 1)],
               start=False, stop=True)
            nc.scalar.activation(ep_im[:, 512 * j : 512 * (j + 1)], pe_h[:, :],
                                 AF.Identity)
        U4, NPG = 704, 176
        ep_rd = eb.tile([H, U4], BF16, tag="ep_rd")
        for j, (c0, c1) in enumerate(((0, 512), (512, U4))):
            pe_h = pG.tile([H, c1 - c0], F32, tag="lgh", name=f"pep{j}",
                           padded_shape=[H, 512])
            MM(pe_h[:, :], ct["wrde"][:, :], ct["embrd_u"][:, c0:c1])
            nc.scalar.activation(ep_rd[:, c0:c1], pe_h[:, :], AF.Identity)
        psum_ep3 = pG.tile([H, NOP], F32, tag="lgh", padded_shape=[H, 512])
        MM(psum_ep3[:, :], ct["wrse"][:, :], ct["embrs"][:, :])
        ep_rs = eb.tile([H, NOP], BF16, tag="ep_rs")
        nc.scalar.activation(ep_rs[:, :], psum_ep3[:, :], AF.Identity)

        # ---- fp (feature partials): one [H, 4*BL] psum tile, one
        # accumulation group per head, emitted head-by-head as w1 DMAs land
        psum_fp = pB.tile([H, 4 * BL], F32, tag="seqB", padded_shape=[H, 512])
        fp = {}

        def emit_fp(hd, nm):
            for k in range(4):
                MM(
                    psum_fp[:, 32 * hd : 32 * hd + BL],
                    ct["w1" + nm][:, 128 * k : 128 * (k + 1)],
                    ct["featT"][:, BL * k : BL * (k + 1)],
                    start=(k == 0),
                    stop=(k == 3),
                )
            if nm == "op":
                t = sb.tile([H, BL], BF16, tag="op_h", name="op_h")
                nc.scalar.activation(t[:, :], psum_fp[:, 96:128], AF.Relu,
                                     bias=ct["b1s"][:, 3:4])
            else:
                t = sb.tile([H, BL], F32, tag=f"fp_{nm}", name=f"fp_{nm}")
                nc.scalar.activation(t[:, :], psum_fp[:, 32 * hd : 32 * hd + BL],
                                     AF.Identity, bias=ct["b1s"][:, hd : hd + 1])
            fp[nm] = t

        emit_fp(0, "im")

        # ---- deferred op/rs head pieces (emitted inside the imm loop)
        st = {}

        def emit_h_rs(b0, b1):
            if "h_rs" not in st:
                st["h_rs"] = cp.tile([H, NOP * BL], BF16, tag="h_rs", name="h_rs")
            for b in range(b0, b1):
                nc.gpsimd.tensor_scalar(
                    st["h_rs"][:, NOP * b : NOP * (b + 1)],
                    ep_rs[:, :], fp["rs"][:, b : b + 1], 0.0,
                    op0=ALU.add, op1=ALU.max,
                )

        def emit_op_logits():
            psum_opl = pG.tile([NO, BL], F32, tag="lgh", padded_shape=[NO, 512],
                               name="psum_opl")
            MM(psum_opl[:, :], ct["w2opT"][:, :], fp["op"][:, :])
            st["exp_op"] = sb.tile([NO, BL], BF16, tag="exp_op", name="exp_op")
            nc.scalar.activation(st["exp_op"][:, :], psum_opl[:, :], AF.Exp,
                                 bias=ct["b2op"][:, :])
            st["lb2_op"] = sb.tile([NO, BL], BF16, tag="lb2_op", name="lb2_op")
            nc.scalar.activation(st["lb2_op"][:, :], psum_opl[:, :], AF.Identity,
                                 bias=ct["b2op"][:, :])

        def emit_rs_logits():
            h_rs_v = st["h_rs"][:, :].rearrange("p (b c) -> p c b", c=NOP)
            psum_rsl = pB.tile([H, 32 * NGRP], F32, tag="seqB", name="psum_rsl")
            for c_ in range(NOP):
                g, s = c_ // 4, c_ % 4
                MM(
                    psum_rsl[32 * s : 32 * s + 32, 32 * g : 32 * g + 32],
                    ct["w2rsT"][:, :],
                    h_rs_v[:, c_, :],
                    tile_position=(0, 32 * s),
                )
            st["exp_rs"] = sb.tile([H, 32 * NGRP], BF16, tag="exp_rs", name="exp_rs")
            nc.scalar.activation(st["exp_rs"][:, :], psum_rsl[:, :], AF.Exp,
                                 bias=ct["b2rs"][:, :])
            st["lb2_rs"] = sb.tile([H, 32 * NGRP], BF16, tag="lb2_rs", name="lb2_rs")
            nc.scalar.activation(st["lb2_rs"][:, :], psum_rsl[:, :], AF.Identity,
                                 bias=ct["b2rs"][:, :])

        def emit_su_sections():
            psum_osu = pG.tile([1, BL], F32, tag="lgh", padded_shape=[1, 512],
                               name="psum_osu")
            MM(psum_osu[:, :], ct["onescol"][0:NO, :], st["exp_op"][:, :])
            st["lnsu_op"] = sb.tile([1, BL], BF16, tag="lnsu_op", name="lnsu_op")
            nc.scalar.activation(st["lnsu_op"][:, :], psum_osu[:, :], AF.Ln)
            psum_rsu = pG.tile([4, 512], F32, tag="lgh", name="psum_rsu")
            MM(psum_rsu[:, :], ct["suind_rs"][:, :], st["exp_rs"][:, 0:512])
            psum_rsu2 = pG.tile([4, 32 * NGRP - 512], F32, tag="lgh",
                                padded_shape=[4, 512], name="psum_rsu2")
            MM(psum_rsu2[:, :], ct["suind_rs"][:, :], st["exp_rs"][:, 512 : 32 * NGRP])
            st["lnsu_rs"] = sb.tile([4, 32 * NGRP], BF16, tag="lnsu_rs",
                                    name="lnsu_rs")
            nc.scalar.activation(st["lnsu_rs"][:, 0:512], psum_rsu[:, :], AF.Ln)
            nc.scalar.activation(st["lnsu_rs"][:, 512 : 32 * NGRP],
                                 psum_rsu2[:, :], AF.Ln)

        def emit_acc():
            psum_acc = pA.tile([BL, A], F32, tag="seqA", name="psum_acc")
            for j in range(2):
                MM(psum_acc[:, 512 * j : 512 * (j + 1)], st["lb2_op"][:, :],
                   ct["gop"][:, 512 * j : 512 * (j + 1)], start=True, stop=False)
                MM(psum_acc[:, 512 * j : 512 * (j + 1)], st["lnsu_op"][:, :],
                   ct["negones"][:, 512 * j : 512 * (j + 1)], start=False,
                   stop=False)
                MM(psum_acc[:, 512 * j : 512 * (j + 1)], ct["onesrow"][:, :],
                   ct["mdb2"][:, 512 * j : 512 * (j + 1)], start=False, stop=False)
            last_for_bank = {}
            for i, (g, lo, hi) in enumerate(chunks):
                last_for_bank[lo // 512] = i
            for i, (g, lo, hi) in enumerate(chunks):
                MM(psum_acc[:, lo:hi], st["lb2_rs"][:, 32 * g : 32 * g + 32],
                   ct["grs"][:, lo:hi], start=False, stop=False)
                MM(psum_acc[:, lo:hi], st["lnsu_rs"][:, 32 * g : 32 * g + 32],
                   ct["g2rs"][:, lo:hi], start=False, stop=False)
            st["psum_acc"] = psum_acc

        # ---- rd-head h table (deduplicated over (o, rs) pairs), b-major
        h_rd_all = cp.tile([H, U4 * BL], BF16, tag="h_rd_all", name="h_rd_all")

        def emit_h_rd(b0, b1, eng):
            for b in range(b0, b1):
                eng.tensor_scalar(
                    h_rd_all[:, U4 * b : U4 * (b + 1)],
                    ep_rd[:, :], fp["rd"][:, b : b + 1], 0.0,
                    op0=ALU.add, op1=ALU.max,
                )

        # ---- imm head phase: h_im tiles + strip matmuls (every psum row of
        # strip s holds d for b=4g+s) -> bf16 copies into d_all -> one
        # SBUF->SBUF gather DMA per strip into compact X32[32, A].
        # op/rs/acc sections are emitted between waves so each engine's
        # in-order queue reaches them right as their inputs land.
        X32 = cp.tile([BL, A], BF16, tag="X32", name="X32")
        d_all = cp.tile([H, 16 * 512], BF16, tag="d_all", name="d_all")
        CP_ENG = ["A", "A", "A", "D"]  # psum reads: ACT/DVE only
        for g in range(8):
            hts_im = []
            for s in range(4):
                b = 4 * g + s
                h_t = hb.tile([H, A], BF16, tag="h", name=f"him{b}")
                nc.vector.tensor_scalar(
                    h_t[:, :], ep_im[:, :], fp["im"][:, b : b + 1], 0.0,
                    op0=ALU.add, op1=ALU.max,
                )
                hts_im.append(h_t)
            for j in range(2):
                psum_d = pG.tile([H, 512], F32, tag="lgh", name=f"d{g}{j}")
                for s in range(4):
                    MM(
                        psum_d[32 * s : 32 * s + 32, :],
                        ct["wd32"][:, :],
                        hts_im[s][:, 512 * j : 512 * (j + 1)],
                        tile_position=(0, 32 * s),
                    )
                dsl = d_all[:, 512 * (2 * g + j) : 512 * (2 * g + j + 1)]
                lane = CP_ENG[(2 * g + j) % len(CP_ENG)]
                if lane == "A":
                    nc.scalar.activation(dsl, psum_d[:, :], AF.Identity)
                else:
                    nc.vector.tensor_copy(dsl, psum_d[:, :])
            if g == 0:
                emit_fp(1, "rs")
            elif g == 1:
                emit_fp(2, "rd")
                emit_h_rs(0, 16)
            elif g == 2:
                emit_fp(3, "op")
                emit_h_rs(16, BL)
                emit_h_rd(0, 2, nc.vector)
            elif g == 3:
                emit_op_logits()
                emit_h_rd(2, 4, nc.vector)
            elif g == 4:
                emit_rs_logits()
                emit_h_rd(4, 6, nc.vector)
            elif g == 5:
                emit_su_sections()
                emit_h_rd(6, 8, nc.vector)
                emit_h_rd(24, 27, nc.gpsimd)
            elif g == 6:
                emit_h_rd(8, 11, nc.vector)
                emit_h_rd(27, 30, nc.gpsimd)
            elif g == 7:
                emit_h_rd(11, 14, nc.vector)
                emit_h_rd(30, BL, nc.gpsimd)
        # gather: X32[4g+s, 512j+c] = d_all[32s, (2g+j)*512 + c]
        for s in range(4):
            dma_engs[s % 2].dma_start(
                X32[s : BL : 4, :].rearrange("g (j c) -> g j c", j=2),
                d_all[32 * s : 32 * s + 1, :].rearrange(
                    "p (g j c) -> p g j c", g=8, j=2
                ),
            )

        # ---- rd head phase (imm tail ops are emitted after wave g==3 so
        # the engines reach them right around when X32's gather DMA lands)
        sp32 = cp.tile([BL, A], BF16, tag="sp32")
        u32 = cp.tile([BL, A], BF16, tag="u32")
        ctr_im = cp.tile([BL, A], BF16, tag="ctr_im")
        psum_su = pA.tile([BL, A], F32, tag="seqA")
        psum_sel = pB.tile([BL, A], F32, tag="seqB")
        for g in range(8):
            hts_rd = []
            for s in range(4):
                b = 4 * g + s
                h_t = hb.tile([H, A], BF16, tag="h", name=f"hrd{b}")
                nc.vector.tensor_scalar(
                    h_t[:, :], ep_rd[:, :], fp["rd"][:, b : b + 1], 0.0,
                    op0=ALU.add, op1=ALU.max,
                )
                hts_rd.append(h_t)
            exp_t = sb.tile([H, A], BF16, tag="exp_t")
            mexp_t = sb.tile([H, A], BF16, tag="mexp_t")
            for j in range(2):
                psum_lg = pG.tile([H, 512], F32, tag="lgh", name=f"lg{g}{j}")
                for s in range(4):
                    MM(
                        psum_lg[32 * s : 32 * s + 32, :],
                        ct["w2rdT"][:, :],
                        hts_rd[s][:, 512 * j : 512 * (j + 1)],
                        tile_position=(0, 32 * s),
                    )
                nc.scalar.activation(
                    exp_t[:, 512 * j : 512 * (j + 1)], psum_lg[:, :], AF.Exp,
                    bias=ct["b2rd"][:, :],
                )
                nc.vector.tensor_mul(
                    mexp_t[:, 512 * j : 512 * (j + 1)],
                    exp_t[:, 512 * j : 512 * (j + 1)],
                    ct["mask_rd"][:, 512 * j : 512 * (j + 1)],
                )
            for j in range(2):
                MM(
                    psum_su[:, 512 * j : 512 * (j + 1)],
                    ct["suind_rd"][:, 32 * g : 32 * g + 32],
                    exp_t[:, 512 * j : 512 * (j + 1)],
                    start=(g == 0),
                    stop=(g == 7),
                )
                MM(
                    psum_sel[:, 512 * j : 512 * (j + 1)],
                    ct["suind_rd"][:, 32 * g : 32 * g + 32],
                    mexp_t[:, 512 * j : 512 * (j + 1)],
                    start=(g == 0),
                    stop=(g == 7),
                )
            if g == 3:
                # imm-head tail: sp = softplus(X + db2) = ln(1 + e^(X+db2))
                # (built from Exp/Ln, which are already ACT-table residents),
                # u = X*m, ctr_im = u - sp (db2*m is already in the acc)
                e32 = cp.tile([BL, A], BF16, tag="e32", name="e32")
                nc.scalar.activation(e32[:, :], X32[:, :], AF.Exp,
                                     bias=ct["db2im"][:, :])
                e1 = cp.tile([BL, A], BF16, tag="e1", name="e1")
                nc.vector.tensor_scalar_add(e1[:, :], e32[:, :], 1.0)
                nc.scalar.activation(sp32[:, :], e1[:, :], AF.Ln)
                nc.vector.tensor_mul(u32[:, :], X32[:, :], ct["m32"][:, :])
                nc.vector.tensor_sub(ctr_im[:, :], u32[:, :], sp32[:, :])
            if g == 4:
                t3 = sb.tile([BL, A], BF16, tag="t3")
                nc.vector.tensor_add(t3[:, :], ctr_im[:, :], st["acc_sb"][:, :])

        # ---- rd-head tail + final combine + store, pipelined by psum bank
        for j in range(2):
            sl = slice(512 * j, 512 * (j + 1))
            lnsu_t = sb.tile([BL, 512], BF16, tag="lnsu_rdt", name=f"lnsu{j}")
            nc.scalar.activation(lnsu_t[:, :], psum_su[:, sl], AF.Ln)
            lnsel_t = sb.tile([BL, 512], BF16, tag="lnsel_rdt", name=f"lnsel{j}")
            nc.scalar.activation(lnsel_t[:, :], psum_sel[:, sl], AF.Ln)
            ctr_rd = sb.tile([BL, 512], BF16, tag="ctr_rd", name=f"ctrrd{j}")
            nc.vector.tensor_sub(ctr_rd[:, :], lnsel_t[:, :], lnsu_t[:, :])
            out_sb = sb.tile([BL, 512], F32, tag="out_sb", name=f"out{j}")
            nc.vector.tensor_add(out_sb[:, :], ctr_rd[:, :], t3[:, sl])
            nc.sync.dma_start(out_d[:, sl], out_sb[:, :])

    return nc


_CACHE = {}


def _get_program(chunks):
    key = chunks
    if key not in _CACHE:
        _CACHE[key] = build_program(chunks)
    return _CACHE[key]


def kernel(**inputs) -> np.ndarray:
    packed, per_core, chunks, perm = _host_prep(inputs)
    nc = _get_program(chunks)
    in_maps = []
    for cid in range(NCORES):
        m = {k: np.ascontiguousarray(v) for k, v in packed.items()}
        m["c_feat"] = np.ascontiguousarray(_bf(per_core[cid]["featT"]))
        in_maps.append(m)
    res = run_bass_kernel_spmd(nc, in_maps, core_ids=list(range(NCORES)))
    out_sorted = np.concatenate(
        [res.results[cid]["out"] for cid in range(NCORES)], axis=0
    )  # [B, A] in sorted-action order
    out = np.empty_like(out_sorted)
    out[:, perm] = out_sorted
    return out.astype(np.float32)


# revision 38
# speedup vs baseline: 1.0479x; 1.0080x over previous
"""Trainium2 Bass kernel for nn_AutoregressiveInstructionHead.

Data-parallel over batch B=256 across 8 NeuronCores (32 rows each).
Head weights / embeddings / action-derived tables are replicated.

Per-core device pipeline (all heavy compute on device):
  - constants packed into a few DRAM tensors -> few big DMAs ordered by
    first consumer (HWDGE per-DMA overhead is ~625ns, so fewer transfers
    and compute starts ~3.5us in instead of ~18us)
  - fp_head = features @ W1_feat.T (+b1)  -> [H=128, B=32] via PE
  - ep tables = embeddings @ W1_emb.T     -> [H=128, A] via PE
  - imm head (NI=2): logp = m*X - softplus(X) with X = l0 - l1, so only a
    single M=1 matmul strip per (b, 512-col block) with w_d = W2[0]-W2[1];
    the strip-scattered psum rows are gathered to a compact X32[32,A] tile
    by a psum->SBUF DMA (free on the DMA engines), then one softplus + two
    DVE ops finish the head.  No exp / mask / su-sel matmuls needed.
  - op head: logits -> exp/sum/ln -> gather via one-hot matmul into PSUM acc
  - rs head: deduplicated over the 65 opcodes ([B,65,17] table), gathered
    back to the 1024 actions with block-sparse one-hot matmuls (actions are
    host-sorted by (opcode, reg_src) so each table chunk touches a
    contiguous column range; inverse permutation applied on host at the end)
  - rd head: h=relu(fp[b]+ep[a]) [128,1024] per b, logits matmul with
    col-tiling (4 b's concurrently in 32-partition strips), exp(+b2) on ACT,
    one-hot mask multiply, partition sums via indicator matmuls,
    contribution = ln(sum mask*exp) - ln(sum exp)
"""

import sys

for _p in ("/opt/trn_rl_repo",):
    if _p not in sys.path:
        sys.path.insert(0, _p)

import numpy as np
from contextlib import ExitStack

import json

import concourse.bass as bass
import concourse.tile as tile
from concourse import mybir
from concourse import bass2jax as _bass2jax
from concourse.bass_utils import run_bass_kernel_spmd
from concourse.bass_utils import compile_bir_kernel as _orig_compile_bir_kernel

# --- workaround: this container's walrus rejects instructions carrying more
# than one sync-wait command ("Too many sync wait commands"), but Tile's
# scheduler emits multi-wait instructions.  Split them in the serialized BIR
# by inserting wait-only EventSemaphore carriers immediately before, on the
# same engine queue (semantically identical: same queue position, waits
# simply execute as separate instructions).
_WSPLIT_UID = [0]


def _split_bir_waits(bir_json: bytes, maxw: int = 1) -> bytes:
    m = json.loads(bir_json)
    tmpl = None
    for fn in m["functions"]:
        for bb in fn["blocks"]:
            for ins in bb["instructions"]:
                if ins.get("opcode") == "EventSemaphore":
                    tmpl = json.loads(json.dumps(ins))
                    break
            if tmpl:
                break
    if tmpl is None:
        return bir_json
    for fn in m["functions"]:
        for bb in fn["blocks"]:
            out = []
            for ins in bb["instructions"]:
                si = ins.get("sync_info")
                waits = (si or {}).get("on_wait") or []
                if len(waits) > maxw:
                    keep = waits[-maxw:]
                    extra = waits[:-maxw]
                    for i in range(0, len(extra), maxw):
                        _WSPLIT_UID[0] += 1
                        d = json.loads(json.dumps(tmpl))
                        d["name"] = f"WSPLIT-{_WSPLIT_UID[0]}"
                        d["engine"] = ins["engine"]
                        d["ins"] = []
                        d["outs"] = []
                        d["sync_info"] = {
                            "on_wait": extra[i : i + maxw],
                            "on_update": [],
                        }
                        d.pop("debug", None)
                        d.pop("bass_addl_debug", None)
                        out.append(d)
                    si["on_wait"] = keep
                out.append(ins)
            bb["instructions"] = out
    return json.dumps(m).encode()


def _patched_compile_bir_kernel(bir_json, tmpdir, neff_name="file.neff"):
    return _orig_compile_bir_kernel(
        _split_bir_waits(bir_json), tmpdir, neff_name=neff_name
    )


_bass2jax.compile_bir_kernel = _patched_compile_bir_kernel

# dims
B, D, A = 256, 512, 1024
NO, NR, NI, E, H = 65, 17, 2, 64, 128
NCORES = 8
BL = B // NCORES  # 32 batch rows per core

F32 = mybir.dt.float32
BF16 = mybir.dt.bfloat16
AF = mybir.ActivationFunctionType
ALU = mybir.AluOpType

NOP = 68  # rs head padded to a multiple of 4 opcodes
NGRP = NOP // 4  # 17 groups of 4 opcodes (rs head)


def _bf(x):
    import ml_dtypes

    return np.asarray(x, dtype=ml_dtypes.bfloat16)


def _f32(x):
    return np.ascontiguousarray(np.asarray(x, dtype=np.float32))


# ---------------------------------------------------------------------------
# packed-constant layout: chunk -> (partitions, dtype, [(name, rows, cols)])
# one DRAM param + one SBUF tile + one DMA per chunk; DMAs are issued in
# this order (first-consumer order)
_PACKS = [
    ("c_a2", 128, BF16, [("embrd", 2 * E, A), ("wrde", 2 * E, H),
                         ("wime1", 2 * E, H), ("wd32", H, 32)]),
    ("c_c64", 64, BF16, [("wrse", E, H), ("embrs", E, NOP), ("wime2", E, H),
                         ("embim2", E, A)]),
    ("c_feat", 128, BF16, [("featT", 128, 128)]),
    ("c_w1im", 128, BF16, [("w1im", 128, 512)]),
    ("c_f32", 128, F32, [("b1s", H, 4), ("b2op", NO, 1), ("b2rs", H, 1),
                         ("b2rd", H, 1), ("db2im", BL, 1)]),
    ("c_a2r", 128, BF16, [("w2opT", H, NO), ("w2rsT", H, 32),
                          ("w2rdT", H, 32), ("suind_rs", H, 4),
                          ("embrd_u", 2 * E, 704), ("onescol", 128, 1),
                          ("onesrow", 1, 32)]),
    ("c_w1rs", 128, BF16, [("w1rs", 128, 512)]),
    ("c_w1rd", 128, BF16, [("w1rd", 128, 512)]),
    ("c_w1op", 128, BF16, [("w1op", 128, 512)]),
    ("c_b1", 128, BF16, [("gop", NO, A), ("grs", H, A)]),
    ("c_low32", BL, BF16, [("m32", BL, A), ("id32", BL, 32)]),
    ("c_low4", 4, BF16, [("negones", 1, A), ("mdb2", 1, A), ("g2rs", 4, A)]),
    ("c_b2", 128, BF16, [("grd", H, A), ("gpair", H, A)]),
]
_PACK_ENG = {n: (i % 2) for i, (n, _, _, _) in enumerate(_PACKS)}


def _host_prep(inputs):
    """Build all per-core / shared device constants on host (index ops only
    plus dtype packing; all real FLOPs happen on device)."""
    feats = _f32(inputs["features"])
    o = np.clip(inputs["act_o"].astype(np.int64), 0, NO - 1)
    rs = np.clip(inputs["act_rs"].astype(np.int64), 0, NR - 1)
    rd = np.clip(inputs["act_rd"].astype(np.int64), 0, NR - 1)
    im = np.clip(inputs["act_imm"].astype(np.int64), 0, NI - 1)

    perm = np.lexsort((rs, o))  # sort by (opcode, reg_src)
    os_, rss, rds, ims = o[perm], rs[perm], rd[perm], im[perm]

    opcode_embed = _f32(inputs["opcode_embed"])  # [65, 64]
    reg_embed = _f32(inputs["reg_embed"])  # [17, 64]
    op_e = opcode_embed[os_]  # [A, 64] sorted
    rs_e = reg_embed[rss]
    rd_e = reg_embed[rds]

    W = {k: _f32(inputs[k]) for k in inputs if k.endswith(("W1", "W2", "b1", "b2"))}

    c = {}
    # feature-path weights per head: [128, 512] with K-chunk k at cols
    # [128k, 128k+128)
    for nm, wk in (("im", "imm_W1"), ("rs", "rs_W1"), ("rd", "rd_W1"),
                   ("op", "op_W1")):
        wT = W[wk][:, :D].T  # [D, H]
        c["w1" + nm] = np.concatenate(
            [wT[128 * k : 128 * (k + 1), :] for k in range(4)], axis=1
        )  # [128, 512]
    c["b1s"] = _f32(
        np.stack([W["imm_b1"], W["rs_b1"], W["rd_b1"], W["op_b1"]], axis=1)
    )  # [128, 4]

    # embedding-path weights + gathered embeddings (stacked on K)
    c["wrse"] = W["rs_W1"][:, D : D + E].T  # [64, 128]
    embrs = np.zeros((E, NOP), np.float32)
    embrs[:, :NO] = opcode_embed.T
    c["embrs"] = embrs  # [64, 68] all opcodes (padded)
    c["wrde"] = np.concatenate(
        [W["rd_W1"][:, D : D + E].T, W["rd_W1"][:, D + E : D + 2 * E].T], axis=0
    )  # [128, 128]
    c["embrd"] = np.concatenate([op_e.T, rs_e.T], axis=0)  # [128, A]
    c["wime1"] = np.concatenate(
        [W["imm_W1"][:, D : D + E].T, W["imm_W1"][:, D + E : D + 2 * E].T], axis=0
    )  # [128, 128]
    c["wime2"] = W["imm_W1"][:, D + 2 * E :].T  # [64, 128]
    c["embim2"] = rd_e.T  # [64, A]

    # head-2 weights (V padded to 32 with zeros so PSUM pad rows are written)
    c["w2opT"] = W["op_W2"].T  # [128, 65]
    w2rs = np.zeros((H, 32), np.float32)
    w2rs[:, :NR] = W["rs_W2"].T
    c["w2rsT"] = w2rs
    w2rd = np.zeros((H, 32), np.float32)
    w2rd[:, :NR] = W["rd_W2"].T
    c["w2rdT"] = w2rd
    # imm head: difference vector w_d = W2[0] - W2[1], replicated to 32
    # PE columns so every psum row of a strip holds d (no garbage rows)
    c["wd32"] = np.tile((W["imm_W2"][0] - W["imm_W2"][1]).reshape(H, 1), (1, 32))
    db2 = float(W["imm_b2"][0] - W["imm_b2"][1])
    c["db2im"] = np.full((BL, 1), db2, np.float32)

    # biases b2 as per-partition columns
    c["b2op"] = _f32(W["op_b2"][:, None])  # [65, 1]
    for nm, b2 in (("b2rs", W["rs_b2"]), ("b2rd", W["rd_b2"])):
        t = np.zeros((H, 1), np.float32)
        for s in range(4):
            t[32 * s : 32 * s + NR, 0] = b2
        c[nm] = t

    # op-head gather one-hot + misc rows
    m = (ims == 0).astype(np.float32)  # [A] imm-head class-0 selector
    gop = np.zeros((NO, A), np.float32)
    gop[os_, np.arange(A)] = 1.0
    c["gop"] = gop
    c["onescol"] = np.ones((128, 1), np.float32)
    c["onesrow"] = np.ones((1, 32), np.float32)
    c["negones"] = -np.ones((1, A), np.float32)
    c["mdb2"] = (db2 * m)[None, :]  # [1, A]
    c["m32"] = np.broadcast_to(m, (BL, A)).copy()

    # rs-head gather tables (block one-hot; actions sorted by opcode)
    grs = np.zeros((H, A), np.float32)
    grs[(os_ % 4) * 32 + rss, np.arange(A)] = 1.0
    c["grs"] = grs
    g2rs = np.zeros((4, A), np.float32)
    g2rs[os_ % 4, np.arange(A)] = -1.0
    c["g2rs"] = g2rs
    suind_rs = np.zeros((H, 4), np.float32)
    for s in range(4):
        suind_rs[32 * s : 32 * s + NR, s] = 1.0
    c["suind_rs"] = suind_rs

    # rd head deduplicated over distinct (opcode, reg_src) pairs
    pairs_all = os_ * NR + rss                      # non-decreasing (sorted)
    u_pairs, pid = np.unique(pairs_all, return_inverse=True)
    U = len(u_pairs)
    U4 = 704  # fixed pad (U ~ 660-680 for random actions; assert below)
    assert U <= U4, f"U={U} exceeds pad {U4}"
    NPG = U4 // 4  # 176 pair-groups of 4
    o_u = u_pairs // NR
    rs_u = u_pairs % NR
    embrd_u = np.zeros((2 * E, U4), np.float32)
    embrd_u[:E, :U] = opcode_embed[o_u].T
    embrd_u[E:, :U] = reg_embed[rs_u].T
    c["embrd_u"] = embrd_u
    # grd: one-hot [(strip, v), a] selecting (pid(a)%4, rd(a))
    grd = np.zeros((H, A), np.float32)
    grd[32 * (pid % 4) + rds, np.arange(A)] = 1.0
    c["grd"] = grd
    # gpair: -1 one-hot [pid%128, a] for the lnsu gather (per 128-pair chunk)
    gpair = np.zeros((H, A), np.float32)
    gpair[pid % H, np.arange(A)] = -1.0
    c["gpair"] = gpair
    c["id32"] = np.eye(BL, 32, dtype=np.float32)

    # rs gather chunk column ranges (static, baked into program; identical
    # on every core since actions are replicated)
    bounds = np.searchsorted(os_, np.arange(0, NO + 4, 4)[: NGRP + 1])
    chunks = []
    for g in range(NGRP):
        lo, hi = int(bounds[g]), int(bounds[g + 1])
        while lo < hi:
            nxt = min(hi, ((lo // 512) + 1) * 512, lo + 512)
            chunks.append((g, lo, nxt))
            lo = nxt
    # rd-head gather chunk ranges: per pair-group (grd/q gather) and per
    # 128-pair chunk (lnsu gather), split at psum bank boundaries
    pgrp_a = pid // 4
    qchunks = []
    for pg in range(int(pgrp_a.max()) + 1):
        lo = int(np.searchsorted(pgrp_a, pg))
        hi = int(np.searchsorted(pgrp_a, pg + 1))
        while lo < hi:
            nxt = min(hi, ((lo // 512) + 1) * 512)
            qchunks.append((pg, lo, nxt))
            lo = nxt
    lchunks = []
    for ch in range((U + H - 1) // H):
        lo = int(np.searchsorted(pid, H * ch))
        hi = int(np.searchsorted(pid, H * (ch + 1)))
        while lo < hi:
            nxt = min(hi, ((lo // 512) + 1) * 512)
            lchunks.append((ch, lo, nxt))
            lo = nxt
    feat_T = feats.T  # [D, B]
    per_core = []
    for cid in range(NCORES):
        ft = feat_T[:, cid * BL : (cid + 1) * BL]  # [512, 32]
        ftp = np.concatenate(
            [ft[128 * k : 128 * (k + 1), :] for k in range(4)], axis=1
        )  # [128, 128]
        per_core.append({"featT": ftp})

    # assemble packed chunk arrays
    packed = {}
    for chunk, parts, dt, entries in _PACKS:
        ncols = sum(e[2] for e in entries)
        arr = np.zeros((parts, ncols), np.float32)
        off = 0
        for name, rows, cols in entries:
            if name != "featT":
                arr[:rows, off : off + cols] = c[name]
            off += cols
        packed[chunk] = arr if dt == F32 else _bf(arr)
    return packed, per_core, (tuple(chunks), tuple(qchunks), tuple(lchunks)), perm


def build_program(allchunks):
    chunks, qchunks, lchunks = allchunks
    nc = bass.Bass()
    dr = {}
    for chunk, parts, dt, entries in _PACKS:
        ncols = sum(e[2] for e in entries)
        dr[chunk] = nc.declare_dram_parameter(chunk, [parts, ncols], dt,
                                              isOutput=False)
    out_d = nc.declare_dram_parameter("out", [BL, A], F32, isOutput=True)

    def MM(*a, **k):
        k.setdefault("skip_group_check", True)
        return nc.tensor.matmul(*a, **k)

    with ExitStack() as ctx:
        tc = ctx.enter_context(tile.TileContext(nc))
        cp = ctx.enter_context(tc.tile_pool(name="consts", bufs=1))
        sb = ctx.enter_context(tc.tile_pool(name="sbuf", bufs=2))
        hb = ctx.enter_context(tc.tile_pool(name="hbuf", bufs=8))
        eb = ctx.enter_context(tc.tile_pool(name="ebuf", bufs=3))
        pA = ctx.enter_context(tc.tile_pool(name="pA", bufs=1, space="PSUM"))
        pB = ctx.enter_context(tc.tile_pool(name="pB", bufs=1, space="PSUM"))
        pG = ctx.enter_context(tc.tile_pool(name="pG", bufs=4, space="PSUM"))

        # ---- load packed constants: one tile + one DMA per chunk, in
        # first-consumer order; ct[] = slices into the chunk tiles
        dma_engs = [nc.sync, nc.gpsimd]
        ct = {}
        for chunk, parts, dt, entries in _PACKS:
            ncols = sum(e[2] for e in entries)
            t = cp.tile([parts, ncols], dt, tag=chunk, name=chunk)
            dma_engs[_PACK_ENG[chunk]].dma_start(t[:, :], dr[chunk][:, :])
            off = 0
            for name, rows, cols in entries:
                ct[name] = t[0:rows, off : off + cols]
                off += cols

        # ---- PE warm-up: the p-state ramp needs ~3us of continuous busy
        # (cold MMs run 2-4x slower).  Burn dummy matmuls on a memset tile
        # while the first const DMAs are still in flight.
        warm = cp.tile([H, 512], BF16, tag="warm", name="warm")
        nc.vector.memset(warm[:, :], 0.0)
        for w in range(5):
            pwu = pG.tile([H, 512], F32, tag="lgh", name=f"warm{w}")
            MM(pwu[:, :], warm[:, 0:128], warm[:, :])

        # ---- ep tables (embedding partials) on PE; psum->sbuf copies on ACT
        ep_im = eb.tile([H, A], BF16, tag="ep_im")
        for j in range(2):
            pe_h = pG.tile([H, 512], F32, tag="lgh", name=f"pei{j}")
            MM(pe_h[:, :], ct["wime1"][:, :], ct["embrd"][:, 512 * j : 512 * (j + 1)],
               start=True, stop=False)
            MM(pe_h[:, :], ct["wime2"][:, :], ct["embim2"][:, 512 * j : 512 * (j + 1)],
               start=False, stop=True)
            nc.scalar.activation(ep_im[:, 512 * j : 512 * (j + 1)], pe_h[:, :],
                                 AF.Identity)
        U4, NPG = 704, 176
        ep_rd = eb.tile([H, U4], BF16, tag="ep_rd")
        for j, (c0, c1) in enumerate(((0, 512), (512, U4))):
            pe_h = pG.tile([H, c1 - c0], F32, tag="lgh", name=f"pep{j}",
                           padded_shape=[H, 512])
            MM(pe_h[:, :], ct["wrde"][:, :], ct["embrd_u"][:, c0:c1])
            nc.scalar.activation(ep_rd[:, c0:c1], pe_h[:, :], AF.Identity)
        psum_ep3 = pG.tile([H, NOP], F32, tag="lgh", padded_shape=[H, 512])
        MM(psum_ep3[:, :], ct["wrse"][:, :], ct["embrs"][:, :])
        ep_rs = eb.tile([H, NOP], BF16, tag="ep_rs")
        nc.scalar.activation(ep_rs[:, :], psum_ep3[:, :], AF.Identity)

        # ---- fp (feature partials): one [H, 4*BL] psum tile, one
        # accumulation group per head, emitted head-by-head as w1 DMAs land
        psum_fp = pB.tile([H, 4 * BL], F32, tag="seqB", padded_shape=[H, 512])
        fp = {}

        def emit_fp(hd, nm):
            for k in range(4):
                MM(
                    psum_fp[:, 32 * hd : 32 * hd + BL],
                    ct["w1" + nm][:, 128 * k : 128 * (k + 1)],
                    ct["featT"][:, BL * k : BL * (k + 1)],
                    start=(k == 0),
                    stop=(k == 3),
                )
            if nm == "op":
                t = sb.tile([H, BL], BF16, tag="op_h", name="op_h")
                nc.scalar.activation(t[:, :], psum_fp[:, 96:128], AF.Relu,
                                     bias=ct["b1s"][:, 3:4])
            else:
                t = sb.tile([H, BL], F32, tag=f"fp_{nm}", name=f"fp_{nm}")
                nc.scalar.activation(t[:, :], psum_fp[:, 32 * hd : 32 * hd + BL],
                                     AF.Identity, bias=ct["b1s"][:, hd : hd + 1])
            fp[nm] = t

        emit_fp(0, "im")

        # ---- deferred op/rs head pieces (emitted inside the imm loop)
        st = {}

        def emit_h_rs(b0, b1):
            if "h_rs" not in st:
                st["h_rs"] = cp.tile([H, NOP * BL], BF16, tag="h_rs", name="h_rs")
            for b in range(b0, b1):
                nc.gpsimd.tensor_scalar(
                    st["h_rs"][:, NOP * b : NOP * (b + 1)],
                    ep_rs[:, :], fp["rs"][:, b : b + 1], 0.0,
                    op0=ALU.add, op1=ALU.max,
                )

        def emit_op_logits():
            psum_opl = pG.tile([NO, BL], F32, tag="lgh", padded_shape=[NO, 512],
                               name="psum_opl")
            MM(psum_opl[:, :], ct["w2opT"][:, :], fp["op"][:, :])
            st["exp_op"] = sb.tile([NO, BL], BF16, tag="exp_op", name="exp_op")
            nc.scalar.activation(st["exp_op"][:, :], psum_opl[:, :], AF.Exp,
                                 bias=ct["b2op"][:, :])
            st["lb2_op"] = sb.tile([NO, BL], BF16, tag="lb2_op", name="lb2_op")
            nc.scalar.activation(st["lb2_op"][:, :], psum_opl[:, :], AF.Identity,
                                 bias=ct["b2op"][:, :])

        def emit_rs_logits():
            h_rs_v = st["h_rs"][:, :].rearrange("p (b c) -> p c b", c=NOP)
            psum_rsl = pB.tile([H, 32 * NGRP], F32, tag="seqB", name="psum_rsl")
            for c_ in range(NOP):
                g, s = c_ // 4, c_ % 4
                MM(
                    psum_rsl[32 * s : 32 * s + 32, 32 * g : 32 * g + 32],
                    ct["w2rsT"][:, :],
                    h_rs_v[:, c_, :],
                    tile_position=(0, 32 * s),
                )
            st["exp_rs"] = sb.tile([H, 32 * NGRP], BF16, tag="exp_rs", name="exp_rs")
            nc.scalar.activation(st["exp_rs"][:, :], psum_rsl[:, :], AF.Exp,
                                 bias=ct["b2rs"][:, :])
            st["lb2_rs"] = sb.tile([H, 32 * NGRP], BF16, tag="lb2_rs", name="lb2_rs")
            nc.scalar.activation(st["lb2_rs"][:, :], psum_rsl[:, :], AF.Identity,
                                 bias=ct["b2rs"][:, :])

        def emit_su_sections():
            psum_osu = pG.tile([1, BL], F32, tag="lgh", padded_shape=[1, 512],
                               name="psum_osu")
            MM(psum_osu[:, :], ct["onescol"][0:NO, :], st["exp_op"][:, :])
            st["lnsu_op"] = sb.tile([1, BL], BF16, tag="lnsu_op", name="lnsu_op")
            nc.scalar.activation(st["lnsu_op"][:, :], psum_osu[:, :], AF.Ln)
            psum_rsu = pG.tile([4, 512], F32, tag="lgh", name="psum_rsu")
            MM(psum_rsu[:, :], ct["suind_rs"][:, :], st["exp_rs"][:, 0:512])
            psum_rsu2 = pG.tile([4, 32 * NGRP - 512], F32, tag="lgh",
                                padded_shape=[4, 512], name="psum_rsu2")
            MM(psum_rsu2[:, :], ct["suind_rs"][:, :], st["exp_rs"][:, 512 : 32 * NGRP])
            st["lnsu_rs"] = sb.tile([4, 32 * NGRP], BF16, tag="lnsu_rs",
                                    name="lnsu_rs")
            nc.scalar.activation(st["lnsu_rs"][:, 0:512], psum_rsu[:, :], AF.Ln)
            nc.scalar.activation(st["lnsu_rs"][:, 512 : 32 * NGRP],
                                 psum_rsu2[:, :], AF.Ln)

        def emit_acc():
            psum_acc = pA.tile([BL, A], F32, tag="seqA", name="psum_acc")
            for j in range(2):
                MM(psum_acc[:, 512 * j : 512 * (j + 1)], st["lb2_op"][:, :],
                   ct["gop"][:, 512 * j : 512 * (j + 1)], start=True, stop=False)
                MM(psum_acc[:, 512 * j : 512 * (j + 1)], st["lnsu_op"][:, :],
                   ct["negones"][:, 512 * j : 512 * (j + 1)], start=False,
                   stop=False)
                MM(psum_acc[:, 512 * j : 512 * (j + 1)], ct["onesrow"][:, :],
                   ct["mdb2"][:, 512 * j : 512 * (j + 1)], start=False, stop=False)
            last_for_bank = {}
            for i, (g, lo, hi) in enumerate(chunks):
                last_for_bank[lo // 512] = i
            for i, (g, lo, hi) in enumerate(chunks):
                MM(psum_acc[:, lo:hi], st["lb2_rs"][:, 32 * g : 32 * g + 32],
                   ct["grs"][:, lo:hi], start=False, stop=False)
                MM(psum_acc[:, lo:hi], st["lnsu_rs"][:, 32 * g : 32 * g + 32],
                   ct["g2rs"][:, lo:hi], start=False, stop=False)
            st["psum_acc"] = psum_acc

        # ---- rd-head h table (deduplicated over (o, rs) pairs), b-major
        h_rd_all = cp.tile([H, U4 * BL], BF16, tag="h_rd_all", name="h_rd_all")

        def emit_h_rd(b0, b1, eng):
            for b in range(b0, b1):
                eng.tensor_scalar(
                    h_rd_all[:, U4 * b : U4 * (b + 1)],
                    ep_rd[:, :], fp["rd"][:, b : b + 1], 0.0,
                    op0=ALU.add, op1=ALU.max,
                )

        # ---- imm head phase: h_im tiles + strip matmuls (every psum row of
        # strip s holds d for b=4g+s) -> bf16 copies into d_all -> one
        # SBUF->SBUF gather DMA per strip into compact X32[32, A].
        # op/rs/acc sections are emitted between waves so each engine's
        # in-order queue reaches them right as their inputs land.
        X32 = cp.tile([BL, A], BF16, tag="X32", name="X32")
        d_all = cp.tile([H, 16 * 512], BF16, tag="d_all", name="d_all")
        CP_ENG = ["A", "A", "A", "D"]  # psum reads: ACT/DVE only
        for g in range(8):
            hts_im = []
            for s in range(4):
                b = 4 * g + s
                h_t = hb.tile([H, A], BF16, tag="h", name=f"him{b}")
                nc.vector.tensor_scalar(
                    h_t[:, :], ep_im[:, :], fp["im"][:, b : b + 1], 0.0,
                    op0=ALU.add, op1=ALU.max,
                )
                hts_im.append(h_t)
            for j in range(2):
                psum_d = pG.tile([H, 512], F32, tag="lgh", name=f"d{g}{j}")
                for s in range(4):
                    MM(
                        psum_d[32 * s : 32 * s + 32, :],
                        ct["wd32"][:, :],
                        hts_im[s][:, 512 * j : 512 * (j + 1)],
                        tile_position=(0, 32 * s),
                    )
                dsl = d_all[:, 512 * (2 * g + j) : 512 * (2 * g + j + 1)]
                lane = CP_ENG[(2 * g + j) % len(CP_ENG)]
                if lane == "A":
                    nc.scalar.activation(dsl, psum_d[:, :], AF.Identity)
                else:
                    nc.vector.tensor_copy(dsl, psum_d[:, :])
            if g == 0:
                emit_fp(1, "rs")
            elif g == 1:
                emit_fp(2, "rd")
                emit_h_rs(0, 16)
            elif g == 2:
                emit_fp(3, "op")
                emit_h_rs(16, BL)
                emit_h_rd(0, 2, nc.vector)
            elif g == 3:
                emit_op_logits()
                emit_h_rd(2, 4, nc.vector)
            elif g == 4:
                emit_rs_logits()
                emit_h_rd(4, 6, nc.vector)
            elif g == 5:
                emit_su_sections()
                emit_h_rd(6, 8, nc.vector)
                emit_h_rd(24, 27, nc.gpsimd)
            elif g == 6:
                emit_h_rd(8, 11, nc.vector)
                emit_h_rd(27, 30, nc.gpsimd)
            elif g == 7:
                emit_h_rd(11, 14, nc.vector)
                emit_h_rd(30, BL, nc.gpsimd)
        # gather: X32[4g+s, 512j+c] = d_all[32s, (2g+j)*512 + c]
        for s in range(4):
            dma_engs[s % 2].dma_start(
                X32[s : BL : 4, :].rearrange("g (j c) -> g j c", j=2),
                d_all[32 * s : 32 * s + 1, :].rearrange(
                    "p (g j c) -> p g j c", g=8, j=2
                ),
            )

        # finish h_rd, then acc gathers overlap the DVE tail
        emit_h_rd(14, 22, nc.vector)
        for b in (22, 23):
            nc.scalar.activation(
                h_rd_all[:, U4 * b : U4 * (b + 1)], ep_rd[:, :], AF.Relu,
                bias=fp["rd"][:, b : b + 1],
            )
        emit_acc()

        # ---- rd head: deduplicated logits table [(strip, v), (pgrp, b)]
        # in 11 psum waves; exp on ACT; su-reduce (4 cols per pair-group)
        sp32 = cp.tile([BL, A], BF16, tag="sp32")
        u32 = cp.tile([BL, A], BF16, tag="u32")
        ctr_im = cp.tile([BL, A], BF16, tag="ctr_im")
        h_rd_v = h_rd_all[:, :].rearrange("p (b c) -> p c b", c=U4)
        exp_tbl = cp.tile([H, 32 * NPG], BF16, tag="exp_tbl", name="exp_tbl")
        psum_sutbl = pB.tile([BL, U4], F32, tag="seqB", name="psum_sutbl")

        # imm-head tail: sp = softplus(X + db2) = ln(1 + e^(X+db2)),
        # u = X*m, ctr_im = u - sp (db2*m is already in the acc); emitted
        # here so ACT does it in the imm->rd transition, not mid table-loop
        e32 = cp.tile([BL, A], BF16, tag="e32", name="e32")
        nc.scalar.activation(e32[:, :], X32[:, :], AF.Exp,
                             bias=ct["db2im"][:, :])
        e1 = cp.tile([BL, A], BF16, tag="e1", name="e1")
        nc.vector.tensor_scalar_add(e1[:, :], e32[:, :], 1.0)
        nc.scalar.activation(sp32[:, :], e1[:, :], AF.Ln)
        nc.vector.tensor_mul(u32[:, :], X32[:, :], ct["m32"][:, :])
        nc.vector.tensor_sub(ctr_im[:, :], u32[:, :], sp32[:, :])

        def emit_su(t):
            # su-reduce for tile t, emitted one tile late so exp(t) is done
            # and the PE wait-queue never blocks mid-loop
            for pgl in range(16):
                pg = 16 * t + pgl
                MM(
                    psum_sutbl[:, 4 * pg : 4 * pg + 4],
                    exp_tbl[:, 512 * t + 32 * pgl : 512 * t + 32 * pgl + 32],
                    ct["suind_rs"][:, :],
                )

        for t in range(NPG // 16):
            ptbl = pG.tile([H, 512], F32, tag="lgh", name=f"tbl{t}")
            for pl in range(64):
                p_ = 64 * t + pl
                s = p_ % 4
                pgl = (p_ // 4) % 16
                MM(
                    ptbl[32 * s : 32 * s + 32, 32 * pgl : 32 * pgl + 32],
                    ct["w2rdT"][:, :],
                    h_rd_v[:, p_, :],
                    tile_position=(0, 32 * s),
                )
            nc.scalar.activation(exp_tbl[:, 512 * t : 512 * (t + 1)], ptbl[:, :],
                                 AF.Exp, bias=ct["b2rd"][:, :])
            if t >= 1:
                emit_su(t - 1)

        emit_su(NPG // 16 - 1)

        # ---- lnsu table [32, U4] -> transpose to [pair, b] chunks
        lnsu_tbl = cp.tile([BL, 768], BF16, tag="lnsu_tbl", name="lnsu_tbl")
        nc.scalar.activation(lnsu_tbl[:, 0:U4], psum_sutbl[:, :], AF.Ln)
        nc.vector.memset(lnsu_tbl[:, U4:768], 0.0)
        lnsuT = cp.tile([H, 6 * 32], BF16, tag="lnsuT", name="lnsuT")
        for ch in range(6):
            pt = pG.tile([H, 32], BF16, tag="lgh", name=f"ptr{ch}",
                         padded_shape=[H, 512])
            nc.tensor.transpose(pt[:, :], lnsu_tbl[:, 128 * ch : 128 * (ch + 1)],
                                ct["id32"][:, :])
            nc.vector.tensor_copy(lnsuT[:, 32 * ch : 32 * (ch + 1)], pt[:, :])

        # ---- gathers: exp_sel into psum_q (by pair-group), -lnsu into the
        # open accumulator (by 128-pair chunk); bank-wise start/stop
        psum_q = pB.tile([BL, A], F32, tag="seqB", name="psum_q")
        qfirst, qlast = {}, {}
        for i, (pg, lo, hi) in enumerate(qchunks):
            b = lo // 512
            qfirst.setdefault(b, i)
            qlast[b] = i
        for i, (pg, lo, hi) in enumerate(qchunks):
            b = lo // 512
            MM(
                psum_q[:, lo:hi],
                exp_tbl[:, 32 * pg : 32 * pg + 32],
                ct["grd"][:, lo:hi],
                start=(qfirst[b] == i),
                stop=(qlast[b] == i),
            )
        lnsel = cp.tile([BL, A], BF16, tag="lnsel")

        llast = {}
        for i, (ch, lo, hi) in enumerate(lchunks):
            llast[lo // 512] = i
        psum_acc = st["psum_acc"]
        for i, (ch, lo, hi) in enumerate(lchunks):
            MM(
                psum_acc[:, lo:hi],
                lnsuT[:, 32 * ch : 32 * ch + 32],
                ct["gpair"][:, lo:hi],
                start=False,
                stop=(llast[lo // 512] == i),
            )

        # ---- final combine + store, per psum bank
        for j in range(2):
            sl = slice(512 * j, 512 * (j + 1))
            nc.scalar.activation(lnsel[:, sl], psum_q[:, sl], AF.Ln)
            w32 = sb.tile([BL, 512], BF16, tag="w32", name=f"w32{j}")
            nc.vector.tensor_add(w32[:, :], ctr_im[:, sl], lnsel[:, sl])
            out_sb = sb.tile([BL, 512], F32, tag="out_sb", name=f"out{j}")
            nc.vector.tensor_add(out_sb[:, :], w32[:, :], psum_acc[:, sl])
            dma_engs[j].dma_start(out_d[:, sl], out_sb[:, :])

    return nc.tensor.matmul(*a, **k)

    with ExitStack() as ctx:
        tc = ctx.enter_context(tile.TileContext(nc))
        cp = ctx.enter_context(tc.tile_pool(name="consts", bufs=1))
        sb = ctx.enter_context(tc.tile_pool(name="sbuf", bufs=2))
        hb = ctx.enter_context(tc.tile_pool(name="hbuf", bufs=8))
        eb = ctx.enter_context(tc.tile_pool(name="ebuf", bufs=3))
        pA = ctx.enter_context(tc.tile_pool(name="pA", bufs=1, space="PSUM"))
        pB = ctx.enter_context(tc.tile_pool(name="pB", bufs=1, space="PSUM"))
        pG = ctx.enter_context(tc.tile_pool(name="pG", bufs=4, space="PSUM"))

        # ---- load packed constants: one tile + one DMA per chunk, in
        # first-consumer order; ct[] = slices into the chunk tiles
        dma_engs = [nc.sync, nc.gpsimd]
        ct = {}
        for chunk, parts, dt, entries in _PACKS:
            ncols = sum(e[2] for e in entries)
            t = cp.tile([parts, ncols], dt, tag=chunk, name=chunk)
            dma_engs[_PACK_ENG[chunk]].dma_start(t[:, :], dr[chunk][:, :])
            off = 0
            for name, rows, cols in entries:
                ct[name] = t[0:rows, off : off + cols]
                off += cols

        # ---- PE warm-up: the p-state ramp needs ~3us of continuous busy
        # (cold MMs run 2-4x slower).  Burn dummy matmuls on a memset tile
        # while the first const DMAs are still in flight.
        warm = cp.tile([H, 512], BF16, tag="warm", name="warm")
        nc.vector.memset(warm[:, :], 0.0)
        for w in range(5):
            pwu = pG.tile([H, 512], F32, tag="lgh", name=f"warm{w}")
            MM(pwu[:, :], warm[:, 0:128], warm[:, :])

        # ---- ep tables (embedding partials) on PE; psum->sbuf copies on ACT
        ep_im = eb.tile([H, A], BF16, tag="ep_im")
        for j in range(2):
            pe_h = pG.tile([H, 512], F32, tag="lgh", name=f"pei{j}")
            MM(pe_h[:, :], ct["wime1"][:, :], ct["embrd"][:, 512 * j : 512 * (j + 1)],
               start=True, stop=False)
            MM(pe_h[:, :], ct["wime2"][:, :], ct["embim2"][:, 512 * j : 512 * (j + 1)],
               start=False, stop=True)
            nc.scalar.activation(ep_im[:, 512 * j : 512 * (j + 1)], pe_h[:, :],
                                 AF.Identity)
        U4, NPG = 704, 176
        ep_rd = eb.tile([H, U4], BF16, tag="ep_rd")
        for j, (c0, c1) in enumerate(((0, 512), (512, U4))):
            pe_h = pG.tile([H, c1 - c0], F32, tag="lgh", name=f"pep{j}",
                           padded_shape=[H, 512])
            MM(pe_h[:, :], ct["wrde"][:, :], ct["embrd_u"][:, c0:c1])
            nc.scalar.activation(ep_rd[:, c0:c1], pe_h[:, :], AF.Identity)
        psum_ep3 = pG.tile([H, NOP], F32, tag="lgh", padded_shape=[H, 512])
        MM(psum_ep3[:, :], ct["wrse"][:, :], ct["embrs"][:, :])
        ep_rs = eb.tile([H, NOP], BF16, tag="ep_rs")
        nc.scalar.activation(ep_rs[:, :], psum_ep3[:, :], AF.Identity)

        # ---- fp (feature partials): one [H, 4*BL] psum tile, one
        # accumulation group per head, emitted head-by-head as w1 DMAs land
        psum_fp = pB.tile([H, 4 * BL], F32, tag="seqB", padded_shape=[H, 512])
        fp = {}

        def emit_fp(hd, nm):
            for k in range(4):
                MM(
                    psum_fp[:, 32 * hd : 32 * hd + BL],
                    ct["w1" + nm][:, 128 * k : 128 * (k + 1)],
                    ct["featT"][:, BL * k : BL * (k + 1)],
                    start=(k == 0),
                    stop=(k == 3),
                )
            if nm == "op":
                t = sb.tile([H, BL], BF16, tag="op_h", name="op_h")
                nc.scalar.activation(t[:, :], psum_fp[:, 96:128], AF.Relu,
                                     bias=ct["b1s"][:, 3:4])
            else:
                t = sb.tile([H, BL], F32, tag=f"fp_{nm}", name=f"fp_{nm}")
                nc.scalar.activation(t[:, :], psum_fp[:, 32 * hd : 32 * hd + BL],
                                     AF.Identity, bias=ct["b1s"][:, hd : hd + 1])
            fp[nm] = t

        emit_fp(0, "im")

        # ---- deferred op/rs head pieces (emitted inside the imm loop)
        st = {}

        def emit_h_rs(b0, b1):
            if "h_rs" not in st:
                st["h_rs"] = cp.tile([H, NOP * BL], BF16, tag="h_rs", name="h_rs")
            for b in range(b0, b1):
                nc.gpsimd.tensor_scalar(
                    st["h_rs"][:, NOP * b : NOP * (b + 1)],
                    ep_rs[:, :], fp["rs"][:, b : b + 1], 0.0,
                    op0=ALU.add, op1=ALU.max,
                )

        def emit_op_logits():
            psum_opl = pG.tile([NO, BL], F32, tag="lgh", padded_shape=[NO, 512],
                               name="psum_opl")
            MM(psum_opl[:, :], ct["w2opT"][:, :], fp["op"][:, :])
            st["exp_op"] = sb.tile([NO, BL], BF16, tag="exp_op", name="exp_op")
            nc.scalar.activation(st["exp_op"][:, :], psum_opl[:, :], AF.Exp,
                                 bias=ct["b2op"][:, :])
            st["lb2_op"] = sb.tile([NO, BL], BF16, tag="lb2_op", name="lb2_op")
            nc.scalar.activation(st["lb2_op"][:, :], psum_opl[:, :], AF.Identity,
                                 bias=ct["b2op"][:, :])

        def emit_rs_logits():
            h_rs_v = st["h_rs"][:, :].rearrange("p (b c) -> p c b", c=NOP)
            psum_rsl = pB.tile([H, 32 * NGRP], F32, tag="seqB", name="psum_rsl")
            for c_ in range(NOP):
                g, s = c_ // 4, c_ % 4
                MM(
                    psum_rsl[32 * s : 32 * s + 32, 32 * g : 32 * g + 32],
                    ct["w2rsT"][:, :],
                    h_rs_v[:, c_, :],
                    tile_position=(0, 32 * s),
                )
            st["exp_rs"] = sb.tile([H, 32 * NGRP], BF16, tag="exp_rs", name="exp_rs")
            nc.scalar.activation(st["exp_rs"][:, :], psum_rsl[:, :], AF.Exp,
                                 bias=ct["b2rs"][:, :])
            st["lb2_rs"] = sb.tile([H, 32 * NGRP], BF16, tag="lb2_rs", name="lb2_rs")
            nc.scalar.activation(st["lb2_rs"][:, :], psum_rsl[:, :], AF.Identity,
                                 bias=ct["b2rs"][:, :])

        def emit_su_sections():
            psum_osu = pG.tile([1, BL], F32, tag="lgh", padded_shape=[1, 512],
                               name="psum_osu")
            MM(psum_osu[:, :], ct["onescol"][0:NO, :], st["exp_op"][:, :])
            st["lnsu_op"] = sb.tile([1, BL], BF16, tag="lnsu_op", name="lnsu_op")
            nc.scalar.activation(st["lnsu_op"][:, :], psum_osu[:, :], AF.Ln)
            psum_rsu = pG.tile([4, 512], F32, tag="lgh", name="psum_rsu")
            MM(psum_rsu[:, :], ct["suind_rs"][:, :], st["exp_rs"][:, 0:512])
            psum_rsu2 = pG.tile([4, 32 * NGRP - 512], F32, tag="lgh",
                                padded_shape=[4, 512], name="psum_rsu2")
            MM(psum_rsu2[:, :], ct["suind_rs"][:, :], st["exp_rs"][:, 512 : 32 * NGRP])
            st["lnsu_rs"] = sb.tile([4, 32 * NGRP], BF16, tag="lnsu_rs",
                                    name="lnsu_rs")
            nc.scalar.activation(st["lnsu_rs"][:, 0:512], psum_rsu[:, :], AF.Ln)
            nc.scalar.activation(st["lnsu_rs"][:, 512 : 32 * NGRP],
                                 psum_rsu2[:, :], AF.Ln)

        def emit_acc():
            psum_acc = pA.tile([BL, A], F32, tag="seqA", name="psum_acc")
            for j in range(2):
                MM(psum_acc[:, 512 * j : 512 * (j + 1)], st["lb2_op"][:, :],
                   ct["gop"][:, 512 * j : 512 * (j + 1)], start=True, stop=False)
                MM(psum_acc[:, 512 * j : 512 * (j + 1)], st["lnsu_op"][:, :],
                   ct["negones"][:, 512 * j : 512 * (j + 1)], start=False,
                   stop=False)
                MM(psum_acc[:, 512 * j : 512 * (j + 1)], ct["onesrow"][:, :],
                   ct["mdb2"][:, 512 * j : 512 * (j + 1)], start=False, stop=False)
            last_for_bank = {}
            for i, (g, lo, hi) in enumerate(chunks):
                last_for_bank[lo // 512] = i
            for i, (g, lo, hi) in enumerate(chunks):
                MM(psum_acc[:, lo:hi], st["lb2_rs"][:, 32 * g : 32 * g + 32],
                   ct["grs"][:, lo:hi], start=False, stop=False)
                MM(psum_acc[:, lo:hi], st["lnsu_rs"][:, 32 * g : 32 * g + 32],
                   ct["g2rs"][:, lo:hi], start=False, stop=False)
            st["psum_acc"] = psum_acc

        # ---- rd-head h table (deduplicated over (o, rs) pairs), b-major
        h_rd_all = cp.tile([H, U4 * BL], BF16, tag="h_rd_all", name="h_rd_all")

        def emit_h_rd(b0, b1, eng):
            for b in range(b0, b1):
                eng.tensor_scalar(
                    h_rd_all[:, U4 * b : U4 * (b + 1)],
                    ep_rd[:, :], fp["rd"][:, b : b + 1], 0.0,
                    op0=ALU.add, op1=ALU.max,
                )

        # ---- imm head phase: h_im tiles + strip matmuls (every psum row of
        # strip s holds d for b=4g+s) -> bf16 copies into d_all -> one
        # SBUF->SBUF gather DMA per strip into compact X32[32, A].
        # op/rs/acc sections are emitted between waves so each engine's
        # in-order queue reaches them right as their inputs land.
        X32 = cp.tile([BL, A], BF16, tag="X32", name="X32")
        d_all = cp.tile([H, 16 * 512], BF16, tag="d_all", name="d_all")
        CP_ENG = ["A", "A", "A", "D"]  # psum reads: ACT/DVE only
        for g in range(8):
            hts_im = []
            for s in range(4):
                b = 4 * g + s
                h_t = hb.tile([H, A], BF16, tag="h", name=f"him{b}")
                nc.vector.tensor_scalar(
                    h_t[:, :], ep_im[:, :], fp["im"][:, b : b + 1], 0.0,
                    op0=ALU.add, op1=ALU.max,
                )
                hts_im.append(h_t)
            for j in range(2):
                psum_d = pG.tile([H, 512], F32, tag="lgh", name=f"d{g}{j}")
                for s in range(4):
                    MM(
                        psum_d[32 * s : 32 * s + 32, :],
                        ct["wd32"][:, :],
                        hts_im[s][:, 512 * j : 512 * (j + 1)],
                        tile_position=(0, 32 * s),
                    )
                dsl = d_all[:, 512 * (2 * g + j) : 512 * (2 * g + j + 1)]
                lane = CP_ENG[(2 * g + j) % len(CP_ENG)]
                if lane == "A":
                    nc.scalar.activation(dsl, psum_d[:, :], AF.Identity)
                else:
                    nc.vector.tensor_copy(dsl, psum_d[:, :])
            if g == 0:
                emit_fp(1, "rs")
            elif g == 1:
                emit_fp(2, "rd")
                emit_h_rs(0, 16)
            elif g == 2:
                emit_fp(3, "op")
                emit_h_rs(16, BL)
                emit_h_rd(0, 2, nc.vector)
            elif g == 3:
                emit_op_logits()
                emit_h_rd(2, 4, nc.vector)
            elif g == 4:
                emit_rs_logits()
                emit_h_rd(4, 6, nc.vector)
            elif g == 5:
                emit_su_sections()
                emit_h_rd(6, 8, nc.vector)
                emit_h_rd(24, 27, nc.gpsimd)
            elif g == 6:
                emit_h_rd(8, 11, nc.vector)
                emit_h_rd(27, 30, nc.gpsimd)
            elif g == 7:
                emit_h_rd(11, 14, nc.vector)
                emit_h_rd(30, BL, nc.gpsimd)
        # gather: X32[4g+s, 512j+c] = d_all[32s, (2g+j)*512 + c]
        for s in range(4):
            dma_engs[s % 2].dma_start(
                X32[s : BL : 4, :].rearrange("g (j c) -> g j c", j=2),
                d_all[32 * s : 32 * s + 1, :].rearrange(
                    "p (g j c) -> p g j c", g=8, j=2
                ),
            )

        # ---- rd head phase (imm tail ops are emitted after wave g==3 so
        # the engines reach them right around when X32's gather DMA lands)
        sp32 = cp.tile([BL, A], BF16, tag="sp32")
        u32 = cp.tile([BL, A], BF16, tag="u32")
        ctr_im = cp.tile([BL, A], BF16, tag="ctr_im")
        psum_su = pA.tile([BL, A], F32, tag="seqA")
        psum_sel = pB.tile([BL, A], F32, tag="seqB")
        for g in range(8):
            hts_rd = []
            for s in range(4):
                b = 4 * g + s
                h_t = hb.tile([H, A], BF16, tag="h", name=f"hrd{b}")
                nc.vector.tensor_scalar(
                    h_t[:, :], ep_rd[:, :], fp["rd"][:, b : b + 1], 0.0,
                    op0=ALU.add, op1=ALU.max,
                )
                hts_rd.append(h_t)
            exp_t = sb.tile([H, A], BF16, tag="exp_t")
            mexp_t = sb.tile([H, A], BF16, tag="mexp_t")
            for j in range(2):
                psum_lg = pG.tile([H, 512], F32, tag="lgh", name=f"lg{g}{j}")
                for s in range(4):
                    MM(
                        psum_lg[32 * s : 32 * s + 32, :],
                        ct["w2rdT"][:, :],
                        hts_rd[s][:, 512 * j : 512 * (j + 1)],
                        tile_position=(0, 32 * s),
                    )
                nc.scalar.activation(
                    exp_t[:, 512 * j : 512 * (j + 1)], psum_lg[:, :], AF.Exp,
                    bias=ct["b2rd"][:, :],
                )
                nc.vector.tensor_mul(
                    mexp_t[:, 512 * j : 512 * (j + 1)],
                    exp_t[:, 512 * j : 512 * (j + 1)],
                    ct["mask_rd"][:, 512 * j : 512 * (j + 1)],
                )
            for j in range(2):
                MM(
                    psum_su[:, 512 * j : 512 * (j + 1)],
                    ct["suind_rd"][:, 32 * g : 32 * g + 32],
                    exp_t[:, 512 * j : 512 * (j + 1)],
                    start=(g == 0),
                    stop=(g == 7),
                )
                MM(
                    psum_sel[:, 512 * j : 512 * (j + 1)],
                    ct["suind_rd"][:, 32 * g : 32 * g + 32],
                    mexp_t[:, 512 * j : 512 * (j + 1)],
                    start=(g == 0),
                    stop=(g == 7),
                )
            if g == 3:
                # imm-head tail: sp = softplus(X + db2) = ln(1 + e^(X+db2))
                # (built from Exp/Ln, which are already ACT-table residents),
                # u = X*m, ctr_im = u - sp (db2*m is already in the acc)
                e32 = cp.tile([BL, A], BF16, tag="e32", name="e32")
                nc.scalar.activation(e32[:, :], X32[:, :], AF.Exp,
                                     bias=ct["db2im"][:, :])
                e1 = cp.tile([BL, A], BF16, tag="e1", name="e1")
                nc.vector.tensor_scalar_add(e1[:, :], e32[:, :], 1.0)
                nc.scalar.activation(sp32[:, :], e1[:, :], AF.Ln)
                nc.vector.tensor_mul(u32[:, :], X32[:, :], ct["m32"][:, :])
                nc.vector.tensor_sub(ctr_im[:, :], u32[:, :], sp32[:, :])
            if g == 4:
                t3 = sb.tile([BL, A], BF16, tag="t3")
                nc.vector.tensor_add(t3[:, :], ctr_im[:, :], st["acc_sb"][:, :])

        # ---- rd-head tail + final combine + store, pipelined by psum bank
        for j in range(2):
            sl = slice(512 * j, 512 * (j + 1))
            lnsu_t = sb.tile([BL, 512], BF16, tag="lnsu_rdt", name=f"lnsu{j}")
            nc.scalar.activation(lnsu_t[:, :], psum_su[:, sl], AF.Ln)
            lnsel_t = sb.tile([BL, 512], BF16, tag="lnsel_rdt", name=f"lnsel{j}")
            nc.scalar.activation(lnsel_t[:, :], psum_sel[:, sl], AF.Ln)
            ctr_rd = sb.tile([BL, 512], BF16, tag="ctr_rd", name=f"ctrrd{j}")
            nc.vector.tensor_sub(ctr_rd[:, :], lnsel_t[:, :], lnsu_t[:, :])
            out_sb = sb.tile([BL, 512], F32, tag="out_sb", name=f"out{j}")
            nc.vector.tensor_add(out_sb[:, :], ctr_rd[:, :], t3[:, sl])
            nc.sync.dma_start(out_d[:, sl], out_sb[:, :])

    return nc


_CACHE = {}


def _get_program(chunks):
    key = chunks
    if key not in _CACHE:
        _CACHE[key] = build_program(chunks)
    return _CACHE[key]


def kernel(**inputs) -> np.ndarray:
    packed, per_core, chunks, perm = _host_prep(inputs)
    nc = _get_program(chunks)
    in_maps = []
    for cid in range(NCORES):
        m = {k: np.ascontiguousarray(v) for k, v in packed.items()}
        m["c_feat"] = np.ascontiguousarray(_bf(per_core[cid]["featT"]))
        in_maps.append(m)
    res = run_bass_kernel_spmd(nc, in_maps, core_ids=list(range(NCORES)))
    out_sorted = np.concatenate(
        [res.results[cid]["out"] for cid in range(NCORES)], axis=0
    )  # [B, A] in sorted-action order
    out = np.empty_like(out_sorted)
    out[:, perm] = out_sorted
    return out.astype(np.float32)
